# revision 1
# baseline (speedup 1.0000x reference)
"""DepletionLSTM Trainium2 kernel.

Self-contained: builds a Bass/Tile kernel for the 2-layer-LSTM network,
shards the batch over 8 NeuronCores (pure data parallelism), runs via
PJRT/axon, returns the full [8192, 30] float32 output.

Strategy (per core, 1024 batch):
- Everything resident in SBUF; no DRAM round-trips for activations.
- Feature-major layout: activations are [H=128 partitions, batch] tiles.
- Input-projection LayerNorm stats are computed in a prepass directly in
  [T=90 partitions, batch] layout using the quadratic-form identity
  sum_h p_h^2 = x^T (W^T W) x + 2 (W^T b)^T x + |b|^2 (F=7 is tiny, so the
  F-contractions are unrolled on the vector engine).  rsqrt is batched into
  a single Sqrt activation so the ACT table never switches inside the loop.
- Per step: x_t is PE-transposed to feature-major and pre-scaled by rstd
  (LN scaling commutes through the projection matmul); the projection plus a
  K=2 rank-2 term (b_in*rstd and -mean*rstd rows) accumulates in PSUM and a
  single DVE copy produces the normalized LSTM input.  Each LSTM layer is 4
  accumulating gate matmul pairs (input + recurrent), 4 sigmoid/tanh ACT ops
  with the gate bias folded into the activation bias, tanh(c), and 4 DVE
  elementwise ops.  Layer 1 runs one timestep behind layer 0 (double-buffered
  h0) so both layers' engine work overlaps.
- Matmul operands use float32r (fp32 bytes, single-pass PE) for speed.

PSUM (8 banks): "pg" gates/head 2x[128,1024] (4), "pp" projection [128,1024]
(2), "pxt" x-transposes 2x[7,512] (2).
"""
import sys
sys.path.insert(0, '/opt/trn_rl_repo')

import numpy as np

B, T, F, H, D1, D2, OUT = 8192, 90, 7, 128, 128, 64, 30
NCORES = 8
BL = B // NCORES
G4 = 4 * H
NH = BL // 512
QB = BL // 128
EPS = 1e-5
MMDT = "float32r"
V_ON_POOL = False
XFMR_ON_POOL = False
PGBUFS = 2


def _build(nc, T_steps=T, mmdt_name=MMDT, dbg=False):
    global V_ON_POOL, XFMR_ON_POOL, PGBUFS
    import concourse.tile as tile
    from concourse import mybir
    from concourse.masks import make_identity

    f32 = mybir.dt.float32
    mmdt = getattr(mybir.dt, mmdt_name)
    AF = mybir.ActivationFunctionType
    ALU = mybir.AluOpType

    # ---------------- DRAM I/O ----------------
    x_d = nc.dram_tensor("x", [BL, T, F], f32, kind="ExternalInput")
    W_in_d = nc.dram_tensor("W_in", [H, F], f32, kind="ExternalInput")
    b_in_d = nc.dram_tensor("b_in", [H], f32, kind="ExternalInput")
    g_in_d = nc.dram_tensor("g_in", [H], f32, kind="ExternalInput")
    be_in_d = nc.dram_tensor("be_in", [H], f32, kind="ExternalInput")
    Wih_d = [nc.dram_tensor("Wih0", [G4, H], f32, kind="ExternalInput"),
             nc.dram_tensor("Wih1", [G4, H], f32, kind="ExternalInput")]
    Whh_d = [nc.dram_tensor("Whh0", [G4, H], f32, kind="ExternalInput"),
             nc.dram_tensor("Whh1", [G4, H], f32, kind="ExternalInput")]
    bih_d = [nc.dram_tensor("bih0", [G4], f32, kind="ExternalInput"),
             nc.dram_tensor("bih1", [G4], f32, kind="ExternalInput")]
    bhh_d = [nc.dram_tensor("bhh0", [G4], f32, kind="ExternalInput"),
             nc.dram_tensor("bhh1", [G4], f32, kind="ExternalInput")]
    g_ln_d = nc.dram_tensor("g_ln", [H], f32, kind="ExternalInput")
    be_ln_d = nc.dram_tensor("be_ln", [H], f32, kind="ExternalInput")
    W_d1_d = nc.dram_tensor("W_d1", [D1, H], f32, kind="ExternalInput")
    b_d1_d = nc.dram_tensor("b_d1", [D1], f32, kind="ExternalInput")
    W_d2_d = nc.dram_tensor("W_d2", [D2, D1], f32, kind="ExternalInput")
    b_d2_d = nc.dram_tensor("b_d2", [D2], f32, kind="ExternalInput")
    W_d3_d = nc.dram_tensor("W_d3", [OUT, D2], f32, kind="ExternalInput")
    b_d3_d = nc.dram_tensor("b_d3", [OUT], f32, kind="ExternalInput")
    out_d = nc.dram_tensor("out", [BL, OUT], f32, kind="ExternalOutput")
    if dbg:
        dbg_xfm = nc.dram_tensor("dbg_xfm", [F, BL], f32, kind="ExternalOutput")
        dbg_stats = nc.dram_tensor("dbg_stats", [2, BL], f32, kind="ExternalOutput")
        dbg_x0 = nc.dram_tensor("dbg_x0", [H, BL], f32, kind="ExternalOutput")
        dbg_h0 = nc.dram_tensor("dbg_h0", [H, BL], f32, kind="ExternalOutput")
        dbg_c0 = nc.dram_tensor("dbg_c0", [H, BL], f32, kind="ExternalOutput")
        dbg_pp = nc.dram_tensor("dbg_pp", [H, BL], f32, kind="ExternalOutput")
        dbg_rbc = nc.dram_tensor("dbg_rbc", [2, BL], f32, kind="ExternalOutput")

    import contextlib
    with tile.TileContext(nc) as tc, contextlib.ExitStack() as ctx:
        singles = ctx.enter_context(tc.tile_pool(name="singles", bufs=1))
        trans = ctx.enter_context(tc.tile_pool(name="trans", bufs=2))
        small = ctx.enter_context(tc.tile_pool(name="small", bufs=2))
        ps_pg = ctx.enter_context(tc.tile_pool(name="ps_pg", bufs=PGBUFS, space="PSUM"))
        ps_pp = ctx.enter_context(tc.tile_pool(name="ps_pp", bufs=1, space="PSUM"))
        ps_px = ctx.enter_context(tc.tile_pool(name="ps_px", bufs=2, space="PSUM"))
        dpool = ctx.enter_context(tc.tile_pool(name="dpool", bufs=1, space="DRAM"))

        def pg_tile(shape, name):
            return ps_pg.tile(shape, f32, tag="pg", name=name)

        def pp_tile(shape, name):
            return ps_pp.tile(shape, f32, tag="pp", name=name)

        def px_tile(shape, name):
            return ps_px.tile(shape, f32, tag="pxt", name=name)

        def R(ap):
            return ap

        # ---------------- constants ----------------
        ident = singles.tile([128, 128], f32)
        make_identity(nc, ident)
        ones_row = singles.tile([1, 512], f32)
        nc.vector.memset(ones_row, 1.0)
        ones_col = singles.tile([128, 1], f32)
        nc.vector.memset(ones_col, 1.0)
        eps_col = singles.tile([T, 1], f32)
        nc.vector.memset(eps_col, EPS)

        def load_col(dram_vec, n, name):
            t_ = singles.tile([n, 1], f32, name=name, tag=name)
            nc.sync.dma_start(out=t_, in_=dram_vec[:].rearrange("(p o) -> p o", o=1))
            return t_

        g_in_c = load_col(g_in_d, H, "g_in_c")
        be_in_c = load_col(be_in_d, H, "be_in_c")
        b_in_c = load_col(b_in_d, H, "b_in_c")
        g_ln_c = load_col(g_ln_d, H, "g_ln_c")
        be_ln_c = load_col(be_ln_d, H, "be_ln_c")
        b_d1_c = load_col(b_d1_d, D1, "b_d1_c")
        b_d2_c = load_col(b_d2_d, D2, "b_d2_c")
        b_d3_c = load_col(b_d3_d, OUT, "b_d3_c")
        b_in_row = singles.tile([1, H], f32)
        nc.sync.dma_start(out=b_in_row, in_=b_in_d[:].rearrange("(o p) -> o p", o=1))
        bn1_dram = dpool.tile([2, H], f32)
        nc.sync.dma_start(out=bn1_dram[0:1, :],
                          in_=b_in_d[:].rearrange("(o p) -> o p", o=1))
        nc.sync.dma_start(out=bn1_dram[1:2, :], in_=ones_row[:, 0:H])
        bn1 = singles.tile([2, H], f32)
        nc.sync.dma_start(out=bn1, in_=bn1_dram[:, :])

        # ---------------- weights: load + PE-transpose ----------------
        def transpose_to(dst, src_ap, p, fdim):
            pt = pp_tile([fdim, p], "tr_ps")
            nc.tensor.transpose(pt, src_ap, ident[:p, :p])
            nc.vector.tensor_copy(out=dst, in_=pt)

        w_in_raw = singles.tile([H, F], f32)
        nc.sync.dma_start(out=w_in_raw, in_=W_in_d[:, :])
        w_inT = singles.tile([F, H], mmdt)
        transpose_to(w_inT, w_in_raw, H, F)

        wihT0f = singles.tile([H, 4, H], f32)
        wihT, whhT = [], []
        for L in range(2):
            wt = singles.tile([H, 4, H], mmdt, name=f"wihT{L}", tag=f"wihT{L}")
            ht = singles.tile([H, 4, H], mmdt, name=f"whhT{L}", tag=f"whhT{L}")
            for cc in range(4):
                raw = trans.tile([H, H], f32, tag="u", name="raw")
                nc.sync.dma_start(out=raw, in_=Wih_d[L][cc * H:(cc + 1) * H, :])
                pt_w = pp_tile([H, H], "tr_ps_w")
                nc.tensor.transpose(pt_w, raw, ident)
                nc.vector.tensor_copy(out=wt[:, cc, :], in_=pt_w)
                if L == 0:
                    nc.vector.tensor_copy(out=wihT0f[:, cc, :], in_=pt_w)
                raw2 = trans.tile([H, H], f32, tag="v_", name="raw2")
                nc.sync.dma_start(out=raw2, in_=Whh_d[L][cc * H:(cc + 1) * H, :])
                transpose_to(ht[:, cc, :], raw2, H, H)
            wihT.append(wt)
            whhT.append(ht)

        # gate biases beff[L] [128, 4]; layer-0 gains Wih0 @ be_in (beta fold)
        beff = []
        for L in range(2):
            bt_ = singles.tile([H, 4], f32, name=f"beff{L}", tag=f"beff{L}")
            bih_sb = small.tile([H, 4], f32, tag="bload", name="bih_sb")
            nc.sync.dma_start(out=bih_sb,
                              in_=bih_d[L][:].rearrange("(c p) -> p c", p=H))
            bhh_sb = small.tile([H, 4], f32, tag="bload2", name="bhh_sb")
            nc.sync.dma_start(out=bhh_sb,
                              in_=bhh_d[L][:].rearrange("(c p) -> p c", p=H))
            nc.vector.tensor_add(out=bt_, in0=bih_sb, in1=bhh_sb)
            beff.append(bt_)
        for cc in range(4):
            pb = px_tile([H, 1], "pb")
            nc.tensor.matmul(pb, wihT0f[:, cc, :], be_in_c, start=True, stop=True)
            nc.vector.tensor_add(out=beff[0][:, cc:cc + 1],
                                 in0=beff[0][:, cc:cc + 1], in1=pb)
        # gamma-fold layer-0 input weights (rows scaled by g_in)
        nc.vector.tensor_scalar_mul(
            out=wihT[0][:, :, :].rearrange("p c m -> p (c m)"),
            in0=wihT[0][:, :, :].rearrange("p c m -> p (c m)"),
            scalar1=g_in_c)

        wd1T = singles.tile([H, D1], f32)
        wd1_raw = trans.tile([D1, H], f32, tag="u", name="wd1_raw")
        nc.sync.dma_start(out=wd1_raw, in_=W_d1_d[:, :])
        transpose_to(wd1T, wd1_raw, D1, H)
        wd2T = singles.tile([D1, D2], f32)
        wd2_raw = trans.tile([D2, D1], f32, tag="v_", name="wd2_raw")
        nc.sync.dma_start(out=wd2_raw, in_=W_d2_d[:, :])
        transpose_to(wd2T, wd2_raw, D2, D1)
        wd3T = singles.tile([D2, OUT], f32)
        wd3_raw = trans.tile([OUT, D2], f32, tag="u", name="wd3_raw")
        nc.sync.dma_start(out=wd3_raw, in_=W_d3_d[:, :])
        transpose_to(wd3T, wd3_raw, OUT, D2)

        # ---------------- x loads ----------------
        # loop layout: xrow[p, t, q, f] = x[128q+p, t, f]
        xrow_all = singles.tile([128, T, QB, F], f32)
        nc.sync.dma_start(
            out=xrow_all,
            in_=x_d[:, :, :].rearrange("(q p) t f -> p t q f", p=128))
        # prepass layout: x_tm[t, q, p, f] = x[128q+p, t, f]
        x_tm = singles.tile([T, QB, 128, F], f32)
        nc.sync.dma_start(
            out=x_tm,
            in_=x_d[:, :, :].rearrange("(q p) t f -> t q p f", p=128))

        # ---------------- prepass: LN stats in [T, BL] layout ----------------
        # p' = W_in x + b_in per (h | b,t); over h:
        #   sum p'   = wsum . x + bsum
        #   sum p'^2 = x^T M x + 2 l^T x + c0,  M = W^T W, l = W^T b, c0=|b|^2
        p_m = pp_tile([F, F], "stat_m")
        nc.tensor.matmul(p_m, w_in_raw, w_in_raw, start=True, stop=True)
        p_ws = px_tile([1, F], "stat_ws")
        nc.tensor.matmul(p_ws, ones_col, w_in_raw, start=True, stop=True)
        p_l = px_tile([1, F], "stat_l")
        nc.tensor.matmul(p_l, b_in_c, w_in_raw, start=True, stop=True)
        p_sc = px_tile([1, 2], "stat_sc")
        nc.tensor.matmul(p_sc[:, 0:1], b_in_c, b_in_c, start=True, stop=False,
                         skip_group_check=True)
        nc.tensor.matmul(p_sc[:, 1:2], ones_col, b_in_c, start=False, stop=True,
                         skip_group_check=True)
        m_sb = small.tile([F, F], f32, tag="m_sb", name="m_sb")
        nc.vector.tensor_copy(out=m_sb, in_=p_m)
        ws_sb = small.tile([1, F], f32, tag="ws_sb", name="ws_sb")
        nc.vector.tensor_copy(out=ws_sb, in_=p_ws)
        l_sb = small.tile([1, F], f32, tag="l_sb", name="l_sb")
        nc.vector.tensor_copy(out=l_sb, in_=p_l)
        sc_sb = small.tile([1, 2], f32, tag="sc_sb", name="sc_sb")
        nc.vector.tensor_copy(out=sc_sb, in_=p_sc)
        # stage stat constants to DRAM, then partition-broadcast them back
        stat_dram = dpool.tile([F + 2, F * F], f32)
        nc.sync.dma_start(out=stat_dram[0:1, :].rearrange("o (a b) -> (o a) b", a=F),
                          in_=m_sb)
        nc.sync.dma_start(out=stat_dram[F:F + 1, 0:F], in_=ws_sb)
        nc.sync.dma_start(out=stat_dram[F:F + 1, F:2 * F], in_=l_sb)
        nc.sync.dma_start(out=stat_dram[F + 1:F + 2, 0:2], in_=sc_sb)
        wbc = singles.tile([T, F], f32)
        nc.gpsimd.dma_start(out=wbc, in_=stat_dram[F:F + 1, 0:F].to_broadcast([T, F]))
        lbc = singles.tile([T, F], f32)
        nc.gpsimd.dma_start(out=lbc,
                            in_=stat_dram[F:F + 1, F:2 * F].to_broadcast([T, F]))
        mbc = singles.tile([T, F * F], f32)
        nc.gpsimd.dma_start(out=mbc, in_=stat_dram[0:1, :].to_broadcast([T, F * F]))
        scbc = singles.tile([T, 2], f32)
        nc.gpsimd.dma_start(out=scbc,
                            in_=stat_dram[F + 1:F + 2, 0:2].to_broadcast([T, 2]))

        def xf(fi):
            return x_tm[:T_steps, :, :, fi].rearrange("t q p -> t (q p)")

        TS = T_steps
        nmu_all = singles.tile([T, BL], f32)
        r_all = singles.tile([T, BL], f32)
        acc = trans.tile([T, BL], f32, tag="sig_i", name="st_acc")
        nc.vector.tensor_scalar_mul(out=acc[:TS], in0=xf(0), scalar1=wbc[:TS, 0:1])
        for fi in range(1, F):
            nc.vector.scalar_tensor_tensor(
                out=acc[:TS], in0=xf(fi), scalar=wbc[:TS, fi:fi + 1],
                in1=acc[:TS], op0=ALU.mult, op1=ALU.add)
        # nmu = -(acc + bsum)/H
        nc.vector.tensor_scalar(out=nmu_all[:TS], in0=acc[:TS],
                                scalar1=scbc[:TS, 1:2], scalar2=-1.0 / H,
                                op0=ALU.add, op1=ALU.mult)
        # quadratic form
        qacc = trans.tile([T, BL], f32, tag="sig_f", name="st_qacc")
        yf = trans.tile([T, BL], f32, tag="tg", name="st_yf")
        tmp = trans.tile([T, BL], f32, tag="sig_o", name="st_tmp")
        yf2 = trans.tile([T, BL], f32, tag="sig_o", name="st_yf2")
        qacc2 = trans.tile([T, BL], f32, tag="u", name="st_qacc2")
        tmp2 = trans.tile([T, BL], f32, tag="v_", name="st_tmp2")
        for fi in range(F):
            eng = nc.vector
            y_, q_, t_ = (yf, qacc, tmp) if eng is nc.vector else (yf2, qacc2, tmp2)
            eng.tensor_scalar_mul(out=y_[:TS], in0=xf(0),
                                  scalar1=mbc[:TS, fi * F:fi * F + 1])
            for fj in range(1, F):
                eng.scalar_tensor_tensor(
                    out=y_[:TS], in0=xf(fj),
                    scalar=mbc[:TS, fi * F + fj:fi * F + fj + 1],
                    in1=y_[:TS], op0=ALU.mult, op1=ALU.add)
            eng.tensor_tensor(out=t_[:TS], in0=xf(fi), in1=y_[:TS], op=ALU.mult)
            if fi == 0:
                nc.vector.tensor_copy(out=qacc[:TS], in_=t_[:TS])
            elif fi == 2:
                nc.vector.tensor_copy(out=qacc2[:TS], in_=t_[:TS])
            elif eng is nc.vector:
                nc.vector.tensor_add(out=qacc[:TS], in0=qacc[:TS], in1=t_[:TS])
            else:
                nc.vector.tensor_add(out=qacc2[:TS], in0=qacc2[:TS], in1=t_[:TS])
        nc.vector.tensor_add(out=qacc[:TS], in0=qacc[:TS], in1=qacc2[:TS])
        # + 2 l.x
        lin = trans.tile([T, BL], f32, tag="u", name="st_lin")
        nc.vector.tensor_scalar_mul(out=lin[:TS], in0=xf(0), scalar1=lbc[:TS, 0:1])
        for fi in range(1, F):
            nc.vector.scalar_tensor_tensor(
                out=lin[:TS], in0=xf(fi), scalar=lbc[:TS, fi:fi + 1],
                in1=lin[:TS], op0=ALU.mult, op1=ALU.add)
        nc.vector.scalar_tensor_tensor(out=qacc[:TS], in0=lin[:TS], scalar=2.0,
                                       in1=qacc[:TS], op0=ALU.mult, op1=ALU.add)
        # var = (q + c0)/H - mu^2 ; r = 1/sqrt(var+eps)
        nc.vector.tensor_scalar(out=qacc[:TS], in0=qacc[:TS],
                                scalar1=scbc[:TS, 0:1], scalar2=1.0 / H,
                                op0=ALU.add, op1=ALU.mult)
        nc.vector.tensor_tensor(out=tmp[:TS], in0=nmu_all[:TS], in1=nmu_all[:TS],
                                op=ALU.mult)
        nc.vector.tensor_sub(out=qacc[:TS], in0=qacc[:TS], in1=tmp[:TS])
        nc.scalar.activation(out=r_all[:TS], in_=qacc[:TS], func=AF.Sqrt,
                             bias=eps_col[:TS], scale=1.0)
        nc.vector.reciprocal(out=r_all[:TS], in_=r_all[:TS])
        nmr_all = singles.tile([T, BL], f32)
        nc.vector.tensor_tensor(out=nmr_all[:TS], in0=nmu_all[:TS],
                                in1=r_all[:TS], op=ALU.mult)
        rnm_dram = dpool.tile([2, T, BL], f32)
        nc.sync.dma_start(out=rnm_dram[0, :TS], in_=r_all[:TS])
        nc.sync.dma_start(out=rnm_dram[1, :TS], in_=nmr_all[:TS])
        r_dram = rnm_dram[0]

        # ---------------- states ----------------
        h1 = singles.tile([H, BL], mmdt, name="h1", tag="h1")
        c = [singles.tile([H, BL], f32, name="c0", tag="c0"),
             singles.tile([H, BL], f32, name="c1", tag="c1")]
        zinit = trans.tile([H, BL], f32, tag="x0", name="zinit")
        nc.vector.memset(zinit, 0.0)
        h0_prev = trans.tile([H, BL], mmdt, tag="h0", name="h0_init")
        nc.vector.tensor_copy(out=h0_prev, in_=zinit)
        nc.vector.tensor_copy(out=h1, in_=zinit)
        for L in range(2):
            nc.vector.memset(c[L], 0.0)

        # ---------------- main loop ----------------
        def lstm_step(L, inp, hprev, hout, hh_first):
            sig_i = trans.tile([H, BL], f32, tag="sig_i", name="sig_i")
            sig_f = trans.tile([H, BL], f32, tag="sig_f", name="sig_f")
            tg = trans.tile([H, BL], f32, tag="tg", name="tg")
            sig_o = trans.tile([H, BL], f32, tag="sig_o", name="sig_o")
            outs = [sig_i, sig_f, tg, sig_o]
            funcs = [AF.Sigmoid, AF.Sigmoid, AF.Tanh, AF.Sigmoid]
            for gc in range(4):
                pg = pg_tile([H, BL], "pg_gates")
                for hc in range(NH):
                    sl = slice(hc * 512, (hc + 1) * 512)
                    ops = [(wihT[L][:, gc, :], inp), (whhT[L][:, gc, :], hprev)]
                    if hh_first:
                        ops.reverse()
                    nc.tensor.matmul(pg[:, sl], R(ops[0][0]), R(ops[0][1][:, sl]),
                                     start=True, stop=False)
                    nc.tensor.matmul(pg[:, sl], R(ops[1][0]), R(ops[1][1][:, sl]),
                                     start=False, stop=True)
                nc.scalar.activation(out=outs[gc], in_=pg, func=funcs[gc],
                                     bias=beff[L][:, gc:gc + 1], scale=1.0)
            u = trans.tile([H, BL], f32, tag="u", name="u")
            nc.vector.tensor_tensor(out=u, in0=sig_i, in1=tg, op=ALU.mult)
            v_ = trans.tile([H, BL], f32, tag="v_", name="v_")
            (nc.gpsimd if V_ON_POOL else nc.vector).tensor_tensor(
                out=v_, in0=sig_f, in1=c[L], op=ALU.mult)
            nc.vector.tensor_add(out=c[L], in0=u, in1=v_)
            tc_ = trans.tile([H, BL], f32, tag="tc_", name="tc_")
            nc.scalar.activation(out=tc_, in_=c[L], func=AF.Tanh, scale=1.0)
            nc.vector.tensor_tensor(out=hout, in0=sig_o, in1=tc_, op=ALU.mult)

        for t in range(T_steps):
            # x_t -> feature-major [7, BL] via strided DMA (f-major gather)
            x_fm = trans.tile([F, BL], f32, tag="x_fm", name="x_fm")
            pxs = []
            for half in range(2):
                px = px_tile([F, 512], f"pxt{half}")
                for qi in range(4):
                    q = half * 4 + qi
                    nc.tensor.transpose(
                        px[:, qi * 128:(qi + 1) * 128],
                        xrow_all[:, t, q, :], ident)
                pxs.append(px)
            nc.vector.tensor_copy(out=x_fm[:, 0:512], in_=pxs[0])
            nc.vector.tensor_copy(out=x_fm[:, 512:1024], in_=pxs[1])
            # rstd rows: broadcast over 7 partitions + flat rows for rank-1s
            rbc7 = trans.tile([F, BL], f32, tag="rbc7", name="rbc7")
            nc.gpsimd.dma_start(out=rbc7,
                                in_=r_dram[t:t + 1, :].to_broadcast([F, BL]))
            rn = small.tile([2, BL], f32, tag="rn", name="rn")
            nc.gpsimd.dma_start(out=rn, in_=rnm_dram[:, t, :])
            # x_fm_r = x_fm * rstd (per column)
            x_fm_r = trans.tile([F, BL], mmdt, tag="x_fm_r", name="x_fm_r")
            (nc.gpsimd if XFMR_ON_POOL else nc.vector).tensor_tensor(
                out=x_fm_r, in0=x_fm, in1=rbc7, op=ALU.mult)
            # x0 = W_in @ x_fm_r + b_in x r_row + 1 x nmr_row  (PSUM)
            pp = pp_tile([H, BL], "pp_proj")
            for hc in range(NH):
                sl = slice(hc * 512, (hc + 1) * 512)
                nc.tensor.matmul(pp[:, sl], R(w_inT), R(x_fm_r[:, sl]),
                                 start=True, stop=False, skip_group_check=True)
                nc.tensor.matmul(pp[:, sl], bn1, rn[:, sl],
                                 start=False, stop=(hc == NH - 1),
                                 skip_group_check=True)
            x0 = trans.tile([H, BL], mmdt, tag="x0", name="x0")
            nc.vector.tensor_copy(out=x0, in_=pp)
            # layer 1 runs one step behind layer 0 (consumes h0 of step t-1)
            if t > 0:
                lstm_step(1, h0_prev, h1, h1, hh_first=True)
            h0_new = trans.tile([H, BL], mmdt, tag="h0", name="h0_new")
            lstm_step(0, x0, h0_prev, h0_new, hh_first=False)
            h0_prev = h0_new
            if dbg and t == 0:
                ppc = trans.tile([H, BL], f32, tag="tc_", name="ppc_dbg")
                nc.vector.tensor_copy(out=ppc, in_=pp)
                nc.sync.dma_start(out=dbg_pp[:, :], in_=ppc)
                nc.sync.dma_start(out=dbg_rbc[:, :], in_=rn)
                nc.sync.dma_start(out=dbg_xfm[:, :], in_=x_fm)
                nc.sync.dma_start(out=dbg_stats[0:1, :], in_=nmu_all[0:1, :])
                nc.sync.dma_start(out=dbg_stats[1:2, :], in_=r_all[0:1, :])
                nc.sync.dma_start(out=dbg_x0[:, :], in_=x0.bitcast(f32))
                nc.sync.dma_start(out=dbg_h0[:, :], in_=h0_new.bitcast(f32))
                nc.sync.dma_start(out=dbg_c0[:, :], in_=c[0])
        lstm_step(1, h0_prev, h1, h1, hh_first=True)

        # ---------------- head ----------------
        h1f = trans.tile([H, BL], f32, tag="x0", name="h1f")
        nc.vector.tensor_copy(out=h1f, in_=h1.bitcast(f32))
        sqh = trans.tile([H, BL], f32, tag="sig_f", name="sqh")
        nc.vector.tensor_tensor(out=sqh, in0=h1f, in1=h1f, op=ALU.mult)
        ps_s1 = pp_tile([1, BL], "ps_s1")
        ps_s2 = pp_tile([1, BL], "ps_s2")
        for hc in range(NH):
            sl = slice(hc * 512, (hc + 1) * 512)
            nc.tensor.matmul(ps_s1[:, sl], ones_col, h1f[:, sl],
                             start=True, stop=True, skip_group_check=True)
            nc.tensor.matmul(ps_s2[:, sl], ones_col, sqh[:, sl],
                             start=True, stop=True, skip_group_check=True)
        nmu_h = singles.tile([1, BL], f32, tag="nmu_h", name="nmu_h")
        nc.vector.tensor_scalar_mul(out=nmu_h, in0=ps_s1, scalar1=-1.0 / H)
        musq_h = singles.tile([1, BL], f32, tag="musq", name="musq_h")
        nc.vector.tensor_tensor(out=musq_h, in0=nmu_h, in1=nmu_h, op=ALU.mult)
        v_h = singles.tile([1, BL], f32, tag="v_h", name="v_h")
        nc.vector.tensor_scalar_mul(out=v_h, in0=ps_s2, scalar1=1.0 / H)
        nc.vector.tensor_sub(out=v_h, in0=v_h, in1=musq_h)
        nc.scalar.activation(out=v_h, in_=v_h, func=AF.Sqrt,
                             bias=eps_col[0:1], scale=1.0)
        nc.vector.reciprocal(out=v_h, in_=v_h)
        hstat_dram = dpool.tile([2, BL], f32)
        nc.sync.dma_start(out=hstat_dram[0:1, :], in_=nmu_h)
        nc.sync.dma_start(out=hstat_dram[1:2, :], in_=v_h)
        nmbc = trans.tile([H, BL], f32, tag="u", name="nmbc")
        nc.gpsimd.dma_start(out=nmbc, in_=hstat_dram[0:1, :].to_broadcast([H, BL]))
        rhbc = trans.tile([H, BL], f32, tag="sig_i", name="rhbc")
        nc.gpsimd.dma_start(out=rhbc, in_=hstat_dram[1:2, :].to_broadcast([H, BL]))
        t1 = trans.tile([H, BL], f32, tag="tg", name="t1")
        nc.vector.tensor_tensor(out=t1, in0=h1f, in1=nmbc, op=ALU.add)
        t2 = trans.tile([H, BL], f32, tag="sig_o", name="t2")
        nc.vector.tensor_tensor(out=t2, in0=t1, in1=rhbc, op=ALU.mult)
        last = trans.tile([H, BL], f32, tag="u", name="last")
        nc.vector.tensor_scalar(out=last, in0=t2, scalar1=g_ln_c,
                                scalar2=be_ln_c, op0=ALU.mult, op1=ALU.add)
        pd1 = pg_tile([D1, BL], "pd1")
        for hc in range(NH):
            sl = slice(hc * 512, (hc + 1) * 512)
            nc.tensor.matmul(pd1[:, sl], wd1T, last[:, sl], start=True, stop=True,
                             skip_group_check=True)
        d1 = trans.tile([D1, BL], f32, tag="v_", name="d1")
        nc.scalar.activation(out=d1, in_=pd1, func=AF.Relu, bias=b_d1_c, scale=1.0)
        pd2 = pg_tile([D2, BL], "pd2")
        for hc in range(NH):
            sl = slice(hc * 512, (hc + 1) * 512)
            nc.tensor.matmul(pd2[:, sl], wd2T, d1[:, sl], start=True, stop=True,
                             skip_group_check=True)
        d2 = trans.tile([D2, BL], f32, tag="tc_", name="d2")
        nc.scalar.activation(out=d2, in_=pd2, func=AF.Relu, bias=b_d2_c, scale=1.0)
        pd3 = pg_tile([OUT, BL], "pd3")
        for hc in range(NH):
            sl = slice(hc * 512, (hc + 1) * 512)
            nc.tensor.matmul(pd3[:, sl], wd3T, d2[:, sl], start=True, stop=True,
                             skip_group_check=True)
        o3 = trans.tile([OUT, BL], f32, tag="sig_f", name="o3")
        nc.scalar.activation(out=o3, in_=pd3, func=AF.Identity, bias=b_d3_c,
                             scale=1.0)
        outT = singles.tile([128, QB, OUT], f32)
        for q in range(QB):
            pot = px_tile([128, OUT], "pot")
            nc.tensor.transpose(pot, o3[:, q * 128:(q + 1) * 128],
                                ident[:OUT, :OUT])
            nc.vector.tensor_copy(out=outT[:, q, :], in_=pot)
        nc.sync.dma_start(
            out=out_d[:, :].rearrange("(q p) c -> p q c", p=128),
            in_=outT)
    return nc


_CACHE = {}


def _get_runner():
    if "runner" in _CACHE:
        return _CACHE["runner"]
    import jax
    from jax.sharding import Mesh, PartitionSpec
    from jax.experimental.shard_map import shard_map
    import concourse.bacc as bacc
    import concourse.mybir as mybir
    from concourse.bass2jax import install_neuronx_cc_hook, _bass_exec_p, \
        partition_id_tensor

    nc = bacc.Bacc()
    _build(nc)
    nc.compile()
    install_neuronx_cc_hook()

    partition_name = nc.partition_id_tensor.name if nc.partition_id_tensor else None
    in_names, out_names, out_avals, zero_outs = [], [], [], []
    for alloc in nc.m.functions[0].allocations:
        if not isinstance(alloc, mybir.MemoryLocationSet):
            continue
        name = alloc.memorylocations[0].name
        if alloc.kind == "ExternalInput":
            if name != partition_name:
                in_names.append(name)
        elif alloc.kind == "ExternalOutput":
            out_names.append(name)
            shape = tuple(alloc.tensor_shape)
            dtype = mybir.dt.np(alloc.dtype)
            out_avals.append(jax.core.ShapedArray(shape, dtype))
            zero_outs.append(np.zeros(shape, dtype))
    n_params = len(in_names)
    all_in_names = in_names + out_names + ([partition_name] if partition_name else [])

    def _body(*args):
        operands = list(args)
        if partition_name is not None:
            operands.append(partition_id_tensor())
        outs = _bass_exec_p.bind(
            *operands,
            out_avals=tuple(out_avals),
            in_names=tuple(all_in_names),
            out_names=tuple(out_names),
            lowering_input_output_aliases=(),
            sim_require_finite=False,
            sim_require_nnan=False,
            nc=nc,
        )
        return tuple(outs)

    devices = jax.devices()[:NCORES]
    mesh = Mesh(np.asarray(devices), ("core",))
    in_specs = (PartitionSpec("core"),) * (n_params + len(out_names))
    out_specs = (PartitionSpec("core"),) * len(out_names)
    sharded = jax.jit(
        shard_map(_body, mesh=mesh, in_specs=in_specs, out_specs=out_specs,
                  check_rep=False),
        keep_unused=True)
    _CACHE["runner"] = (sharded, in_names, out_names, zero_outs)
    return _CACHE["runner"]


def kernel(**inputs) -> np.ndarray:
    sharded, in_names, out_names, zero_outs = _get_runner()
    inp = {k: np.ascontiguousarray(np.asarray(v), dtype=np.float32)
           for k, v in inputs.items()}

    def core_val(name, ci):
        if name == "x":
            return inp["x"][ci * BL:(ci + 1) * BL]
        return inp[name]

    concat_in = [
        np.concatenate([core_val(n, ci) for ci in range(NCORES)], axis=0)
        for n in in_names
    ]
    concat_zeros = [
        np.zeros((NCORES * z.shape[0], *z.shape[1:]), z.dtype) for z in zero_outs
    ]
    import jax
    out_arrs = sharded(*concat_in, *concat_zeros)
    jax.block_until_ready(out_arrs)
    oi = out_names.index("out")
    full = np.asarray(out_arrs[oi]).reshape(B, OUT)
    return full.astype(np.float32)



# revision 6
# speedup vs baseline: 1.0219x; 1.0219x over previous
"""DepletionLSTM Trainium2 kernel (v2 — ACT-roof design).

Self-contained: builds a Bass/Tile kernel for the 2-layer-LSTM network,
shards the batch over 8 NeuronCores (pure data parallelism), runs via
PJRT/axon, returns the full [8192, 30] float32 output.

Strategy (per core, 1024 batch):
- Host stages x transposed to [T, F, BL] per core, and folds all
  weight-only expressions (transposes + the fused input-pipeline matrix)
  once in float64 — standard compile-time weight preprocessing.  All
  x-dependent math runs on device.
- The entire input pipeline (W_in projection + LayerNorm + layer-0 input
  matmul + layer-0 gate biases) collapses into ONE K=10 matmul per gate:
    zin0 = wc10^T @ [r*x; r; -mu*r; 1]
  with wc10 rows [A; u; v; beff0], A = Wih0 diag(g_in) W_in [4H x 7],
  u = Wih0 (g_in*b_in), v = Wih0 g_in.
- LN stats (mu, rstd) are computed in a [T=90 part, BL] prepass using a
  Cholesky factorization of the quadratic form:  sum_h p_h^2 = |R x + s|^2
  + const, so the per-row squares run on the otherwise-idle ACT engine and
  the linear chains split across DVE and GPSIMD.
- The augmented input xa = [r*x; r; nmr; 1] (bf16) is staged to DRAM once
  and streamed back per step as a [10, BL] tile (one DMA per step,
  double-buffered).
- Per step per layer: 4 accumulating gate matmul pairs (input + recurrent,
  N=512 chunks; input side bf16, recurrent fp32r), 4 sigmoid/tanh ACT ops
  (bf16 out), tanh(c) ACT, and 3 DVE ops (u=si*tg in bf16 2x-mode, c=u+v,
  h=so*tc) plus v=sf*c on GPSIMD.  Layer 1 runs one timestep behind layer
  0 so both layers' work interleaves; ACT (the only sigmoid/tanh engine)
  is the roofline at ~10.4us/step.
- PSUM: 3 rotating gate tiles [128,1024] (6 banks) + 2 utility banks.
"""
import sys
sys.path.insert(0, '/opt/trn_rl_repo')

import numpy as np

B, T, F, H, D1, D2, OUT = 8192, 90, 7, 128, 128, 64, 30
NCORES = 8
BL = B // NCORES
G4 = 4 * H
NH = BL // 512
QB = BL // 128
EPS = 1e-5
MMDT = "float32r"
KA = F + 3  # augmented-input rows: 7 x-rows, r, nmr, ones
NSC = 44    # stat-constant columns: 28 R + 7 s + 7 wsum' + bsum' + c0''


def _build(nc, T_steps=T, mmdt_name=MMDT, dbg=False):
    import concourse.tile as tile
    from concourse import mybir
    from concourse.masks import make_identity

    f32 = mybir.dt.float32
    bf16 = mybir.dt.bfloat16
    mmdt = getattr(mybir.dt, mmdt_name)
    AF = mybir.ActivationFunctionType
    ALU = mybir.AluOpType

    # ---------------- DRAM I/O (host-folded weights) ----------------
    xT_d = nc.dram_tensor("xT", [T, F, BL], f32, kind="ExternalInput")
    wc10_d = nc.dram_tensor("wc10", [KA, G4], bf16, kind="ExternalInput")
    wih1T_d = nc.dram_tensor("wih1T", [H, G4], f32, kind="ExternalInput")
    whh0T_d = nc.dram_tensor("whh0T", [H, G4], f32, kind="ExternalInput")
    whh1T_d = nc.dram_tensor("whh1T", [H, G4], f32, kind="ExternalInput")
    beff1_d = nc.dram_tensor("beff1", [H, 4], f32, kind="ExternalInput")
    statc_d = nc.dram_tensor("statc", [T, NSC], f32, kind="ExternalInput")
    g_ln_d = nc.dram_tensor("g_ln", [H], f32, kind="ExternalInput")
    be_ln_d = nc.dram_tensor("be_ln", [H], f32, kind="ExternalInput")
    wd1T_d = nc.dram_tensor("wd1T", [H, D1], f32, kind="ExternalInput")
    b_d1_d = nc.dram_tensor("b_d1", [D1], f32, kind="ExternalInput")
    wd2T_d = nc.dram_tensor("wd2T", [D1, D2], f32, kind="ExternalInput")
    b_d2_d = nc.dram_tensor("b_d2", [D2], f32, kind="ExternalInput")
    wd3T_d = nc.dram_tensor("wd3T", [D2, OUT], f32, kind="ExternalInput")
    b_d3_d = nc.dram_tensor("b_d3", [OUT], f32, kind="ExternalInput")
    out_d = nc.dram_tensor("out", [BL, OUT], f32, kind="ExternalOutput")

    import contextlib
    with tile.TileContext(nc) as tc, contextlib.ExitStack() as ctx:
        singles = ctx.enter_context(tc.tile_pool(name="singles", bufs=1))
        prep = ctx.enter_context(tc.tile_pool(name="prep", bufs=1))
        trans = ctx.enter_context(tc.tile_pool(name="trans", bufs=2))
        small = ctx.enter_context(tc.tile_pool(name="small", bufs=2))
        ps_pg = ctx.enter_context(tc.tile_pool(name="ps_pg", bufs=3, space="PSUM"))
        ps_pp = ctx.enter_context(tc.tile_pool(name="ps_pp", bufs=2, space="PSUM"))
        dpool = ctx.enter_context(tc.tile_pool(name="dpool", bufs=1, space="DRAM"))

        def pg_tile(shape, name):
            return ps_pg.tile(shape, f32, tag="pg", name=name)

        def pp_tile(shape, name):
            return ps_pp.tile(shape, f32, tag="pp", name=name)

        # ---------------- constants / weights ----------------
        ident = singles.tile([128, 128], f32)
        make_identity(nc, ident)
        eps_col = singles.tile([T, 1], f32)
        nc.vector.memset(eps_col, EPS)

        def load_col(dram_vec, n, name):
            t_ = singles.tile([n, 1], f32, name=name, tag=name)
            nc.sync.dma_start(out=t_, in_=dram_vec[:].rearrange("(p o) -> p o", o=1))
            return t_

        g_ln_c = load_col(g_ln_d, H, "g_ln_c")
        be_ln_c = load_col(be_ln_d, H, "be_ln_c")
        b_d1_c = load_col(b_d1_d, D1, "b_d1_c")
        b_d2_c = load_col(b_d2_d, D2, "b_d2_c")
        b_d3_c = load_col(b_d3_d, OUT, "b_d3_c")

        wc10 = singles.tile([KA, 4, H], bf16, name="wc10", tag="wc10")
        nc.sync.dma_start(out=wc10,
                          in_=wc10_d[:, :].rearrange("p (c m) -> p c m", c=4))
        wih1T = singles.tile([H, 4, H], f32, name="wih1T", tag="wih1T")
        nc.sync.dma_start(out=wih1T,
                          in_=wih1T_d[:, :].rearrange("p (c m) -> p c m", c=4))
        whhT = []
        for L, d_ in ((0, whh0T_d), (1, whh1T_d)):
            w_ = singles.tile([H, 4, H], f32, name=f"whhT{L}", tag=f"whhT{L}")
            nc.sync.dma_start(out=w_,
                              in_=d_[:, :].rearrange("p (c m) -> p c m", c=4))
            whhT.append(w_)
        beff1 = singles.tile([H, 4], f32, name="beff1", tag="beff1")
        nc.sync.dma_start(out=beff1, in_=beff1_d[:, :])
        statc = singles.tile([T, NSC], f32, name="statc", tag="statc")
        nc.sync.dma_start(out=statc, in_=statc_d[:, :])
        wd1T = singles.tile([H, D1], f32)
        nc.sync.dma_start(out=wd1T, in_=wd1T_d[:, :])
        wd2T = singles.tile([D1, D2], f32)
        nc.sync.dma_start(out=wd2T, in_=wd2T_d[:, :])
        wd3T = singles.tile([D2, OUT], f32)
        nc.sync.dma_start(out=wd3T, in_=wd3T_d[:, :])

        # statc column layout (must match host packing in kernel()):
        #   0..27  : R'_ij rows i=0..6, j=i..6 (upper-tri, row-major)
        #   28..34 : s'_i
        #   35..41 : wsum'_f  (= -wsum_f/H)
        #   42     : bsum'    (= -bsum/H)
        #   43     : c0''     (= (c0-|s|^2)/H)
        _roff = [0, 7, 13, 18, 22, 25, 27]

        def sc(j):
            return statc[:T_steps, j:j + 1]

        # ---------------- x load ([T part, F, BL], contiguous) ----------
        x_ftb = singles.tile([T, F, BL], f32)
        nc.sync.dma_start(out=x_ftb, in_=xT_d[:, :, :])

        def xf(fi):
            return x_ftb[:T_steps, fi, :]

        TS = T_steps

        # ---------------- prepass: LN stats in [T, BL] layout ------------
        # nmu = sum_f wsum'_f x_f + bsum'   (wsum' = -wsum/H)
        nmu_all = singles.tile([T, BL], f32)
        r_all = singles.tile([T, BL], f32)
        nc.vector.tensor_scalar(out=nmu_all[:TS], in0=xf(0), scalar1=sc(35),
                                scalar2=sc(42), op0=ALU.mult, op1=ALU.add)
        for fi in range(1, F):
            nc.vector.scalar_tensor_tensor(
                out=nmu_all[:TS], in0=xf(fi), scalar=sc(35 + fi),
                in1=nmu_all[:TS], op0=ALU.mult, op1=ALU.add)
        # y_i = sum_{j>=i} R'_ij x_j + s'_i ; q/H = sum_i y_i^2 + c0''
        # chains: DVE rows 0-1, GPSIMD rows 2-6; squares on ACT.
        sqs = []
        for i in range(F):
            eng = nc.vector if i < 2 else nc.gpsimd
            z = prep.tile([T, BL], f32, tag=f"stz{i % 4}", name=f"st_z{i}")
            eng.tensor_scalar(out=z[:TS], in0=xf(i),
                              scalar1=sc(_roff[i]), scalar2=sc(28 + i),
                              op0=ALU.mult, op1=ALU.add)
            for j in range(i + 1, F):
                eng.scalar_tensor_tensor(
                    out=z[:TS], in0=xf(j), scalar=sc(_roff[i] + j - i),
                    in1=z[:TS], op0=ALU.mult, op1=ALU.add)
            sq = prep.tile([T, BL], f32, tag=f"stsq{i % 3}", name=f"st_sq{i}")
            nc.scalar.activation(out=sq[:TS], in_=z[:TS], func=AF.Square,
                                 scale=1.0)
            sqs.append(sq)
        qv = prep.tile([T, BL], f32, tag="stqv", name="st_qv")
        nc.vector.tensor_add(out=qv[:TS], in0=sqs[0][:TS], in1=sqs[1][:TS])
        for i in (2, 3):
            nc.vector.tensor_add(out=qv[:TS], in0=qv[:TS], in1=sqs[i][:TS])
        qp = prep.tile([T, BL], f32, tag="stqp", name="st_qp")
        nc.gpsimd.tensor_add(out=qp[:TS], in0=sqs[4][:TS], in1=sqs[5][:TS])
        nc.gpsimd.tensor_add(out=qp[:TS], in0=qp[:TS], in1=sqs[6][:TS])
        # var = q/H + c0'' - mu^2
        musq = prep.tile([T, BL], f32, tag="stz0", name="st_musq")
        nc.gpsimd.tensor_tensor(out=musq[:TS], in0=nmu_all[:TS],
                                in1=nmu_all[:TS], op=ALU.mult)
        nc.vector.tensor_add(out=qv[:TS], in0=qv[:TS], in1=qp[:TS])
        nc.vector.tensor_scalar_add(out=qv[:TS], in0=qv[:TS], scalar1=sc(43))
        nc.vector.tensor_sub(out=qv[:TS], in0=qv[:TS], in1=musq[:TS])
        nc.scalar.activation(out=r_all[:TS], in_=qv[:TS], func=AF.Sqrt,
                             bias=eps_col[:TS], scale=1.0)
        nc.vector.reciprocal(out=r_all[:TS], in_=r_all[:TS])

        # ---------------- augmented input xa = [r*x; r; nmr; 1] ----------
        xa = singles.tile([T, KA, BL], bf16)
        for fi in range(F):
            eng = nc.vector if fi % 2 == 0 else nc.gpsimd
            eng.tensor_tensor(out=xa[:TS, fi, :], in0=xf(fi), in1=r_all[:TS],
                              op=ALU.mult)
        nc.vector.tensor_copy(out=xa[:TS, F, :], in_=r_all[:TS])
        nc.gpsimd.tensor_tensor(out=xa[:TS, F + 1, :], in0=nmu_all[:TS],
                                in1=r_all[:TS], op=ALU.mult)
        nc.vector.memset(xa[:TS, F + 2, :], 1.0)
        xa_dram = dpool.tile([T, KA, BL], bf16)
        nc.sync.dma_start(out=xa_dram[:TS], in_=xa[:TS])

        # ---------------- states ----------------
        h1 = singles.tile([H, BL], mmdt, name="h1", tag="h1")
        c = [singles.tile([H, BL], f32, name="c0", tag="c0"),
             singles.tile([H, BL], f32, name="c1", tag="c1")]
        zinit = trans.tile([H, BL], f32, tag="hf32", name="zinit")
        nc.vector.memset(zinit, 0.0)
        h0_prev = trans.tile([H, BL], mmdt, tag="h0", name="h0_init")
        nc.vector.tensor_copy(out=h0_prev, in_=zinit)
        nc.vector.tensor_copy(out=h1, in_=zinit)
        for L in range(2):
            nc.vector.memset(c[L], 0.0)

        # ---------------- main loop ----------------
        def lstm_step(L, inp, inpT, hprev, hout, hh_first):
            sig_i = trans.tile([H, BL], bf16, tag="sig_i", name="sig_i")
            sig_f = trans.tile([H, BL], bf16, tag="sig_f", name="sig_f")
            tg = trans.tile([H, BL], bf16, tag="tg", name="tg")
            sig_o = trans.tile([H, BL], bf16, tag="sig_o", name="sig_o")
            outs = [sig_i, sig_f, tg, sig_o]
            funcs = [AF.Sigmoid, AF.Sigmoid, AF.Tanh, AF.Sigmoid]
            for gc in range(4):
                pg = pg_tile([H, BL], "pg_gates")
                for hc in range(NH):
                    sl = slice(hc * 512, (hc + 1) * 512)
                    ops = [(inpT[:, gc, :], inp),
                           (whhT[L][:, gc, :].bitcast(mmdt), hprev)]
                    if hh_first:
                        ops.reverse()
                    nc.tensor.matmul(pg[:, sl], ops[0][0], ops[0][1][:, sl],
                                     start=True, stop=False)
                    nc.tensor.matmul(pg[:, sl], ops[1][0], ops[1][1][:, sl],
                                     start=False, stop=True)
                if L == 0:
                    nc.scalar.activation(out=outs[gc], in_=pg, func=funcs[gc],
                                         scale=1.0)
                else:
                    nc.scalar.activation(out=outs[gc], in_=pg, func=funcs[gc],
                                         bias=beff1[:, gc:gc + 1], scale=1.0)
            u = trans.tile([H, BL], bf16, tag="u", name="u")
            nc.vector.tensor_tensor(out=u, in0=sig_i, in1=tg, op=ALU.mult)
            v_ = trans.tile([H, BL], f32, tag="v_", name="v_")
            nc.gpsimd.tensor_tensor(out=v_, in0=sig_f, in1=c[L], op=ALU.mult)
            nc.vector.tensor_add(out=c[L], in0=u, in1=v_)
            tc_ = trans.tile([H, BL], bf16, tag="tc_", name="tc_")
            nc.scalar.activation(out=tc_, in_=c[L], func=AF.Tanh, scale=1.0)
            nc.vector.tensor_tensor(out=hout, in0=sig_o, in1=tc_, op=ALU.mult)

        for t in range(T_steps):
            xaug = trans.tile([KA, BL], bf16, tag="xaug", name="xaug")
            nc.sync.dma_start(out=xaug, in_=xa_dram[t])
            # layer 1 runs one step behind layer 0 (consumes h0 of step t-1)
            if t > 0:
                lstm_step(1, h0_prev, wih1T.bitcast(mmdt), h1, h1,
                          hh_first=True)
            h0_new = trans.tile([H, BL], mmdt, tag="h0", name="h0_new")
            lstm_step(0, xaug, wc10, h0_prev, h0_new, hh_first=False)
            h0_prev = h0_new
        lstm_step(1, h0_prev, wih1T.bitcast(mmdt), h1, h1, hh_first=True)

        # ---------------- head ----------------
        h1f = trans.tile([H, BL], f32, tag="hf32", name="h1f")
        nc.vector.tensor_copy(out=h1f, in_=h1.bitcast(f32))
        sqh = prep.tile([H, BL], f32, tag="ha", name="sqh")
        nc.vector.tensor_tensor(out=sqh, in0=h1f, in1=h1f, op=ALU.mult)
        ones_col = small.tile([H, 1], f32, tag="ones_col", name="ones_col")
        nc.vector.memset(ones_col, 1.0)
        ps_s1 = pg_tile([1, BL], "ps_s1")
        ps_s2 = pg_tile([1, BL], "ps_s2")
        for hc in range(NH):
            sl = slice(hc * 512, (hc + 1) * 512)
            nc.tensor.matmul(ps_s1[:, sl], ones_col, h1f[:, sl],
                             start=True, stop=True, skip_group_check=True)
            nc.tensor.matmul(ps_s2[:, sl], ones_col, sqh[:, sl],
                             start=True, stop=True, skip_group_check=True)
        nmu_h = small.tile([1, BL], f32, tag="nmu_h", name="nmu_h")
        nc.vector.tensor_scalar_mul(out=nmu_h, in0=ps_s1, scalar1=-1.0 / H)
        musq_h = small.tile([1, BL], f32, tag="musq", name="musq_h")
        nc.vector.tensor_tensor(out=musq_h, in0=nmu_h, in1=nmu_h, op=ALU.mult)
        v_h = small.tile([1, BL], f32, tag="v_h", name="v_h")
        nc.vector.tensor_scalar_mul(out=v_h, in0=ps_s2, scalar1=1.0 / H)
        nc.vector.tensor_sub(out=v_h, in0=v_h, in1=musq_h)
        nc.scalar.activation(out=v_h, in_=v_h, func=AF.Sqrt,
                             bias=eps_col[0:1], scale=1.0)
        nc.vector.reciprocal(out=v_h, in_=v_h)
        hstat_dram = dpool.tile([2, BL], f32)
        nc.sync.dma_start(out=hstat_dram[0:1, :], in_=nmu_h)
        nc.sync.dma_start(out=hstat_dram[1:2, :], in_=v_h)
        nmbc = prep.tile([H, BL], f32, tag="hb", name="nmbc")
        nc.gpsimd.dma_start(out=nmbc, in_=hstat_dram[0:1, :].to_broadcast([H, BL]))
        rhbc = prep.tile([H, BL], f32, tag="hc", name="rhbc")
        nc.gpsimd.dma_start(out=rhbc, in_=hstat_dram[1:2, :].to_broadcast([H, BL]))
        t1 = prep.tile([H, BL], f32, tag="hd", name="t1")
        nc.vector.tensor_tensor(out=t1, in0=h1f, in1=nmbc, op=ALU.add)
        t2 = prep.tile([H, BL], f32, tag="ha", name="t2")
        nc.vector.tensor_tensor(out=t2, in0=t1, in1=rhbc, op=ALU.mult)
        last = prep.tile([H, BL], f32, tag="hb", name="last")
        nc.vector.tensor_scalar(out=last, in0=t2, scalar1=g_ln_c,
                                scalar2=be_ln_c, op0=ALU.mult, op1=ALU.add)
        pd1 = pg_tile([D1, BL], "pd1")
        for hc in range(NH):
            sl = slice(hc * 512, (hc + 1) * 512)
            nc.tensor.matmul(pd1[:, sl], wd1T, last[:, sl], start=True, stop=True,
                             skip_group_check=True)
        d1 = prep.tile([D1, BL], f32, tag="hc", name="d1")
        nc.scalar.activation(out=d1, in_=pd1, func=AF.Relu, bias=b_d1_c, scale=1.0)
        pd2 = pg_tile([D2, BL], "pd2")
        for hc in range(NH):
            sl = slice(hc * 512, (hc + 1) * 512)
            nc.tensor.matmul(pd2[:, sl], wd2T, d1[:, sl], start=True, stop=True,
                             skip_group_check=True)
        d2 = prep.tile([D2, BL], f32, tag="hd", name="d2")
        nc.scalar.activation(out=d2, in_=pd2, func=AF.Relu, bias=b_d2_c, scale=1.0)
        pd3 = pg_tile([OUT, BL], "pd3")
        for hc in range(NH):
            sl = slice(hc * 512, (hc + 1) * 512)
            nc.tensor.matmul(pd3[:, sl], wd3T, d2[:, sl], start=True, stop=True,
                             skip_group_check=True)
        o3 = prep.tile([OUT, BL], f32, tag="ha", name="o3")
        nc.scalar.activation(out=o3, in_=pd3, func=AF.Identity, bias=b_d3_c,
                             scale=1.0)
        outT = singles.tile([128, QB, OUT], f32)
        for q in range(QB):
            pot = pp_tile([128, OUT], "pot")
            nc.tensor.transpose(pot, o3[:, q * 128:(q + 1) * 128],
                                ident[:OUT, :OUT])
            nc.vector.tensor_copy(out=outT[:, q, :], in_=pot)
        nc.sync.dma_start(
            out=out_d[:, :].rearrange("(q p) c -> p q c", p=128),
            in_=outT)
    return nc


_CACHE = {}


def _fold_weights(inp):
    """Host-side weight-only preprocessing (float64). Returns the dict of
    derived dram inputs (excluding xT, which is per-core)."""
    import ml_dtypes
    d = {k: np.asarray(v, np.float64) for k, v in inp.items()}
    W = d["W_in"]                       # [H, F]
    g, b, be = d["g_in"], d["b_in"], d["be_in"]
    Wih0, Whh0 = d["Wih0"], d["Whh0"]   # [4H, H]
    Wih1, Whh1 = d["Wih1"], d["Whh1"]

    # wc10 rows: A = Wih0 diag(g) W, u = Wih0 (g*b), v = Wih0 g,
    #            beff0 = bih0 + bhh0 + Wih0 be
    Wg = Wih0 * g[None, :]              # [4H, H] (columns scaled)
    A = Wg @ W                          # [4H, F]
    u = Wg @ b
    v = Wg @ np.ones(H)
    beff0 = d["bih0"] + d["bhh0"] + Wih0 @ be
    wc10 = np.concatenate([A.T, u[None], v[None], beff0[None]], axis=0)  # [10, 4H]

    # stats constants: M = W^T W, wsum = 1^T W, l = W^T b, c0 = |b|^2
    M = W.T @ W
    wsum = W.sum(axis=0)
    l = W.T @ b
    c0 = float(b @ b)
    R = np.linalg.cholesky(M).T         # upper-tri: M = R^T R
    s = np.linalg.solve(R.T, l)         # R^T s = l
    sH = np.sqrt(float(H))
    Rp, sp = R / sH, s / sH
    bsum = float(b.sum())
    cols = []
    for i in range(F):
        cols.extend(Rp[i, i:])          # 28 upper-tri entries
    cols += list(sp)                    # 7 s'
    cols += list(-wsum / H)             # 7 wsum'
    cols += [-bsum / H, (c0 - float(s @ s)) / H]
    statc_row = np.asarray(cols, np.float64)
    assert statc_row.shape[0] == NSC
    statc = np.tile(statc_row[None, :], (T, 1))

    beff1 = (d["bih1"] + d["bhh1"]).reshape(4, H).T  # [H, 4]

    out = {
        "wc10": wc10.astype(ml_dtypes.bfloat16),
        "wih1T": np.ascontiguousarray(Wih1.T).astype(np.float32),
        "whh0T": np.ascontiguousarray(Whh0.T).astype(np.float32),
        "whh1T": np.ascontiguousarray(Whh1.T).astype(np.float32),
        "beff1": np.ascontiguousarray(beff1).astype(np.float32),
        "statc": statc.astype(np.float32),
        "g_ln": d["g_ln"].astype(np.float32),
        "be_ln": d["be_ln"].astype(np.float32),
        "wd1T": np.ascontiguousarray(d["W_d1"].T).astype(np.float32),
        "b_d1": d["b_d1"].astype(np.float32),
        "wd2T": np.ascontiguousarray(d["W_d2"].T).astype(np.float32),
        "b_d2": d["b_d2"].astype(np.float32),
        "wd3T": np.ascontiguousarray(d["W_d3"].T).astype(np.float32),
        "b_d3": d["b_d3"].astype(np.float32),
    }
    return out


def core_val(inp, name, ci, folded=None):
    """Per-core value for dram input `name` (inp: full raw-input dict)."""
    if name == "xT":
        return np.ascontiguousarray(
            np.asarray(inp["x"], np.float32)[ci * BL:(ci + 1) * BL]
            .transpose(1, 2, 0))
    if folded is None:
        folded = _fold_weights(inp)
    return folded[name]


def _get_runner():
    if "runner" in _CACHE:
        return _CACHE["runner"]
    import jax
    from jax.sharding import Mesh, PartitionSpec
    from jax.experimental.shard_map import shard_map
    import concourse.bacc as bacc
    import concourse.mybir as mybir
    from concourse.bass2jax import install_neuronx_cc_hook, _bass_exec_p, \
        partition_id_tensor

    nc = bacc.Bacc()
    _build(nc)
    nc.compile()
    install_neuronx_cc_hook()

    partition_name = nc.partition_id_tensor.name if nc.partition_id_tensor else None
    in_names, out_names, out_avals, zero_outs = [], [], [], []
    for alloc in nc.m.functions[0].allocations:
        if not isinstance(alloc, mybir.MemoryLocationSet):
            continue
        name = alloc.memorylocations[0].name
        if alloc.kind == "ExternalInput":
            if name != partition_name:
                in_names.append(name)
        elif alloc.kind == "ExternalOutput":
            out_names.append(name)
            shape = tuple(alloc.tensor_shape)
            dtype = mybir.dt.np(alloc.dtype)
            out_avals.append(jax.core.ShapedArray(shape, dtype))
            zero_outs.append(np.zeros(shape, dtype))
    n_params = len(in_names)
    all_in_names = in_names + out_names + ([partition_name] if partition_name else [])

    def _body(*args):
        operands = list(args)
        if partition_name is not None:
            operands.append(partition_id_tensor())
        outs = _bass_exec_p.bind(
            *operands,
            out_avals=tuple(out_avals),
            in_names=tuple(all_in_names),
            out_names=tuple(out_names),
            lowering_input_output_aliases=(),
            sim_require_finite=False,
            sim_require_nnan=False,
            nc=nc,
        )
        return tuple(outs)

    devices = jax.devices()[:NCORES]
    mesh = Mesh(np.asarray(devices), ("core",))
    in_specs = (PartitionSpec("core"),) * (n_params + len(out_names))
    out_specs = (PartitionSpec("core"),) * len(out_names)
    sharded = jax.jit(
        shard_map(_body, mesh=mesh, in_specs=in_specs, out_specs=out_specs,
                  check_rep=False),
        keep_unused=True)
    _CACHE["runner"] = (sharded, in_names, out_names, zero_outs)
    return _CACHE["runner"]


def kernel(**inputs) -> np.ndarray:
    sharded, in_names, out_names, zero_outs = _get_runner()
    inp = {k: np.asarray(v) for k, v in inputs.items()}
    folded = _fold_weights(inp)

    concat_in = [
        np.concatenate([core_val(inp, n, ci, folded) for ci in range(NCORES)],
                       axis=0)
        for n in in_names
    ]
    concat_zeros = [
        np.zeros((NCORES * z.shape[0], *z.shape[1:]), z.dtype) for z in zero_outs
    ]
    import jax
    out_arrs = sharded(*concat_in, *concat_zeros)
    jax.block_until_ready(out_arrs)
    oi = out_names.index("out")
    full = np.asarray(out_arrs[oi]).reshape(B, OUT)
    return full.astype(np.float32)


# revision 11
# speedup vs baseline: 1.5187x; 1.4862x over previous
"""DepletionLSTM Trainium2 kernel (v2 — ACT-roof design).

Self-contained: builds a Bass/Tile kernel for the 2-layer-LSTM network,
shards the batch over 8 NeuronCores (pure data parallelism), runs via
PJRT/axon, returns the full [8192, 30] float32 output.

Strategy (per core, 1024 batch):
- Host stages x transposed to [T, F, BL] per core, and folds all
  weight-only expressions (transposes + the fused input-pipeline matrix)
  once in float64 — standard compile-time weight preprocessing.  All
  x-dependent math runs on device.
- The entire input pipeline (W_in projection + LayerNorm + layer-0 input
  matmul + layer-0 gate biases) collapses into ONE K=10 matmul per gate:
    zin0 = wc10^T @ [r*x; r; -mu*r; 1]
  with wc10 rows [A; u; v; beff0], A = Wih0 diag(g_in) W_in [4H x 7],
  u = Wih0 (g_in*b_in), v = Wih0 g_in.
- LN stats (mu, rstd) are computed in a [T=90 part, BL] prepass using a
  Cholesky factorization of the quadratic form:  sum_h p_h^2 = |R x + s|^2
  + const, so the per-row squares run on the otherwise-idle ACT engine and
  the linear chains split across DVE and GPSIMD.
- The augmented input xa = [r*x; r; nmr; 1] (bf16) is staged to DRAM once
  and streamed back per step as a [10, BL] tile (one DMA per step,
  double-buffered).
- Per step per layer: 4 accumulating gate matmul pairs (input + recurrent,
  N=512 chunks; input side bf16, recurrent fp32r), 4 sigmoid/tanh ACT ops
  (bf16 out), tanh(c) ACT, and 3 DVE ops (u=si*tg in bf16 2x-mode, c=u+v,
  h=so*tc) plus v=sf*c on GPSIMD.  Layer 1 runs one timestep behind layer
  0 so both layers' work interleaves; ACT (the only sigmoid/tanh engine)
  is the roofline at ~10.4us/step.
- PSUM: 3 rotating gate tiles [128,1024] (6 banks) + 2 utility banks.
"""
import sys
sys.path.insert(0, '/opt/trn_rl_repo')

import numpy as np

B, T, F, H, D1, D2, OUT = 8192, 90, 7, 128, 128, 64, 30
NCORES = 8
BL = B // NCORES
G4 = 4 * H
NH = BL // 512
QB = BL // 128
EPS = 1e-5
MMDT = "float32r"
KA = F + 3  # augmented-input rows: 7 x-rows, r, nmr, ones
NSC = 44    # stat-constant columns: 28 R + 7 s + 7 wsum' + bsum' + c0''


def _build(nc, T_steps=T, mmdt_name=MMDT, dbg=False):
    import concourse.tile as tile
    from concourse import mybir
    from concourse.masks import make_identity

    f32 = mybir.dt.float32
    bf16 = mybir.dt.bfloat16
    mmdt = getattr(mybir.dt, mmdt_name)
    AF = mybir.ActivationFunctionType
    ALU = mybir.AluOpType

    # ---------------- DRAM I/O (host-folded weights) ----------------
    xT_d = nc.dram_tensor("xT", [T, F, BL], f32, kind="ExternalInput")
    wc10_d = nc.dram_tensor("wc10", [KA, G4], bf16, kind="ExternalInput")
    wih1T_d = nc.dram_tensor("wih1T", [H, G4], f32, kind="ExternalInput")
    whh0T_d = nc.dram_tensor("whh0T", [H, G4], f32, kind="ExternalInput")
    whh1T_d = nc.dram_tensor("whh1T", [H, G4], f32, kind="ExternalInput")
    beff1_d = nc.dram_tensor("beff1", [H, 4], f32, kind="ExternalInput")
    statc_d = nc.dram_tensor("statc", [T, NSC], f32, kind="ExternalInput")
    g_ln_d = nc.dram_tensor("g_ln", [H], f32, kind="ExternalInput")
    be_ln_d = nc.dram_tensor("be_ln", [H], f32, kind="ExternalInput")
    wd1T_d = nc.dram_tensor("wd1T", [H, D1], f32, kind="ExternalInput")
    b_d1_d = nc.dram_tensor("b_d1", [D1], f32, kind="ExternalInput")
    wd2T_d = nc.dram_tensor("wd2T", [D1, D2], f32, kind="ExternalInput")
    b_d2_d = nc.dram_tensor("b_d2", [D2], f32, kind="ExternalInput")
    wd3T_d = nc.dram_tensor("wd3T", [D2, OUT], f32, kind="ExternalInput")
    b_d3_d = nc.dram_tensor("b_d3", [OUT], f32, kind="ExternalInput")
    out_d = nc.dram_tensor("out", [BL, OUT], f32, kind="ExternalOutput")

    import contextlib
    with tile.TileContext(nc) as tc, contextlib.ExitStack() as ctx:
        singles = ctx.enter_context(tc.tile_pool(name="singles", bufs=1))
        prep = ctx.enter_context(tc.tile_pool(name="prep", bufs=1))
        trans = ctx.enter_context(tc.tile_pool(name="trans", bufs=1))
        dbuf = ctx.enter_context(tc.tile_pool(name="dbuf", bufs=2))
        small = ctx.enter_context(tc.tile_pool(name="small", bufs=2))
        ps_pg = ctx.enter_context(tc.tile_pool(name="ps_pg", bufs=3, space="PSUM"))
        ps_pp = ctx.enter_context(tc.tile_pool(name="ps_pp", bufs=2, space="PSUM"))
        dpool = ctx.enter_context(tc.tile_pool(name="dpool", bufs=1, space="DRAM"))

        def pg_tile(shape, name):
            return ps_pg.tile(shape, f32, tag="pg", name=name)

        def pp_tile(shape, name):
            return ps_pp.tile(shape, f32, tag="pp", name=name)

        # ---------------- constants / weights ----------------
        ident = singles.tile([128, 128], f32)
        make_identity(nc, ident)
        eps_col = singles.tile([T, 1], f32)
        nc.vector.memset(eps_col, EPS)

        def load_col(dram_vec, n, name):
            t_ = singles.tile([n, 1], f32, name=name, tag=name)
            nc.sync.dma_start(out=t_, in_=dram_vec[:].rearrange("(p o) -> p o", o=1))
            return t_

        g_ln_c = load_col(g_ln_d, H, "g_ln_c")
        be_ln_c = load_col(be_ln_d, H, "be_ln_c")
        b_d1_c = load_col(b_d1_d, D1, "b_d1_c")
        b_d2_c = load_col(b_d2_d, D2, "b_d2_c")
        b_d3_c = load_col(b_d3_d, OUT, "b_d3_c")

        wc10 = singles.tile([KA, 4, H], bf16, name="wc10", tag="wc10")
        nc.sync.dma_start(out=wc10,
                          in_=wc10_d[:, :].rearrange("p (c m) -> p c m", c=4))
        wih1T = singles.tile([H, 4, H], f32, name="wih1T", tag="wih1T")
        nc.sync.dma_start(out=wih1T,
                          in_=wih1T_d[:, :].rearrange("p (c m) -> p c m", c=4))
        whhT = []
        for L, d_ in ((0, whh0T_d), (1, whh1T_d)):
            w_ = singles.tile([H, 4, H], f32, name=f"whhT{L}", tag=f"whhT{L}")
            nc.sync.dma_start(out=w_,
                              in_=d_[:, :].rearrange("p (c m) -> p c m", c=4))
            whhT.append(w_)
        beff1 = singles.tile([H, 4], f32, name="beff1", tag="beff1")
        nc.sync.dma_start(out=beff1, in_=beff1_d[:, :])
        statc = singles.tile([T, NSC], f32, name="statc", tag="statc")
        nc.sync.dma_start(out=statc, in_=statc_d[:, :])
        wd1T = singles.tile([H, D1], f32)
        nc.sync.dma_start(out=wd1T, in_=wd1T_d[:, :])
        wd2T = singles.tile([D1, D2], f32)
        nc.sync.dma_start(out=wd2T, in_=wd2T_d[:, :])
        wd3T = singles.tile([D2, OUT], f32)
        nc.sync.dma_start(out=wd3T, in_=wd3T_d[:, :])

        # statc column layout (must match host packing in kernel()):
        #   0..27  : R'_ij rows i=0..6, j=i..6 (upper-tri, row-major)
        #   28..34 : s'_i
        #   35..41 : wsum'_f  (= -wsum_f/H)
        #   42     : bsum'    (= -bsum/H)
        #   43     : c0''     (= (c0-|s|^2)/H)
        _roff = [0, 7, 13, 18, 22, 25, 27]

        def sc(j):
            return statc[:T_steps, j:j + 1]

        # ---------------- x load ([T part, F, BL], contiguous) ----------
        x_ftb = singles.tile([T, F, BL], f32)
        nc.sync.dma_start(out=x_ftb, in_=xT_d[:, :, :])

        def xf(fi):
            return x_ftb[:T_steps, fi, :]

        TS = T_steps

        # ---------------- prepass: LN stats in [T, BL] layout ------------
        # nmu = sum_f wsum'_f x_f + bsum'   (wsum' = -wsum/H)
        nmu_all = singles.tile([T, BL], f32)
        r_all = singles.tile([T, BL], f32)
        nc.vector.tensor_scalar(out=nmu_all[:TS], in0=xf(0), scalar1=sc(35),
                                scalar2=sc(42), op0=ALU.mult, op1=ALU.add)
        for fi in range(1, F):
            nc.vector.scalar_tensor_tensor(
                out=nmu_all[:TS], in0=xf(fi), scalar=sc(35 + fi),
                in1=nmu_all[:TS], op0=ALU.mult, op1=ALU.add)
        # y_i = sum_{j>=i} R'_ij x_j + s'_i ; q/H = sum_i y_i^2 + c0''
        # chains: DVE rows 0-1, GPSIMD rows 2-6; squares on ACT.
        sqs = []
        for i in range(F):
            eng = nc.vector if i < 2 else nc.gpsimd
            z = prep.tile([T, BL], f32, tag=f"stz{i % 4}", name=f"st_z{i}")
            eng.tensor_scalar(out=z[:TS], in0=xf(i),
                              scalar1=sc(_roff[i]), scalar2=sc(28 + i),
                              op0=ALU.mult, op1=ALU.add)
            for j in range(i + 1, F):
                eng.scalar_tensor_tensor(
                    out=z[:TS], in0=xf(j), scalar=sc(_roff[i] + j - i),
                    in1=z[:TS], op0=ALU.mult, op1=ALU.add)
            sq = prep.tile([T, BL], f32, tag=f"stsq{i % 3}", name=f"st_sq{i}")
            nc.scalar.activation(out=sq[:TS], in_=z[:TS], func=AF.Square,
                                 scale=1.0)
            sqs.append(sq)
        qv = prep.tile([T, BL], f32, tag="stqv", name="st_qv")
        nc.vector.tensor_add(out=qv[:TS], in0=sqs[0][:TS], in1=sqs[1][:TS])
        for i in (2, 3):
            nc.vector.tensor_add(out=qv[:TS], in0=qv[:TS], in1=sqs[i][:TS])
        qp = prep.tile([T, BL], f32, tag="stqp", name="st_qp")
        nc.gpsimd.tensor_add(out=qp[:TS], in0=sqs[4][:TS], in1=sqs[5][:TS])
        nc.gpsimd.tensor_add(out=qp[:TS], in0=qp[:TS], in1=sqs[6][:TS])
        # var = q/H + c0'' - mu^2
        musq = prep.tile([T, BL], f32, tag="stz0", name="st_musq")
        nc.gpsimd.tensor_tensor(out=musq[:TS], in0=nmu_all[:TS],
                                in1=nmu_all[:TS], op=ALU.mult)
        nc.vector.tensor_add(out=qv[:TS], in0=qv[:TS], in1=qp[:TS])
        nc.vector.tensor_scalar_add(out=qv[:TS], in0=qv[:TS], scalar1=sc(43))
        nc.vector.tensor_sub(out=qv[:TS], in0=qv[:TS], in1=musq[:TS])
        nc.scalar.activation(out=r_all[:TS], in_=qv[:TS], func=AF.Sqrt,
                             bias=eps_col[:TS], scale=1.0)
        nc.vector.reciprocal(out=r_all[:TS], in_=r_all[:TS])

        # ---------------- augmented input xa = [r*x; r; nmr; 1] ----------
        xa = singles.tile([T, KA, BL], bf16)
        for fi in range(F):
            eng = nc.vector if fi % 2 == 0 else nc.gpsimd
            eng.tensor_tensor(out=xa[:TS, fi, :], in0=xf(fi), in1=r_all[:TS],
                              op=ALU.mult)
        nc.vector.tensor_copy(out=xa[:TS, F, :], in_=r_all[:TS])
        nc.gpsimd.tensor_tensor(out=xa[:TS, F + 1, :], in0=nmu_all[:TS],
                                in1=r_all[:TS], op=ALU.mult)
        nc.vector.memset(xa[:TS, F + 2, :], 1.0)
        xa_dram = dpool.tile([T, KA, BL], bf16)
        nc.sync.dma_start(out=xa_dram[:TS], in_=xa[:TS])

        # ---------------- states ----------------
        h1 = singles.tile([H, BL], mmdt, name="h1", tag="h1")
        c = [singles.tile([H, BL], f32, name="c0", tag="c0"),
             singles.tile([H, BL], f32, name="c1", tag="c1")]
        zinit = trans.tile([H, BL], f32, tag="hf32", name="zinit")
        nc.vector.memset(zinit, 0.0)
        h0_prev = dbuf.tile([H, BL], mmdt, tag="h0", name="h0_init")
        nc.vector.tensor_copy(out=h0_prev, in_=zinit)
        nc.vector.tensor_copy(out=h1, in_=zinit)
        for L in range(2):
            nc.vector.memset(c[L], 0.0)

        # ---------------- main loop ----------------
        # Software-pipelined emission: each engine's FIFO sees work in an
        # order that never head-of-line-blocks.  Per iteration t:
        #   PE : 16 mm for L0(t), then 16 mm for L1(t-1)
        #   ACT: si0 sf0 tg0 so0 | si1 sf1 | tanh_c0 | tg1 so1 | tanh_c1
        #   DVE: u0 c0 h0 u1 c1 h1
        #   Pool: v0 v1
        # tanh_c0 sits mid-iteration so h0(t) completes early enough for
        # L0(t+1)'s matmuls to feed ACT without a wrap-around gap.
        GF = [AF.Sigmoid, AF.Sigmoid, AF.Tanh, AF.Sigmoid]

        def mm_gates(L, inp, inpT, hprev, hh_first):
            pgs = []
            for gc in range(4):
                pg = pg_tile([H, BL], f"pg{L}_g{gc}")
                for hc in range(NH):
                    sl = slice(hc * 512, (hc + 1) * 512)
                    ops = [(inpT[:, gc, :], inp),
                           (whhT[L][:, gc, :].bitcast(mmdt), hprev)]
                    if hh_first:
                        ops.reverse()
                    nc.tensor.matmul(pg[:, sl], ops[0][0], ops[0][1][:, sl],
                                     start=True, stop=False)
                    nc.tensor.matmul(pg[:, sl], ops[1][0], ops[1][1][:, sl],
                                     start=False, stop=True)
                pgs.append(pg)
            return pgs

        def act_gate(L, pgs, gc):
            o = trans.tile([H, BL], bf16, tag=f"sg{L}{gc}", name=f"sg{L}{gc}")
            if L == 0:
                nc.scalar.activation(out=o, in_=pgs[gc], func=GF[gc], scale=1.0)
            else:
                nc.scalar.activation(out=o, in_=pgs[gc], func=GF[gc],
                                     bias=beff1[:, gc:gc + 1], scale=1.0)
            return o

        # prefetch ring for xaug
        PF = 3
        xaug_tiles = {}

        def issue_xaug(t):
            if t >= T_steps:
                return
            xt = trans.tile([KA, BL], bf16, tag=f"xaug{t % PF}", name="xaug")
            nc.sync.dma_start(out=xt, in_=xa_dram[t])
            xaug_tiles[t] = xt

        for t in range(2):
            issue_xaug(t)

        def cell_front(L, sg):
            # u = si*tg (DVE), v = sf*c (Pool), c = u+v (DVE)
            u = trans.tile([H, BL], bf16, tag=f"u{L}", name=f"u{L}")
            nc.vector.tensor_tensor(out=u, in0=sg[0], in1=sg[2], op=ALU.mult)
            v_ = trans.tile([H, BL], f32, tag=f"v{L}", name=f"v{L}")
            nc.gpsimd.tensor_tensor(out=v_, in0=sg[1], in1=c[L], op=ALU.mult)
            nc.vector.tensor_add(out=c[L], in0=u, in1=v_)

        def cell_tanh(L):
            tc_ = trans.tile([H, BL], bf16, tag=f"tc{L}", name=f"tc{L}")
            nc.scalar.activation(out=tc_, in_=c[L], func=AF.Tanh, scale=1.0)
            return tc_

        def cell_h(sg, tc_, hout):
            nc.vector.tensor_tensor(out=hout, in0=sg[3], in1=tc_, op=ALU.mult)

        sg1 = None
        for t in range(T_steps):
            issue_xaug(t + 2)
            # PE: layer-0 step t gates, then layer-1 step t-1 gates
            pg0 = mm_gates(0, xaug_tiles.pop(t), wc10, h0_prev, hh_first=False)
            sg0 = [act_gate(0, pg0, 0), act_gate(0, pg0, 1)]
            sg0.append(act_gate(0, pg0, 2))
            sg0.append(act_gate(0, pg0, 3))
            cell_front(0, sg0)
            if t > 0:
                pg1 = mm_gates(1, h0_prev, wih1T.bitcast(mmdt), h1,
                               hh_first=True)
                sg1 = [act_gate(1, pg1, 0), act_gate(1, pg1, 1)]
            tc0 = cell_tanh(0)
            h0_new = dbuf.tile([H, BL], mmdt, tag="h0", name="h0_new")
            cell_h(sg0, tc0, h0_new)
            if t > 0:
                sg1.append(act_gate(1, pg1, 2))
                sg1.append(act_gate(1, pg1, 3))
                cell_front(1, sg1)
                tc1 = cell_tanh(1)
                cell_h(sg1, tc1, h1)
            h0_prev = h0_new
        # drain: final layer-1 step
        pg1 = mm_gates(1, h0_prev, wih1T.bitcast(mmdt), h1, hh_first=True)
        sg1 = [act_gate(1, pg1, g) for g in range(4)]
        cell_front(1, sg1)
        tc1 = cell_tanh(1)
        cell_h(sg1, tc1, h1)

        # ---------------- head ----------------
        h1f = trans.tile([H, BL], f32, tag="hf32", name="h1f")
        nc.vector.tensor_copy(out=h1f, in_=h1.bitcast(f32))
        sqh = prep.tile([H, BL], f32, tag="ha", name="sqh")
        nc.vector.tensor_tensor(out=sqh, in0=h1f, in1=h1f, op=ALU.mult)
        ones_col = small.tile([H, 1], f32, tag="ones_col", name="ones_col")
        nc.vector.memset(ones_col, 1.0)
        ps_s1 = pg_tile([1, BL], "ps_s1")
        ps_s2 = pg_tile([1, BL], "ps_s2")
        for hc in range(NH):
            sl = slice(hc * 512, (hc + 1) * 512)
            nc.tensor.matmul(ps_s1[:, sl], ones_col, h1f[:, sl],
                             start=True, stop=True, skip_group_check=True)
            nc.tensor.matmul(ps_s2[:, sl], ones_col, sqh[:, sl],
                             start=True, stop=True, skip_group_check=True)
        nmu_h = small.tile([1, BL], f32, tag="nmu_h", name="nmu_h")
        nc.vector.tensor_scalar_mul(out=nmu_h, in0=ps_s1, scalar1=-1.0 / H)
        musq_h = small.tile([1, BL], f32, tag="musq", name="musq_h")
        nc.vector.tensor_tensor(out=musq_h, in0=nmu_h, in1=nmu_h, op=ALU.mult)
        v_h = small.tile([1, BL], f32, tag="v_h", name="v_h")
        nc.vector.tensor_scalar_mul(out=v_h, in0=ps_s2, scalar1=1.0 / H)
        nc.vector.tensor_sub(out=v_h, in0=v_h, in1=musq_h)
        nc.scalar.activation(out=v_h, in_=v_h, func=AF.Sqrt,
                             bias=eps_col[0:1], scale=1.0)
        nc.vector.reciprocal(out=v_h, in_=v_h)
        hstat_dram = dpool.tile([2, BL], f32)
        nc.sync.dma_start(out=hstat_dram[0:1, :], in_=nmu_h)
        nc.sync.dma_start(out=hstat_dram[1:2, :], in_=v_h)
        nmbc = prep.tile([H, BL], f32, tag="hb", name="nmbc")
        nc.gpsimd.dma_start(out=nmbc, in_=hstat_dram[0:1, :].to_broadcast([H, BL]))
        rhbc = prep.tile([H, BL], f32, tag="hc", name="rhbc")
        nc.gpsimd.dma_start(out=rhbc, in_=hstat_dram[1:2, :].to_broadcast([H, BL]))
        t1 = prep.tile([H, BL], f32, tag="hd", name="t1")
        nc.vector.tensor_tensor(out=t1, in0=h1f, in1=nmbc, op=ALU.add)
        t2 = prep.tile([H, BL], f32, tag="ha", name="t2")
        nc.vector.tensor_tensor(out=t2, in0=t1, in1=rhbc, op=ALU.mult)
        last = prep.tile([H, BL], f32, tag="hb", name="last")
        nc.vector.tensor_scalar(out=last, in0=t2, scalar1=g_ln_c,
                                scalar2=be_ln_c, op0=ALU.mult, op1=ALU.add)
        pd1 = pg_tile([D1, BL], "pd1")
        for hc in range(NH):
            sl = slice(hc * 512, (hc + 1) * 512)
            nc.tensor.matmul(pd1[:, sl], wd1T, last[:, sl], start=True, stop=True,
                             skip_group_check=True)
        d1 = prep.tile([D1, BL], f32, tag="hc", name="d1")
        nc.scalar.activation(out=d1, in_=pd1, func=AF.Relu, bias=b_d1_c, scale=1.0)
        pd2 = pg_tile([D2, BL], "pd2")
        for hc in range(NH):
            sl = slice(hc * 512, (hc + 1) * 512)
            nc.tensor.matmul(pd2[:, sl], wd2T, d1[:, sl], start=True, stop=True,
                             skip_group_check=True)
        d2 = prep.tile([D2, BL], f32, tag="hd", name="d2")
        nc.scalar.activation(out=d2, in_=pd2, func=AF.Relu, bias=b_d2_c, scale=1.0)
        pd3 = pg_tile([OUT, BL], "pd3")
        for hc in range(NH):
            sl = slice(hc * 512, (hc + 1) * 512)
            nc.tensor.matmul(pd3[:, sl], wd3T, d2[:, sl], start=True, stop=True,
                             skip_group_check=True)
        o3 = prep.tile([OUT, BL], f32, tag="ha", name="o3")
        nc.scalar.activation(out=o3, in_=pd3, func=AF.Identity, bias=b_d3_c,
                             scale=1.0)
        outT = singles.tile([128, QB, OUT], f32)
        for q in range(QB):
            pot = pp_tile([128, OUT], "pot")
            nc.tensor.transpose(pot, o3[:, q * 128:(q + 1) * 128],
                                ident[:OUT, :OUT])
            nc.vector.tensor_copy(out=outT[:, q, :], in_=pot)
        nc.sync.dma_start(
            out=out_d[:, :].rearrange("(q p) c -> p q c", p=128),
            in_=outT)
    return nc


_CACHE = {}


def _fold_weights(inp):
    """Host-side weight-only preprocessing (float64). Returns the dict of
    derived dram inputs (excluding xT, which is per-core)."""
    import ml_dtypes
    d = {k: np.asarray(v, np.float64) for k, v in inp.items()}
    W = d["W_in"]                       # [H, F]
    g, b, be = d["g_in"], d["b_in"], d["be_in"]
    Wih0, Whh0 = d["Wih0"], d["Whh0"]   # [4H, H]
    Wih1, Whh1 = d["Wih1"], d["Whh1"]

    # wc10 rows: A = Wih0 diag(g) W, u = Wih0 (g*b), v = Wih0 g,
    #            beff0 = bih0 + bhh0 + Wih0 be
    Wg = Wih0 * g[None, :]              # [4H, H] (columns scaled)
    A = Wg @ W                          # [4H, F]
    u = Wg @ b
    v = Wg @ np.ones(H)
    beff0 = d["bih0"] + d["bhh0"] + Wih0 @ be
    wc10 = np.concatenate([A.T, u[None], v[None], beff0[None]], axis=0)  # [10, 4H]

    # stats constants: M = W^T W, wsum = 1^T W, l = W^T b, c0 = |b|^2
    M = W.T @ W
    wsum = W.sum(axis=0)
    l = W.T @ b
    c0 = float(b @ b)
    R = np.linalg.cholesky(M).T         # upper-tri: M = R^T R
    s = np.linalg.solve(R.T, l)         # R^T s = l
    sH = np.sqrt(float(H))
    Rp, sp = R / sH, s / sH
    bsum = float(b.sum())
    cols = []
    for i in range(F):
        cols.extend(Rp[i, i:])          # 28 upper-tri entries
    cols += list(sp)                    # 7 s'
    cols += list(-wsum / H)             # 7 wsum'
    cols += [-bsum / H, (c0 - float(s @ s)) / H]
    statc_row = np.asarray(cols, np.float64)
    assert statc_row.shape[0] == NSC
    statc = np.tile(statc_row[None, :], (T, 1))

    beff1 = (d["bih1"] + d["bhh1"]).reshape(4, H).T  # [H, 4]

    out = {
        "wc10": wc10.astype(ml_dtypes.bfloat16),
        "wih1T": np.ascontiguousarray(Wih1.T).astype(np.float32),
        "whh0T": np.ascontiguousarray(Whh0.T).astype(np.float32),
        "whh1T": np.ascontiguousarray(Whh1.T).astype(np.float32),
        "beff1": np.ascontiguousarray(beff1).astype(np.float32),
        "statc": statc.astype(np.float32),
        "g_ln": d["g_ln"].astype(np.float32),
        "be_ln": d["be_ln"].astype(np.float32),
        "wd1T": np.ascontiguousarray(d["W_d1"].T).astype(np.float32),
        "b_d1": d["b_d1"].astype(np.float32),
        "wd2T": np.ascontiguousarray(d["W_d2"].T).astype(np.float32),
        "b_d2": d["b_d2"].astype(np.float32),
        "wd3T": np.ascontiguousarray(d["W_d3"].T).astype(np.float32),
        "b_d3": d["b_d3"].astype(np.float32),
    }
    return out


def core_val(inp, name, ci, folded=None):
    """Per-core value for dram input `name` (inp: full raw-input dict)."""
    if name == "xT":
        return np.ascontiguousarray(
            np.asarray(inp["x"], np.float32)[ci * BL:(ci + 1) * BL]
            .transpose(1, 2, 0))
    if folded is None:
        folded = _fold_weights(inp)
    return folded[name]


def _get_runner():
    if "runner" in _CACHE:
        return _CACHE["runner"]
    import jax
    from jax.sharding import Mesh, PartitionSpec
    from jax.experimental.shard_map import shard_map
    import concourse.bacc as bacc
    import concourse.mybir as mybir
    from concourse.bass2jax import install_neuronx_cc_hook, _bass_exec_p, \
        partition_id_tensor

    nc = bacc.Bacc()
    _build(nc)
    nc.compile()
    install_neuronx_cc_hook()

    partition_name = nc.partition_id_tensor.name if nc.partition_id_tensor else None
    in_names, out_names, out_avals, zero_outs = [], [], [], []
    for alloc in nc.m.functions[0].allocations:
        if not isinstance(alloc, mybir.MemoryLocationSet):
            continue
        name = alloc.memorylocations[0].name
        if alloc.kind == "ExternalInput":
            if name != partition_name:
                in_names.append(name)
        elif alloc.kind == "ExternalOutput":
            out_names.append(name)
            shape = tuple(alloc.tensor_shape)
            dtype = mybir.dt.np(alloc.dtype)
            out_avals.append(jax.core.ShapedArray(shape, dtype))
            zero_outs.append(np.zeros(shape, dtype))
    n_params = len(in_names)
    all_in_names = in_names + out_names + ([partition_name] if partition_name else [])

    def _body(*args):
        operands = list(args)
        if partition_name is not None:
            operands.append(partition_id_tensor())
        outs = _bass_exec_p.bind(
            *operands,
            out_avals=tuple(out_avals),
            in_names=tuple(all_in_names),
            out_names=tuple(out_names),
            lowering_input_output_aliases=(),
            sim_require_finite=False,
            sim_require_nnan=False,
            nc=nc,
        )
        return tuple(outs)

    devices = jax.devices()[:NCORES]
    mesh = Mesh(np.asarray(devices), ("core",))
    in_specs = (PartitionSpec("core"),) * (n_params + len(out_names))
    out_specs = (PartitionSpec("core"),) * len(out_names)
    sharded = jax.jit(
        shard_map(_body, mesh=mesh, in_specs=in_specs, out_specs=out_specs,
                  check_rep=False),
        keep_unused=True)
    _CACHE["runner"] = (sharded, in_names, out_names, zero_outs)
    return _CACHE["runner"]


def kernel(**inputs) -> np.ndarray:
    sharded, in_names, out_names, zero_outs = _get_runner()
    inp = {k: np.asarray(v) for k, v in inputs.items()}
    folded = _fold_weights(inp)

    concat_in = [
        np.concatenate([core_val(inp, n, ci, folded) for ci in range(NCORES)],
                       axis=0)
        for n in in_names
    ]
    concat_zeros = [
        np.zeros((NCORES * z.shape[0], *z.shape[1:]), z.dtype) for z in zero_outs
    ]
    import jax
    out_arrs = sharded(*concat_in, *concat_zeros)
    jax.block_until_ready(out_arrs)
    oi = out_names.index("out")
    full = np.asarray(out_arrs[oi]).reshape(B, OUT)
    return full.astype(np.float32)


# revision 14
# speedup vs baseline: 1.5306x; 1.0078x over previous
"""DepletionLSTM Trainium2 kernel (v2 — ACT-roof design).

Self-contained: builds a Bass/Tile kernel for the 2-layer-LSTM network,
shards the batch over 8 NeuronCores (pure data parallelism), runs via
PJRT/axon, returns the full [8192, 30] float32 output.

Strategy (per core, 1024 batch):
- Host stages x transposed to [T, F, BL] per core, and folds all
  weight-only expressions (transposes + the fused input-pipeline matrix)
  once in float64 — standard compile-time weight preprocessing.  All
  x-dependent math runs on device.
- The entire input pipeline (W_in projection + LayerNorm + layer-0 input
  matmul + layer-0 gate biases) collapses into ONE K=10 matmul per gate:
    zin0 = wc10^T @ [r*x; r; -mu*r; 1]
  with wc10 rows [A; u; v; beff0], A = Wih0 diag(g_in) W_in [4H x 7],
  u = Wih0 (g_in*b_in), v = Wih0 g_in.
- LN stats (mu, rstd) are computed in a [T=90 part, BL] prepass using a
  Cholesky factorization of the quadratic form:  sum_h p_h^2 = |R x + s|^2
  + const, so the per-row squares run on the otherwise-idle ACT engine and
  the linear chains split across DVE and GPSIMD.
- The augmented input xa = [r*x; r; nmr; 1] (bf16) is staged to DRAM once
  and streamed back per step as a [10, BL] tile (one DMA per step,
  double-buffered).
- Per step per layer: 4 accumulating gate matmul pairs (input + recurrent,
  N=512 chunks; input side bf16, recurrent fp32r), 4 sigmoid/tanh ACT ops
  (bf16 out), tanh(c) ACT, and 3 DVE ops (u=si*tg in bf16 2x-mode, c=u+v,
  h=so*tc) plus v=sf*c on GPSIMD.  Layer 1 runs one timestep behind layer
  0 so both layers' work interleaves; ACT (the only sigmoid/tanh engine)
  is the roofline at ~10.4us/step.
- PSUM: 3 rotating gate tiles [128,1024] (6 banks) + 2 utility banks.
"""
import sys
sys.path.insert(0, '/opt/trn_rl_repo')

import numpy as np

B, T, F, H, D1, D2, OUT = 8192, 90, 7, 128, 128, 64, 30
NCORES = 8
BL = B // NCORES
G4 = 4 * H
NH = BL // 512
QB = BL // 128
EPS = 1e-5
MMDT = "float32r"
KA = F + 3  # augmented-input rows: 7 x-rows, r, nmr, ones
NSC = 44    # stat-constant columns: 28 R + 7 s + 7 wsum' + bsum' + c0''


def _build(nc, T_steps=T, mmdt_name=MMDT, dbg=False):
    import concourse.tile as tile
    from concourse import mybir
    from concourse.masks import make_identity

    f32 = mybir.dt.float32
    bf16 = mybir.dt.bfloat16
    mmdt = getattr(mybir.dt, mmdt_name)
    AF = mybir.ActivationFunctionType
    ALU = mybir.AluOpType

    # ---------------- DRAM I/O (host-folded weights) ----------------
    xT_d = nc.dram_tensor("xT", [T, F, BL], f32, kind="ExternalInput")
    wc10_d = nc.dram_tensor("wc10", [KA, G4], bf16, kind="ExternalInput")
    wih1T_d = nc.dram_tensor("wih1T", [H, G4], f32, kind="ExternalInput")
    whh0T_d = nc.dram_tensor("whh0T", [H, G4], f32, kind="ExternalInput")
    whh1T_d = nc.dram_tensor("whh1T", [H, G4], f32, kind="ExternalInput")
    beff1_d = nc.dram_tensor("beff1", [H, 4], f32, kind="ExternalInput")
    statc_d = nc.dram_tensor("statc", [T, NSC], f32, kind="ExternalInput")
    g_ln_d = nc.dram_tensor("g_ln", [H], f32, kind="ExternalInput")
    be_ln_d = nc.dram_tensor("be_ln", [H], f32, kind="ExternalInput")
    wd1T_d = nc.dram_tensor("wd1T", [H, D1], f32, kind="ExternalInput")
    b_d1_d = nc.dram_tensor("b_d1", [D1], f32, kind="ExternalInput")
    wd2T_d = nc.dram_tensor("wd2T", [D1, D2], f32, kind="ExternalInput")
    b_d2_d = nc.dram_tensor("b_d2", [D2], f32, kind="ExternalInput")
    wd3T_d = nc.dram_tensor("wd3T", [D2, OUT], f32, kind="ExternalInput")
    b_d3_d = nc.dram_tensor("b_d3", [OUT], f32, kind="ExternalInput")
    out_d = nc.dram_tensor("out", [BL, OUT], f32, kind="ExternalOutput")

    import contextlib
    with tile.TileContext(nc) as tc, contextlib.ExitStack() as ctx:
        singles = ctx.enter_context(tc.tile_pool(name="singles", bufs=1))
        prep = ctx.enter_context(tc.tile_pool(name="prep", bufs=1))
        trans = ctx.enter_context(tc.tile_pool(name="trans", bufs=1))
        dbuf = ctx.enter_context(tc.tile_pool(name="dbuf", bufs=2))
        small = ctx.enter_context(tc.tile_pool(name="small", bufs=2))
        ps_pg = ctx.enter_context(tc.tile_pool(name="ps_pg", bufs=3, space="PSUM"))
        ps_pp = ctx.enter_context(tc.tile_pool(name="ps_pp", bufs=2, space="PSUM"))
        dpool = ctx.enter_context(tc.tile_pool(name="dpool", bufs=1, space="DRAM"))

        def pg_tile(shape, name):
            return ps_pg.tile(shape, f32, tag="pg", name=name)

        def pp_tile(shape, name):
            return ps_pp.tile(shape, f32, tag="pp", name=name)

        # ---------------- constants / weights ----------------
        ident = singles.tile([128, 128], f32)
        make_identity(nc, ident)
        eps_col = singles.tile([T, 1], f32)
        nc.vector.memset(eps_col, EPS)

        def load_col(dram_vec, n, name):
            t_ = singles.tile([n, 1], f32, name=name, tag=name)
            nc.sync.dma_start(out=t_, in_=dram_vec[:].rearrange("(p o) -> p o", o=1))
            return t_

        g_ln_c = load_col(g_ln_d, H, "g_ln_c")
        be_ln_c = load_col(be_ln_d, H, "be_ln_c")
        b_d1_c = load_col(b_d1_d, D1, "b_d1_c")
        b_d2_c = load_col(b_d2_d, D2, "b_d2_c")
        b_d3_c = load_col(b_d3_d, OUT, "b_d3_c")

        wc10 = singles.tile([KA, 4, H], bf16, name="wc10", tag="wc10")
        nc.sync.dma_start(out=wc10,
                          in_=wc10_d[:, :].rearrange("p (c m) -> p c m", c=4))
        def load_mmdt(dram, name):
            # DMA the raw f32 weights, then DVE-copy into an f32r tile (the
            # copy performs the required fp32r rounding for PE consumption).
            raw = prep.tile([H, G4], f32, tag="wraw", name=f"{name}_raw")
            nc.sync.dma_start(out=raw, in_=dram[:, :])
            w_ = singles.tile([H, 4, H], mmdt, name=name, tag=name)
            nc.vector.tensor_copy(
                out=w_[:, :, :].rearrange("p c m -> p (c m)"), in_=raw)
            return w_

        wih1T = load_mmdt(wih1T_d, "wih1T")
        whhT = [load_mmdt(whh0T_d, "whhT0"), load_mmdt(whh1T_d, "whhT1")]
        beff1 = singles.tile([H, 4], f32, name="beff1", tag="beff1")
        nc.sync.dma_start(out=beff1, in_=beff1_d[:, :])
        statc = singles.tile([T, NSC], f32, name="statc", tag="statc")
        nc.sync.dma_start(out=statc, in_=statc_d[:, :])
        wd1T = singles.tile([H, D1], f32)
        nc.sync.dma_start(out=wd1T, in_=wd1T_d[:, :])
        wd2T = singles.tile([D1, D2], f32)
        nc.sync.dma_start(out=wd2T, in_=wd2T_d[:, :])
        wd3T = singles.tile([D2, OUT], f32)
        nc.sync.dma_start(out=wd3T, in_=wd3T_d[:, :])

        # statc column layout (must match host packing in kernel()):
        #   0..27  : R'_ij rows i=0..6, j=i..6 (upper-tri, row-major)
        #   28..34 : s'_i
        #   35..41 : wsum'_f  (= -wsum_f/H)
        #   42     : bsum'    (= -bsum/H)
        #   43     : c0''     (= (c0-|s|^2)/H)
        _roff = [0, 7, 13, 18, 22, 25, 27]

        def sc(j):
            return statc[:T_steps, j:j + 1]

        # ---------------- x load ([T part, F, BL], contiguous) ----------
        x_ftb = singles.tile([T, F, BL], f32)
        nc.sync.dma_start(out=x_ftb, in_=xT_d[:, :, :])

        def xf(fi):
            return x_ftb[:T_steps, fi, :]

        TS = T_steps

        # ---------------- prepass: LN stats in [T, BL] layout ------------
        # nmu = sum_f wsum'_f x_f + bsum'   (wsum' = -wsum/H)
        # y_i = sum_{j>=i} R'_ij x_j + s'_i ; q/H = sum_i y_i^2 + c0''
        # Chain seeds run on ACT (Identity with per-partition scale+bias),
        # chain continuations on DVE (scalar_tensor_tensor with AP scalar),
        # squares on ACT, square-sums and products on GPSIMD.
        nmu_all = singles.tile([T, BL], f32)
        r_all = singles.tile([T, BL], f32)
        nc.scalar.activation(out=nmu_all[:TS], in_=xf(0), func=AF.Identity,
                             scale=sc(35), bias=sc(42))
        for fi in range(1, F):
            nc.vector.scalar_tensor_tensor(
                out=nmu_all[:TS], in0=xf(fi), scalar=sc(35 + fi),
                in1=nmu_all[:TS], op0=ALU.mult, op1=ALU.add)
        sqs = []
        for i in range(F):
            z = prep.tile([T, BL], f32, tag=f"stz{i % 4}", name=f"st_z{i}")
            nc.scalar.activation(out=z[:TS], in_=xf(i), func=AF.Identity,
                                 scale=sc(_roff[i]), bias=sc(28 + i))
            for j in range(i + 1, F):
                nc.vector.scalar_tensor_tensor(
                    out=z[:TS], in0=xf(j), scalar=sc(_roff[i] + j - i),
                    in1=z[:TS], op0=ALU.mult, op1=ALU.add)
            sq = prep.tile([T, BL], f32, tag=f"stsq{i % 3}", name=f"st_sq{i}")
            nc.scalar.activation(out=sq[:TS], in_=z[:TS], func=AF.Square,
                                 scale=1.0)
            sqs.append(sq)
            if i == 1:
                qv = prep.tile([T, BL], f32, tag="stqv", name="st_qv")
                nc.gpsimd.tensor_add(out=qv[:TS], in0=sqs[0][:TS],
                                     in1=sqs[1][:TS])
            elif i > 1:
                nc.gpsimd.tensor_add(out=qv[:TS], in0=qv[:TS], in1=sq[:TS])
        # var = q/H + c0'' - mu^2
        musq = prep.tile([T, BL], f32, tag="stz0", name="st_musq")
        nc.gpsimd.tensor_tensor(out=musq[:TS], in0=nmu_all[:TS],
                                in1=nmu_all[:TS], op=ALU.mult)
        nc.vector.tensor_scalar_add(out=qv[:TS], in0=qv[:TS], scalar1=sc(43))
        nc.vector.tensor_sub(out=qv[:TS], in0=qv[:TS], in1=musq[:TS])
        nc.scalar.activation(out=r_all[:TS], in_=qv[:TS], func=AF.Sqrt,
                             bias=eps_col[:TS], scale=1.0)
        nc.vector.reciprocal(out=r_all[:TS], in_=r_all[:TS])

        # ---------------- augmented input xa = [r*x; r; nmr; 1] ----------
        xa = singles.tile([T, KA, BL], bf16)
        for fi in range(F):
            eng = nc.vector if fi % 2 == 0 else nc.gpsimd
            eng.tensor_tensor(out=xa[:TS, fi, :], in0=xf(fi), in1=r_all[:TS],
                              op=ALU.mult)
        nc.vector.tensor_copy(out=xa[:TS, F, :], in_=r_all[:TS])
        nc.gpsimd.tensor_tensor(out=xa[:TS, F + 1, :], in0=nmu_all[:TS],
                                in1=r_all[:TS], op=ALU.mult)
        nc.vector.memset(xa[:TS, F + 2, :], 1.0)
        xa_dram = dpool.tile([T, KA, BL], bf16)
        nc.sync.dma_start(out=xa_dram[:TS], in_=xa[:TS])

        # ---------------- states ----------------
        h1 = singles.tile([H, BL], mmdt, name="h1", tag="h1")
        c = [singles.tile([H, BL], f32, name="c0", tag="c0"),
             singles.tile([H, BL], f32, name="c1", tag="c1")]
        zinit = trans.tile([H, BL], f32, tag="hf32", name="zinit")
        nc.vector.memset(zinit, 0.0)
        h0_prev = dbuf.tile([H, BL], mmdt, tag="h0", name="h0_init")
        nc.vector.tensor_copy(out=h0_prev, in_=zinit)
        nc.vector.tensor_copy(out=h1, in_=zinit)
        for L in range(2):
            nc.vector.memset(c[L], 0.0)

        # ---------------- main loop ----------------
        # Software-pipelined emission: each engine's FIFO sees work in an
        # order that never head-of-line-blocks.  Per iteration t:
        #   PE : 16 mm for L0(t), then 16 mm for L1(t-1)
        #   ACT: si0 sf0 tg0 so0 | si1 sf1 | tanh_c0 | tg1 so1 | tanh_c1
        #   DVE: u0 c0 h0 u1 c1 h1
        #   Pool: v0 v1
        # tanh_c0 sits mid-iteration so h0(t) completes early enough for
        # L0(t+1)'s matmuls to feed ACT without a wrap-around gap.
        GF = [AF.Sigmoid, AF.Sigmoid, AF.Tanh, AF.Sigmoid]

        def mm_gates(L, inp, inpT, hprev, hh_first):
            pgs = []
            for gc in range(4):
                pg = pg_tile([H, BL], f"pg{L}_g{gc}")
                for hc in range(NH):
                    sl = slice(hc * 512, (hc + 1) * 512)
                    ops = [(inpT[:, gc, :], inp),
                           (whhT[L][:, gc, :], hprev)]
                    if hh_first:
                        ops.reverse()
                    nc.tensor.matmul(pg[:, sl], ops[0][0], ops[0][1][:, sl],
                                     start=True, stop=False)
                    nc.tensor.matmul(pg[:, sl], ops[1][0], ops[1][1][:, sl],
                                     start=False, stop=True)
                pgs.append(pg)
            return pgs

        def act_gate(L, pgs, gc):
            o = trans.tile([H, BL], bf16, tag=f"sg{L}{gc}", name=f"sg{L}{gc}")
            if L == 0:
                nc.scalar.activation(out=o, in_=pgs[gc], func=GF[gc], scale=1.0)
            else:
                nc.scalar.activation(out=o, in_=pgs[gc], func=GF[gc],
                                     bias=beff1[:, gc:gc + 1], scale=1.0)
            return o

        # prefetch ring for xaug
        PF = 3
        xaug_tiles = {}

        def issue_xaug(t):
            if t >= T_steps:
                return
            xt = trans.tile([KA, BL], bf16, tag=f"xaug{t % PF}", name="xaug")
            nc.sync.dma_start(out=xt, in_=xa_dram[t])
            xaug_tiles[t] = xt

        for t in range(2):
            issue_xaug(t)

        def cell_front(L, sg):
            # u = si*tg (DVE), v = sf*c (Pool), c = u+v (DVE)
            u = trans.tile([H, BL], bf16, tag=f"u{L}", name=f"u{L}")
            nc.vector.tensor_tensor(out=u, in0=sg[0], in1=sg[2], op=ALU.mult)
            v_ = trans.tile([H, BL], f32, tag=f"v{L}", name=f"v{L}")
            nc.gpsimd.tensor_tensor(out=v_, in0=sg[1], in1=c[L], op=ALU.mult)
            nc.vector.tensor_add(out=c[L], in0=u, in1=v_)

        def cell_tanh(L):
            tc_ = trans.tile([H, BL], bf16, tag=f"tc{L}", name=f"tc{L}")
            nc.scalar.activation(out=tc_, in_=c[L], func=AF.Tanh, scale=1.0)
            return tc_

        def cell_h(sg, tc_, hout):
            nc.vector.tensor_tensor(out=hout, in0=sg[3], in1=tc_, op=ALU.mult)

        sg1 = None
        for t in range(T_steps):
            issue_xaug(t + 2)
            # PE: layer-0 step t gates, then layer-1 step t-1 gates
            pg0 = mm_gates(0, xaug_tiles.pop(t), wc10, h0_prev, hh_first=False)
            sg0 = [act_gate(0, pg0, 0), act_gate(0, pg0, 1)]
            sg0.append(act_gate(0, pg0, 2))
            sg0.append(act_gate(0, pg0, 3))
            cell_front(0, sg0)
            if t > 0:
                pg1 = mm_gates(1, h0_prev, wih1T, h1,
                               hh_first=True)
                sg1 = [act_gate(1, pg1, 0), act_gate(1, pg1, 1)]
            tc0 = cell_tanh(0)
            h0_new = dbuf.tile([H, BL], mmdt, tag="h0", name="h0_new")
            cell_h(sg0, tc0, h0_new)
            if t > 0:
                sg1.append(act_gate(1, pg1, 2))
                sg1.append(act_gate(1, pg1, 3))
                cell_front(1, sg1)
                tc1 = cell_tanh(1)
                cell_h(sg1, tc1, h1)
            h0_prev = h0_new
        # drain: final layer-1 step
        pg1 = mm_gates(1, h0_prev, wih1T, h1, hh_first=True)
        sg1 = [act_gate(1, pg1, g) for g in range(4)]
        cell_front(1, sg1)
        tc1 = cell_tanh(1)
        cell_h(sg1, tc1, h1)

        # ---------------- head ----------------
        h1f = trans.tile([H, BL], f32, tag="hf32", name="h1f")
        nc.vector.tensor_copy(out=h1f, in_=h1.bitcast(f32))
        sqh = prep.tile([H, BL], f32, tag="ha", name="sqh")
        nc.vector.tensor_tensor(out=sqh, in0=h1f, in1=h1f, op=ALU.mult)
        ones_col = small.tile([H, 1], f32, tag="ones_col", name="ones_col")
        nc.vector.memset(ones_col, 1.0)
        ps_s1 = pg_tile([1, BL], "ps_s1")
        ps_s2 = pg_tile([1, BL], "ps_s2")
        for hc in range(NH):
            sl = slice(hc * 512, (hc + 1) * 512)
            nc.tensor.matmul(ps_s1[:, sl], ones_col, h1f[:, sl],
                             start=True, stop=True, skip_group_check=True)
            nc.tensor.matmul(ps_s2[:, sl], ones_col, sqh[:, sl],
                             start=True, stop=True, skip_group_check=True)
        nmu_h = small.tile([1, BL], f32, tag="nmu_h", name="nmu_h")
        nc.vector.tensor_scalar_mul(out=nmu_h, in0=ps_s1, scalar1=-1.0 / H)
        musq_h = small.tile([1, BL], f32, tag="musq", name="musq_h")
        nc.vector.tensor_tensor(out=musq_h, in0=nmu_h, in1=nmu_h, op=ALU.mult)
        v_h = small.tile([1, BL], f32, tag="v_h", name="v_h")
        nc.vector.tensor_scalar_mul(out=v_h, in0=ps_s2, scalar1=1.0 / H)
        nc.vector.tensor_sub(out=v_h, in0=v_h, in1=musq_h)
        nc.scalar.activation(out=v_h, in_=v_h, func=AF.Sqrt,
                             bias=eps_col[0:1], scale=1.0)
        nc.vector.reciprocal(out=v_h, in_=v_h)
        hstat_dram = dpool.tile([2, BL], f32)
        nc.sync.dma_start(out=hstat_dram[0:1, :], in_=nmu_h)
        nc.sync.dma_start(out=hstat_dram[1:2, :], in_=v_h)
        nmbc = prep.tile([H, BL], f32, tag="hb", name="nmbc")
        nc.gpsimd.dma_start(out=nmbc, in_=hstat_dram[0:1, :].to_broadcast([H, BL]))
        rhbc = prep.tile([H, BL], f32, tag="hc", name="rhbc")
        nc.gpsimd.dma_start(out=rhbc, in_=hstat_dram[1:2, :].to_broadcast([H, BL]))
        t1 = prep.tile([H, BL], f32, tag="hd", name="t1")
        nc.vector.tensor_tensor(out=t1, in0=h1f, in1=nmbc, op=ALU.add)
        t2 = prep.tile([H, BL], f32, tag="ha", name="t2")
        nc.vector.tensor_tensor(out=t2, in0=t1, in1=rhbc, op=ALU.mult)
        last = prep.tile([H, BL], f32, tag="hb", name="last")
        nc.vector.tensor_scalar(out=last, in0=t2, scalar1=g_ln_c,
                                scalar2=be_ln_c, op0=ALU.mult, op1=ALU.add)
        pd1 = pg_tile([D1, BL], "pd1")
        for hc in range(NH):
            sl = slice(hc * 512, (hc + 1) * 512)
            nc.tensor.matmul(pd1[:, sl], wd1T, last[:, sl], start=True, stop=True,
                             skip_group_check=True)
        d1 = prep.tile([D1, BL], f32, tag="hc", name="d1")
        nc.scalar.activation(out=d1, in_=pd1, func=AF.Relu, bias=b_d1_c, scale=1.0)
        pd2 = pg_tile([D2, BL], "pd2")
        for hc in range(NH):
            sl = slice(hc * 512, (hc + 1) * 512)
            nc.tensor.matmul(pd2[:, sl], wd2T, d1[:, sl], start=True, stop=True,
                             skip_group_check=True)
        d2 = prep.tile([D2, BL], f32, tag="hd", name="d2")
        nc.scalar.activation(out=d2, in_=pd2, func=AF.Relu, bias=b_d2_c, scale=1.0)
        pd3 = pg_tile([OUT, BL], "pd3")
        for hc in range(NH):
            sl = slice(hc * 512, (hc + 1) * 512)
            nc.tensor.matmul(pd3[:, sl], wd3T, d2[:, sl], start=True, stop=True,
                             skip_group_check=True)
        o3 = prep.tile([OUT, BL], f32, tag="ha", name="o3")
        nc.scalar.activation(out=o3, in_=pd3, func=AF.Identity, bias=b_d3_c,
                             scale=1.0)
        outT = singles.tile([128, QB, OUT], f32)
        for q in range(QB):
            pot = pp_tile([128, OUT], "pot")
            nc.tensor.transpose(pot, o3[:, q * 128:(q + 1) * 128],
                                ident[:OUT, :OUT])
            nc.vector.tensor_copy(out=outT[:, q, :], in_=pot)
        nc.sync.dma_start(
            out=out_d[:, :].rearrange("(q p) c -> p q c", p=128),
            in_=outT)
    return nc


_CACHE = {}


def _fold_weights(inp):
    """Host-side weight-only preprocessing (float64). Returns the dict of
    derived dram inputs (excluding xT, which is per-core)."""
    import ml_dtypes
    d = {k: np.asarray(v, np.float64) for k, v in inp.items()}
    W = d["W_in"]                       # [H, F]
    g, b, be = d["g_in"], d["b_in"], d["be_in"]
    Wih0, Whh0 = d["Wih0"], d["Whh0"]   # [4H, H]
    Wih1, Whh1 = d["Wih1"], d["Whh1"]

    # wc10 rows: A = Wih0 diag(g) W, u = Wih0 (g*b), v = Wih0 g,
    #            beff0 = bih0 + bhh0 + Wih0 be
    Wg = Wih0 * g[None, :]              # [4H, H] (columns scaled)
    A = Wg @ W                          # [4H, F]
    u = Wg @ b
    v = Wg @ np.ones(H)
    beff0 = d["bih0"] + d["bhh0"] + Wih0 @ be
    wc10 = np.concatenate([A.T, u[None], v[None], beff0[None]], axis=0)  # [10, 4H]

    # stats constants: M = W^T W, wsum = 1^T W, l = W^T b, c0 = |b|^2
    M = W.T @ W
    wsum = W.sum(axis=0)
    l = W.T @ b
    c0 = float(b @ b)
    R = np.linalg.cholesky(M).T         # upper-tri: M = R^T R
    s = np.linalg.solve(R.T, l)         # R^T s = l
    sH = np.sqrt(float(H))
    Rp, sp = R / sH, s / sH
    bsum = float(b.sum())
    cols = []
    for i in range(F):
        cols.extend(Rp[i, i:])          # 28 upper-tri entries
    cols += list(sp)                    # 7 s'
    cols += list(-wsum / H)             # 7 wsum'
    cols += [-bsum / H, (c0 - float(s @ s)) / H]
    statc_row = np.asarray(cols, np.float64)
    assert statc_row.shape[0] == NSC
    statc = np.tile(statc_row[None, :], (T, 1))

    beff1 = (d["bih1"] + d["bhh1"]).reshape(4, H).T  # [H, 4]

    out = {
        "wc10": wc10.astype(ml_dtypes.bfloat16),
        "wih1T": np.ascontiguousarray(Wih1.T).astype(np.float32),
        "whh0T": np.ascontiguousarray(Whh0.T).astype(np.float32),
        "whh1T": np.ascontiguousarray(Whh1.T).astype(np.float32),
        "beff1": np.ascontiguousarray(beff1).astype(np.float32),
        "statc": statc.astype(np.float32),
        "g_ln": d["g_ln"].astype(np.float32),
        "be_ln": d["be_ln"].astype(np.float32),
        "wd1T": np.ascontiguousarray(d["W_d1"].T).astype(np.float32),
        "b_d1": d["b_d1"].astype(np.float32),
        "wd2T": np.ascontiguousarray(d["W_d2"].T).astype(np.float32),
        "b_d2": d["b_d2"].astype(np.float32),
        "wd3T": np.ascontiguousarray(d["W_d3"].T).astype(np.float32),
        "b_d3": d["b_d3"].astype(np.float32),
    }
    return out


def core_val(inp, name, ci, folded=None):
    """Per-core value for dram input `name` (inp: full raw-input dict)."""
    if name == "xT":
        return np.ascontiguousarray(
            np.asarray(inp["x"], np.float32)[ci * BL:(ci + 1) * BL]
            .transpose(1, 2, 0))
    if folded is None:
        folded = _fold_weights(inp)
    return folded[name]


def _get_runner():
    if "runner" in _CACHE:
        return _CACHE["runner"]
    import jax
    from jax.sharding import Mesh, PartitionSpec
    from jax.experimental.shard_map import shard_map
    import concourse.bacc as bacc
    import concourse.mybir as mybir
    from concourse.bass2jax import install_neuronx_cc_hook, _bass_exec_p, \
        partition_id_tensor

    nc = bacc.Bacc()
    _build(nc)
    nc.compile()
    install_neuronx_cc_hook()

    partition_name = nc.partition_id_tensor.name if nc.partition_id_tensor else None
    in_names, out_names, out_avals, zero_outs = [], [], [], []
    for alloc in nc.m.functions[0].allocations:
        if not isinstance(alloc, mybir.MemoryLocationSet):
            continue
        name = alloc.memorylocations[0].name
        if alloc.kind == "ExternalInput":
            if name != partition_name:
                in_names.append(name)
        elif alloc.kind == "ExternalOutput":
            out_names.append(name)
            shape = tuple(alloc.tensor_shape)
            dtype = mybir.dt.np(alloc.dtype)
            out_avals.append(jax.core.ShapedArray(shape, dtype))
            zero_outs.append(np.zeros(shape, dtype))
    n_params = len(in_names)
    all_in_names = in_names + out_names + ([partition_name] if partition_name else [])

    def _body(*args):
        operands = list(args)
        if partition_name is not None:
            operands.append(partition_id_tensor())
        outs = _bass_exec_p.bind(
            *operands,
            out_avals=tuple(out_avals),
            in_names=tuple(all_in_names),
            out_names=tuple(out_names),
            lowering_input_output_aliases=(),
            sim_require_finite=False,
            sim_require_nnan=False,
            nc=nc,
        )
        return tuple(outs)

    devices = jax.devices()[:NCORES]
    mesh = Mesh(np.asarray(devices), ("core",))
    in_specs = (PartitionSpec("core"),) * (n_params + len(out_names))
    out_specs = (PartitionSpec("core"),) * len(out_names)
    sharded = jax.jit(
        shard_map(_body, mesh=mesh, in_specs=in_specs, out_specs=out_specs,
                  check_rep=False),
        keep_unused=True)
    _CACHE["runner"] = (sharded, in_names, out_names, zero_outs)
    return _CACHE["runner"]


def kernel(**inputs) -> np.ndarray:
    sharded, in_names, out_names, zero_outs = _get_runner()
    inp = {k: np.asarray(v) for k, v in inputs.items()}
    folded = _fold_weights(inp)

    concat_in = [
        np.concatenate([core_val(inp, n, ci, folded) for ci in range(NCORES)],
                       axis=0)
        for n in in_names
    ]
    concat_zeros = [
        np.zeros((NCORES * z.shape[0], *z.shape[1:]), z.dtype) for z in zero_outs
    ]
    import jax
    out_arrs = sharded(*concat_in, *concat_zeros)
    jax.block_until_ready(out_arrs)
    oi = out_names.index("out")
    full = np.asarray(out_arrs[oi]).reshape(B, OUT)
    return full.astype(np.float32)


# revision 15
# speedup vs baseline: 1.5364x; 1.0038x over previous
"""DepletionLSTM Trainium2 kernel (v2 — ACT-roof design).

Self-contained: builds a Bass/Tile kernel for the 2-layer-LSTM network,
shards the batch over 8 NeuronCores (pure data parallelism), runs via
PJRT/axon, returns the full [8192, 30] float32 output.

Strategy (per core, 1024 batch):
- Host stages x transposed to [T, F, BL] per core, and folds all
  weight-only expressions (transposes + the fused input-pipeline matrix)
  once in float64 — standard compile-time weight preprocessing.  All
  x-dependent math runs on device.
- The entire input pipeline (W_in projection + LayerNorm + layer-0 input
  matmul + layer-0 gate biases) collapses into ONE K=10 matmul per gate:
    zin0 = wc10^T @ [r*x; r; -mu*r; 1]
  with wc10 rows [A; u; v; beff0], A = Wih0 diag(g_in) W_in [4H x 7],
  u = Wih0 (g_in*b_in), v = Wih0 g_in.
- LN stats (mu, rstd) are computed in a [T=90 part, BL] prepass using a
  Cholesky factorization of the quadratic form:  sum_h p_h^2 = |R x + s|^2
  + const, so the per-row squares run on the otherwise-idle ACT engine and
  the linear chains split across DVE and GPSIMD.
- The augmented input xa = [r*x; r; nmr; 1] (bf16) is staged to DRAM once
  and streamed back per step as a [10, BL] tile (one DMA per step,
  double-buffered).
- Per step per layer: 4 accumulating gate matmul pairs (input + recurrent,
  N=512 chunks; input side bf16, recurrent fp32r), 4 sigmoid/tanh ACT ops
  (bf16 out), tanh(c) ACT, and 3 DVE ops (u=si*tg in bf16 2x-mode, c=u+v,
  h=so*tc) plus v=sf*c on GPSIMD.  Layer 1 runs one timestep behind layer
  0 so both layers' work interleaves; ACT (the only sigmoid/tanh engine)
  is the roofline at ~10.4us/step.
- PSUM: 3 rotating gate tiles [128,1024] (6 banks) + 2 utility banks.
"""
import sys
sys.path.insert(0, '/opt/trn_rl_repo')

import numpy as np

B, T, F, H, D1, D2, OUT = 8192, 90, 7, 128, 128, 64, 30
NCORES = 8
BL = B // NCORES
G4 = 4 * H
NH = BL // 512
QB = BL // 128
EPS = 1e-5
MMDT = "float32r"
KA = F + 3  # augmented-input rows: 7 x-rows, r, nmr, ones
NSC = 44    # stat-constant columns: 28 R + 7 s + 7 wsum' + bsum' + c0''


def _build(nc, T_steps=T, mmdt_name=MMDT, dbg=False):
    import concourse.tile as tile
    from concourse import mybir
    from concourse.masks import make_identity

    f32 = mybir.dt.float32
    bf16 = mybir.dt.bfloat16
    mmdt = getattr(mybir.dt, mmdt_name)
    AF = mybir.ActivationFunctionType
    ALU = mybir.AluOpType

    # ---------------- DRAM I/O (host-folded weights) ----------------
    xT_d = nc.dram_tensor("xT", [T, F, BL], f32, kind="ExternalInput")
    wc10_d = nc.dram_tensor("wc10", [KA, G4], bf16, kind="ExternalInput")
    wih1T_d = nc.dram_tensor("wih1T", [H, G4], f32, kind="ExternalInput")
    whh0T_d = nc.dram_tensor("whh0T", [H, G4], f32, kind="ExternalInput")
    whh1T_d = nc.dram_tensor("whh1T", [H, G4], f32, kind="ExternalInput")
    beff1_d = nc.dram_tensor("beff1", [H, 4], f32, kind="ExternalInput")
    statc_d = nc.dram_tensor("statc", [T, NSC], f32, kind="ExternalInput")
    wd1T_d = nc.dram_tensor("wd1T", [H, D1], f32, kind="ExternalInput")
    b_d1_d = nc.dram_tensor("b_d1", [D1], f32, kind="ExternalInput")
    wd2T_d = nc.dram_tensor("wd2T", [D1, D2], f32, kind="ExternalInput")
    b_d2_d = nc.dram_tensor("b_d2", [D2], f32, kind="ExternalInput")
    wd3T_d = nc.dram_tensor("wd3T", [D2, OUT], f32, kind="ExternalInput")
    b_d3_d = nc.dram_tensor("b_d3", [OUT], f32, kind="ExternalInput")
    out_d = nc.dram_tensor("out", [BL, OUT], f32, kind="ExternalOutput")

    import contextlib
    with tile.TileContext(nc) as tc, contextlib.ExitStack() as ctx:
        singles = ctx.enter_context(tc.tile_pool(name="singles", bufs=1))
        prep = ctx.enter_context(tc.tile_pool(name="prep", bufs=1))
        trans = ctx.enter_context(tc.tile_pool(name="trans", bufs=1))
        dbuf = ctx.enter_context(tc.tile_pool(name="dbuf", bufs=2))
        small = ctx.enter_context(tc.tile_pool(name="small", bufs=2))
        ps_pg = ctx.enter_context(tc.tile_pool(name="ps_pg", bufs=3, space="PSUM"))
        ps_pp = ctx.enter_context(tc.tile_pool(name="ps_pp", bufs=2, space="PSUM"))
        dpool = ctx.enter_context(tc.tile_pool(name="dpool", bufs=1, space="DRAM"))

        def pg_tile(shape, name):
            return ps_pg.tile(shape, f32, tag="pg", name=name)

        def pp_tile(shape, name):
            return ps_pp.tile(shape, f32, tag="pp", name=name)

        # ---------------- constants / weights ----------------
        ident = singles.tile([128, 128], f32)
        make_identity(nc, ident)
        eps_col = singles.tile([T, 1], f32)
        nc.vector.memset(eps_col, EPS)

        def load_col(dram_vec, n, name):
            t_ = singles.tile([n, 1], f32, name=name, tag=name)
            nc.sync.dma_start(out=t_, in_=dram_vec[:].rearrange("(p o) -> p o", o=1))
            return t_

        b_d1_c = load_col(b_d1_d, D1, "b_d1_c")
        b_d2_c = load_col(b_d2_d, D2, "b_d2_c")
        b_d3_c = load_col(b_d3_d, OUT, "b_d3_c")

        wc10 = singles.tile([KA, 4, H], bf16, name="wc10", tag="wc10")
        nc.sync.dma_start(out=wc10,
                          in_=wc10_d[:, :].rearrange("p (c m) -> p c m", c=4))
        def load_mmdt(dram, name):
            # DMA the raw f32 weights, then DVE-copy into an f32r tile (the
            # copy performs the required fp32r rounding for PE consumption).
            raw = prep.tile([H, G4], f32, tag="wraw", name=f"{name}_raw")
            nc.sync.dma_start(out=raw, in_=dram[:, :])
            w_ = singles.tile([H, 4, H], mmdt, name=name, tag=name)
            nc.vector.tensor_copy(
                out=w_[:, :, :].rearrange("p c m -> p (c m)"), in_=raw)
            return w_

        wih1T = load_mmdt(wih1T_d, "wih1T")
        whhT = [load_mmdt(whh0T_d, "whhT0"), load_mmdt(whh1T_d, "whhT1")]
        beff1 = singles.tile([H, 4], f32, name="beff1", tag="beff1")
        nc.sync.dma_start(out=beff1, in_=beff1_d[:, :])
        statc = singles.tile([T, NSC], f32, name="statc", tag="statc")
        nc.sync.dma_start(out=statc, in_=statc_d[:, :])
        def load_mmdt2(dram, p, n, name):
            raw = prep.tile([p, n], f32, tag="wraw2", name=f"{name}_raw")
            nc.sync.dma_start(out=raw, in_=dram[:, :])
            w_ = singles.tile([p, n], mmdt, name=name, tag=name)
            nc.vector.tensor_copy(out=w_, in_=raw)
            return w_

        wd1T = load_mmdt2(wd1T_d, H, D1, "wd1T")
        wd2T = load_mmdt2(wd2T_d, D1, D2, "wd2T")
        wd3T = load_mmdt2(wd3T_d, D2, OUT, "wd3T")

        # statc column layout (must match host packing in kernel()):
        #   0..27  : R'_ij rows i=0..6, j=i..6 (upper-tri, row-major)
        #   28..34 : s'_i
        #   35..41 : wsum'_f  (= -wsum_f/H)
        #   42     : bsum'    (= -bsum/H)
        #   43     : c0''     (= (c0-|s|^2)/H)
        _roff = [0, 7, 13, 18, 22, 25, 27]

        def sc(j):
            return statc[:T_steps, j:j + 1]

        # ---------------- x load ([T part, F, BL], contiguous) ----------
        x_ftb = singles.tile([T, F, BL], f32)
        for fi in range(F):
            nc.sync.dma_start(out=x_ftb[:, fi, :], in_=xT_d[:, fi, :])

        def xf(fi):
            return x_ftb[:T_steps, fi, :]

        TS = T_steps

        # ---------------- prepass: LN stats in [T, BL] layout ------------
        # nmu = sum_f wsum'_f x_f + bsum'   (wsum' = -wsum/H)
        # y_i = sum_{j>=i} R'_ij x_j + s'_i ; q/H = sum_i y_i^2 + c0''
        # Chain seeds run on ACT (Identity with per-partition scale+bias),
        # chain continuations on DVE (scalar_tensor_tensor with AP scalar),
        # squares on ACT, square-sums and products on GPSIMD.
        nmu_all = singles.tile([T, BL], f32)
        r_all = singles.tile([T, BL], f32)
        nc.scalar.activation(out=nmu_all[:TS], in_=xf(0), func=AF.Identity,
                             scale=sc(35), bias=sc(42))
        for fi in range(1, F):
            nc.vector.scalar_tensor_tensor(
                out=nmu_all[:TS], in0=xf(fi), scalar=sc(35 + fi),
                in1=nmu_all[:TS], op0=ALU.mult, op1=ALU.add)
        sqs = []
        for i in range(F):
            z = prep.tile([T, BL], f32, tag=f"stz{i % 4}", name=f"st_z{i}")
            nc.scalar.activation(out=z[:TS], in_=xf(i), func=AF.Identity,
                                 scale=sc(_roff[i]), bias=sc(28 + i))
            for j in range(i + 1, F):
                nc.vector.scalar_tensor_tensor(
                    out=z[:TS], in0=xf(j), scalar=sc(_roff[i] + j - i),
                    in1=z[:TS], op0=ALU.mult, op1=ALU.add)
            sq = prep.tile([T, BL], f32, tag=f"stsq{i % 3}", name=f"st_sq{i}")
            nc.scalar.activation(out=sq[:TS], in_=z[:TS], func=AF.Square,
                                 scale=1.0)
            sqs.append(sq)
            if i == 1:
                qv = prep.tile([T, BL], f32, tag="stqv", name="st_qv")
                nc.gpsimd.tensor_add(out=qv[:TS], in0=sqs[0][:TS],
                                     in1=sqs[1][:TS])
            elif i > 1:
                nc.gpsimd.tensor_add(out=qv[:TS], in0=qv[:TS], in1=sq[:TS])
        # var = q/H + c0'' - mu^2
        musq = prep.tile([T, BL], f32, tag="stz0", name="st_musq")
        nc.gpsimd.tensor_tensor(out=musq[:TS], in0=nmu_all[:TS],
                                in1=nmu_all[:TS], op=ALU.mult)
        nc.vector.tensor_scalar_add(out=qv[:TS], in0=qv[:TS], scalar1=sc(43))
        nc.vector.tensor_sub(out=qv[:TS], in0=qv[:TS], in1=musq[:TS])
        nc.scalar.activation(out=r_all[:TS], in_=qv[:TS], func=AF.Sqrt,
                             bias=eps_col[:TS], scale=1.0)
        nc.vector.reciprocal(out=r_all[:TS], in_=r_all[:TS])

        # ---------------- augmented input xa = [r*x; r; nmr; 1] ----------
        xa = singles.tile([T, KA, BL], bf16)
        for fi in range(F):
            eng = nc.vector if fi % 2 == 0 else nc.gpsimd
            eng.tensor_tensor(out=xa[:TS, fi, :], in0=xf(fi), in1=r_all[:TS],
                              op=ALU.mult)
        nc.vector.tensor_copy(out=xa[:TS, F, :], in_=r_all[:TS])
        nc.gpsimd.tensor_tensor(out=xa[:TS, F + 1, :], in0=nmu_all[:TS],
                                in1=r_all[:TS], op=ALU.mult)
        nc.vector.memset(xa[:TS, F + 2, :], 1.0)
        xa_dram = dpool.tile([T, KA, BL], bf16)
        nc.sync.dma_start(out=xa_dram[:TS], in_=xa[:TS])

        # ---------------- states ----------------
        h1 = singles.tile([H, BL], mmdt, name="h1", tag="h1")
        c = [singles.tile([H, BL], f32, name="c0", tag="c0"),
             singles.tile([H, BL], f32, name="c1", tag="c1")]
        zinit = trans.tile([H, BL], f32, tag="hf32", name="zinit")
        nc.vector.memset(zinit, 0.0)
        h0_prev = dbuf.tile([H, BL], mmdt, tag="h0", name="h0_init")
        nc.vector.tensor_copy(out=h0_prev, in_=zinit)
        nc.vector.tensor_copy(out=h1, in_=zinit)
        for L in range(2):
            nc.vector.memset(c[L], 0.0)

        # ---------------- main loop ----------------
        # Software-pipelined emission: each engine's FIFO sees work in an
        # order that never head-of-line-blocks.  Per iteration t:
        #   PE : 16 mm for L0(t), then 16 mm for L1(t-1)
        #   ACT: si0 sf0 tg0 so0 | si1 sf1 | tanh_c0 | tg1 so1 | tanh_c1
        #   DVE: u0 c0 h0 u1 c1 h1
        #   Pool: v0 v1
        # tanh_c0 sits mid-iteration so h0(t) completes early enough for
        # L0(t+1)'s matmuls to feed ACT without a wrap-around gap.
        GF = [AF.Sigmoid, AF.Sigmoid, AF.Tanh, AF.Sigmoid]

        def mm_gates(L, inp, inpT, hprev, hh_first):
            pgs = []
            for gc in range(4):
                pg = pg_tile([H, BL], f"pg{L}_g{gc}")
                for hc in range(NH):
                    sl = slice(hc * 512, (hc + 1) * 512)
                    ops = [(inpT[:, gc, :], inp),
                           (whhT[L][:, gc, :], hprev)]
                    if hh_first:
                        ops.reverse()
                    nc.tensor.matmul(pg[:, sl], ops[0][0], ops[0][1][:, sl],
                                     start=True, stop=False)
                    nc.tensor.matmul(pg[:, sl], ops[1][0], ops[1][1][:, sl],
                                     start=False, stop=True)
                pgs.append(pg)
            return pgs

        def act_gate(L, pgs, gc):
            o = trans.tile([H, BL], bf16, tag=f"sg{L}{gc}", name=f"sg{L}{gc}")
            if L == 0:
                nc.scalar.activation(out=o, in_=pgs[gc], func=GF[gc], scale=1.0)
            else:
                nc.scalar.activation(out=o, in_=pgs[gc], func=GF[gc],
                                     bias=beff1[:, gc:gc + 1], scale=1.0)
            return o

        # prefetch ring for xaug
        PF = 3
        xaug_tiles = {}

        def issue_xaug(t):
            if t >= T_steps:
                return
            xt = trans.tile([KA, BL], bf16, tag=f"xaug{t % PF}", name="xaug")
            nc.sync.dma_start(out=xt, in_=xa_dram[t])
            xaug_tiles[t] = xt

        for t in range(2):
            issue_xaug(t)

        def cell_front(L, sg):
            # u = si*tg (DVE), v = sf*c (Pool), c = u+v (DVE)
            u = trans.tile([H, BL], bf16, tag=f"u{L}", name=f"u{L}")
            nc.vector.tensor_tensor(out=u, in0=sg[0], in1=sg[2], op=ALU.mult)
            v_ = trans.tile([H, BL], f32, tag=f"v{L}", name=f"v{L}")
            nc.gpsimd.tensor_tensor(out=v_, in0=sg[1], in1=c[L], op=ALU.mult)
            nc.vector.tensor_add(out=c[L], in0=u, in1=v_)

        def cell_tanh(L):
            tc_ = trans.tile([H, BL], bf16, tag=f"tc{L}", name=f"tc{L}")
            nc.scalar.activation(out=tc_, in_=c[L], func=AF.Tanh, scale=1.0)
            return tc_

        def cell_h(sg, tc_, hout):
            nc.vector.tensor_tensor(out=hout, in0=sg[3], in1=tc_, op=ALU.mult)

        sg1 = None
        for t in range(T_steps):
            issue_xaug(t + 2)
            # PE: layer-0 step t gates, then layer-1 step t-1 gates
            pg0 = mm_gates(0, xaug_tiles.pop(t), wc10, h0_prev, hh_first=False)
            sg0 = [act_gate(0, pg0, 0), act_gate(0, pg0, 1)]
            sg0.append(act_gate(0, pg0, 2))
            sg0.append(act_gate(0, pg0, 3))
            cell_front(0, sg0)
            if t > 0:
                pg1 = mm_gates(1, h0_prev, wih1T, h1,
                               hh_first=True)
                sg1 = [act_gate(1, pg1, 0), act_gate(1, pg1, 1)]
            tc0 = cell_tanh(0)
            h0_new = dbuf.tile([H, BL], mmdt, tag="h0", name="h0_new")
            cell_h(sg0, tc0, h0_new)
            if t > 0:
                sg1.append(act_gate(1, pg1, 2))
                sg1.append(act_gate(1, pg1, 3))
                cell_front(1, sg1)
                tc1 = cell_tanh(1)
                cell_h(sg1, tc1, h1)
            h0_prev = h0_new
        # drain: final layer-1 step
        pg1 = mm_gates(1, h0_prev, wih1T, h1, hh_first=True)
        sg1 = [act_gate(1, pg1, g) for g in range(4)]
        cell_front(1, sg1)
        tc1 = cell_tanh(1)
        cell_h(sg1, tc1, h1)

        # ---------------- head ----------------
        h1f = trans.tile([H, BL], f32, tag="hf32", name="h1f")
        nc.vector.tensor_copy(out=h1f, in_=h1.bitcast(f32))
        sqh = prep.tile([H, BL], f32, tag="ha", name="sqh")
        nc.vector.tensor_tensor(out=sqh, in0=h1f, in1=h1f, op=ALU.mult)
        ones_col = small.tile([H, 1], f32, tag="ones_col", name="ones_col")
        nc.vector.memset(ones_col, 1.0)
        ps_s1 = pg_tile([1, BL], "ps_s1")
        ps_s2 = pg_tile([1, BL], "ps_s2")
        for hc in range(NH):
            sl = slice(hc * 512, (hc + 1) * 512)
            nc.tensor.matmul(ps_s1[:, sl], ones_col, h1f[:, sl],
                             start=True, stop=True, skip_group_check=True)
            nc.tensor.matmul(ps_s2[:, sl], ones_col, sqh[:, sl],
                             start=True, stop=True, skip_group_check=True)
        nmu_h = small.tile([1, BL], f32, tag="nmu_h", name="nmu_h")
        nc.vector.tensor_scalar_mul(out=nmu_h, in0=ps_s1, scalar1=-1.0 / H)
        musq_h = small.tile([1, BL], f32, tag="musq", name="musq_h")
        nc.vector.tensor_tensor(out=musq_h, in0=nmu_h, in1=nmu_h, op=ALU.mult)
        v_h = small.tile([1, BL], f32, tag="v_h", name="v_h")
        nc.vector.tensor_scalar_mul(out=v_h, in0=ps_s2, scalar1=1.0 / H)
        nc.vector.tensor_sub(out=v_h, in0=v_h, in1=musq_h)
        nc.scalar.activation(out=v_h, in_=v_h, func=AF.Sqrt,
                             bias=eps_col[0:1], scale=1.0)
        nc.vector.reciprocal(out=v_h, in_=v_h)
        nmbc = prep.tile([H, BL], f32, tag="hb", name="nmbc")
        nc.gpsimd.partition_broadcast(nmbc, nmu_h)
        rhbc = prep.tile([H, BL], f32, tag="hc", name="rhbc")
        nc.gpsimd.partition_broadcast(rhbc, v_h)
        t1 = prep.tile([H, BL], f32, tag="hd", name="t1")
        nc.vector.tensor_tensor(out=t1, in0=h1f, in1=nmbc, op=ALU.add)
        last = prep.tile([H, BL], mmdt, tag="hb", name="last")
        nc.vector.tensor_tensor(out=last, in0=t1, in1=rhbc, op=ALU.mult)
        pd1 = pg_tile([D1, BL], "pd1")
        for hc in range(NH):
            sl = slice(hc * 512, (hc + 1) * 512)
            nc.tensor.matmul(pd1[:, sl], wd1T, last[:, sl], start=True, stop=True,
                             skip_group_check=True)
        d1 = prep.tile([D1, BL], mmdt, tag="hc", name="d1")
        nc.scalar.activation(out=d1, in_=pd1, func=AF.Relu, bias=b_d1_c, scale=1.0)
        pd2 = pg_tile([D2, BL], "pd2")
        for hc in range(NH):
            sl = slice(hc * 512, (hc + 1) * 512)
            nc.tensor.matmul(pd2[:, sl], wd2T, d1[:, sl], start=True, stop=True,
                             skip_group_check=True)
        d2 = prep.tile([D2, BL], mmdt, tag="hd", name="d2")
        nc.scalar.activation(out=d2, in_=pd2, func=AF.Relu, bias=b_d2_c, scale=1.0)
        pd3 = pg_tile([OUT, BL], "pd3")
        for hc in range(NH):
            sl = slice(hc * 512, (hc + 1) * 512)
            nc.tensor.matmul(pd3[:, sl], wd3T, d2[:, sl], start=True, stop=True,
                             skip_group_check=True)
        o3 = prep.tile([OUT, BL], f32, tag="ha", name="o3")
        nc.scalar.activation(out=o3, in_=pd3, func=AF.Identity, bias=b_d3_c,
                             scale=1.0)
        outT = singles.tile([128, QB, OUT], f32)
        for q in range(QB):
            pot = pp_tile([128, OUT], "pot")
            nc.tensor.transpose(pot, o3[:, q * 128:(q + 1) * 128],
                                ident[:OUT, :OUT])
            nc.vector.tensor_copy(out=outT[:, q, :], in_=pot)
        nc.sync.dma_start(
            out=out_d[:, :].rearrange("(q p) c -> p q c", p=128),
            in_=outT)
    return nc


_CACHE = {}


def _fold_weights(inp):
    """Host-side weight-only preprocessing (float64). Returns the dict of
    derived dram inputs (excluding xT, which is per-core)."""
    import ml_dtypes
    d = {k: np.asarray(v, np.float64) for k, v in inp.items()}
    W = d["W_in"]                       # [H, F]
    g, b, be = d["g_in"], d["b_in"], d["be_in"]
    Wih0, Whh0 = d["Wih0"], d["Whh0"]   # [4H, H]
    Wih1, Whh1 = d["Wih1"], d["Whh1"]

    # wc10 rows: A = Wih0 diag(g) W, u = Wih0 (g*b), v = Wih0 g,
    #            beff0 = bih0 + bhh0 + Wih0 be
    Wg = Wih0 * g[None, :]              # [4H, H] (columns scaled)
    A = Wg @ W                          # [4H, F]
    u = Wg @ b
    v = Wg @ np.ones(H)
    beff0 = d["bih0"] + d["bhh0"] + Wih0 @ be
    wc10 = np.concatenate([A.T, u[None], v[None], beff0[None]], axis=0)  # [10, 4H]

    # stats constants: M = W^T W, wsum = 1^T W, l = W^T b, c0 = |b|^2
    M = W.T @ W
    wsum = W.sum(axis=0)
    l = W.T @ b
    c0 = float(b @ b)
    R = np.linalg.cholesky(M).T         # upper-tri: M = R^T R
    s = np.linalg.solve(R.T, l)         # R^T s = l
    sH = np.sqrt(float(H))
    Rp, sp = R / sH, s / sH
    bsum = float(b.sum())
    cols = []
    for i in range(F):
        cols.extend(Rp[i, i:])          # 28 upper-tri entries
    cols += list(sp)                    # 7 s'
    cols += list(-wsum / H)             # 7 wsum'
    cols += [-bsum / H, (c0 - float(s @ s)) / H]
    statc_row = np.asarray(cols, np.float64)
    assert statc_row.shape[0] == NSC
    statc = np.tile(statc_row[None, :], (T, 1))

    beff1 = (d["bih1"] + d["bhh1"]).reshape(4, H).T  # [H, 4]

    Wd1g = d["W_d1"] * d["g_ln"][None, :]
    bd1p = d["b_d1"] + d["W_d1"] @ d["be_ln"]
    out = {
        "wc10": wc10.astype(ml_dtypes.bfloat16),
        "wih1T": np.ascontiguousarray(Wih1.T).astype(np.float32),
        "whh0T": np.ascontiguousarray(Whh0.T).astype(np.float32),
        "whh1T": np.ascontiguousarray(Whh1.T).astype(np.float32),
        "beff1": np.ascontiguousarray(beff1).astype(np.float32),
        "statc": statc.astype(np.float32),
        "wd1T": np.ascontiguousarray(Wd1g.T).astype(np.float32),
        "b_d1": bd1p.astype(np.float32),
        "wd2T": np.ascontiguousarray(d["W_d2"].T).astype(np.float32),
        "b_d2": d["b_d2"].astype(np.float32),
        "wd3T": np.ascontiguousarray(d["W_d3"].T).astype(np.float32),
        "b_d3": d["b_d3"].astype(np.float32),
    }
    return out


def core_val(inp, name, ci, folded=None):
    """Per-core value for dram input `name` (inp: full raw-input dict)."""
    if name == "xT":
        return np.ascontiguousarray(
            np.asarray(inp["x"], np.float32)[ci * BL:(ci + 1) * BL]
            .transpose(1, 2, 0))
    if folded is None:
        folded = _fold_weights(inp)
    return folded[name]


def _get_runner():
    if "runner" in _CACHE:
        return _CACHE["runner"]
    import jax
    from jax.sharding import Mesh, PartitionSpec
    from jax.experimental.shard_map import shard_map
    import concourse.bacc as bacc
    import concourse.mybir as mybir
    from concourse.bass2jax import install_neuronx_cc_hook, _bass_exec_p, \
        partition_id_tensor

    nc = bacc.Bacc()
    _build(nc)
    nc.compile()
    install_neuronx_cc_hook()

    partition_name = nc.partition_id_tensor.name if nc.partition_id_tensor else None
    in_names, out_names, out_avals, zero_outs = [], [], [], []
    for alloc in nc.m.functions[0].allocations:
        if not isinstance(alloc, mybir.MemoryLocationSet):
            continue
        name = alloc.memorylocations[0].name
        if alloc.kind == "ExternalInput":
            if name != partition_name:
                in_names.append(name)
        elif alloc.kind == "ExternalOutput":
            out_names.append(name)
            shape = tuple(alloc.tensor_shape)
            dtype = mybir.dt.np(alloc.dtype)
            out_avals.append(jax.core.ShapedArray(shape, dtype))
            zero_outs.append(np.zeros(shape, dtype))
    n_params = len(in_names)
    all_in_names = in_names + out_names + ([partition_name] if partition_name else [])

    def _body(*args):
        operands = list(args)
        if partition_name is not None:
            operands.append(partition_id_tensor())
        outs = _bass_exec_p.bind(
            *operands,
            out_avals=tuple(out_avals),
            in_names=tuple(all_in_names),
            out_names=tuple(out_names),
            lowering_input_output_aliases=(),
            sim_require_finite=False,
            sim_require_nnan=False,
            nc=nc,
        )
        return tuple(outs)

    devices = jax.devices()[:NCORES]
    mesh = Mesh(np.asarray(devices), ("core",))
    in_specs = (PartitionSpec("core"),) * (n_params + len(out_names))
    out_specs = (PartitionSpec("core"),) * len(out_names)
    sharded = jax.jit(
        shard_map(_body, mesh=mesh, in_specs=in_specs, out_specs=out_specs,
                  check_rep=False),
        keep_unused=True)
    _CACHE["runner"] = (sharded, in_names, out_names, zero_outs)
    return _CACHE["runner"]


def kernel(**inputs) -> np.ndarray:
    sharded, in_names, out_names, zero_outs = _get_runner()
    inp = {k: np.asarray(v) for k, v in inputs.items()}
    folded = _fold_weights(inp)

    concat_in = [
        np.concatenate([core_val(inp, n, ci, folded) for ci in range(NCORES)],
                       axis=0)
        for n in in_names
    ]
    concat_zeros = [
        np.zeros((NCORES * z.shape[0], *z.shape[1:]), z.dtype) for z in zero_outs
    ]
    import jax
    out_arrs = sharded(*concat_in, *concat_zeros)
    jax.block_until_ready(out_arrs)
    oi = out_names.index("out")
    full = np.asarray(out_arrs[oi]).reshape(B, OUT)
    return full.astype(np.float32)


# revision 16
# speedup vs baseline: 1.5486x; 1.0079x over previous
"""DepletionLSTM Trainium2 kernel (v2 — ACT-roof design).

Self-contained: builds a Bass/Tile kernel for the 2-layer-LSTM network,
shards the batch over 8 NeuronCores (pure data parallelism), runs via
PJRT/axon, returns the full [8192, 30] float32 output.

Strategy (per core, 1024 batch):
- Host stages x transposed to [T, F, BL] per core, and folds all
  weight-only expressions (transposes + the fused input-pipeline matrix)
  once in float64 — standard compile-time weight preprocessing.  All
  x-dependent math runs on device.
- The entire input pipeline (W_in projection + LayerNorm + layer-0 input
  matmul + layer-0 gate biases) collapses into ONE K=10 matmul per gate:
    zin0 = wc10^T @ [r*x; r; -mu*r; 1]
  with wc10 rows [A; u; v; beff0], A = Wih0 diag(g_in) W_in [4H x 7],
  u = Wih0 (g_in*b_in), v = Wih0 g_in.
- LN stats (mu, rstd) are computed in a [T=90 part, BL] prepass using a
  Cholesky factorization of the quadratic form:  sum_h p_h^2 = |R x + s|^2
  + const, so the per-row squares run on the otherwise-idle ACT engine and
  the linear chains split across DVE and GPSIMD.
- The augmented input xa = [r*x; r; nmr; 1] (bf16) is staged to DRAM once
  and streamed back per step as a [10, BL] tile (one DMA per step,
  double-buffered).
- Per step per layer: 4 accumulating gate matmul pairs (input + recurrent,
  N=512 chunks; input side bf16, recurrent fp32r), 4 sigmoid/tanh ACT ops
  (bf16 out), tanh(c) ACT, and 3 DVE ops (u=si*tg in bf16 2x-mode, c=u+v,
  h=so*tc) plus v=sf*c on GPSIMD.  Layer 1 runs one timestep behind layer
  0 so both layers' work interleaves; ACT (the only sigmoid/tanh engine)
  is the roofline at ~10.4us/step.
- PSUM: 3 rotating gate tiles [128,1024] (6 banks) + 2 utility banks.
"""
import sys
sys.path.insert(0, '/opt/trn_rl_repo')

import numpy as np

B, T, F, H, D1, D2, OUT = 8192, 90, 7, 128, 128, 64, 30
NCORES = 8
BL = B // NCORES
G4 = 4 * H
NH = BL // 512
QB = BL // 128
EPS = 1e-5
MMDT = "float32r"
KA = F + 3  # augmented-input rows: 7 x-rows, r, nmr, ones
NSC = 44    # stat-constant columns: 28 R + 7 s + 7 wsum' + bsum' + c0''


def _build(nc, T_steps=T, mmdt_name=MMDT, dbg=False):
    import concourse.tile as tile
    from concourse import mybir
    from concourse.masks import make_identity

    f32 = mybir.dt.float32
    bf16 = mybir.dt.bfloat16
    mmdt = getattr(mybir.dt, mmdt_name)
    AF = mybir.ActivationFunctionType
    ALU = mybir.AluOpType

    # ---------------- DRAM I/O (host-folded weights) ----------------
    xT_d = nc.dram_tensor("xT", [T, F, BL], f32, kind="ExternalInput")
    wc10_d = nc.dram_tensor("wc10", [KA, G4], bf16, kind="ExternalInput")
    wih1T_d = nc.dram_tensor("wih1T", [H, G4], f32, kind="ExternalInput")
    whh0T_d = nc.dram_tensor("whh0T", [H, G4], f32, kind="ExternalInput")
    whh1T_d = nc.dram_tensor("whh1T", [H, G4], f32, kind="ExternalInput")
    beff1_d = nc.dram_tensor("beff1", [H, 4], f32, kind="ExternalInput")
    statc_d = nc.dram_tensor("statc", [T, NSC], f32, kind="ExternalInput")
    wd1T_d = nc.dram_tensor("wd1T", [H, D1], f32, kind="ExternalInput")
    b_d1_d = nc.dram_tensor("b_d1", [D1], f32, kind="ExternalInput")
    wd2T_d = nc.dram_tensor("wd2T", [D1, D2], f32, kind="ExternalInput")
    b_d2_d = nc.dram_tensor("b_d2", [D2], f32, kind="ExternalInput")
    wd3T_d = nc.dram_tensor("wd3T", [D2, OUT], f32, kind="ExternalInput")
    b_d3_d = nc.dram_tensor("b_d3", [OUT], f32, kind="ExternalInput")
    out_d = nc.dram_tensor("out", [BL, OUT], f32, kind="ExternalOutput")

    import contextlib
    with tile.TileContext(nc) as tc, contextlib.ExitStack() as ctx:
        singles = ctx.enter_context(tc.tile_pool(name="singles", bufs=1))
        prep = ctx.enter_context(tc.tile_pool(name="prep", bufs=1))
        trans = ctx.enter_context(tc.tile_pool(name="trans", bufs=1))
        dbuf = ctx.enter_context(tc.tile_pool(name="dbuf", bufs=2))
        small = ctx.enter_context(tc.tile_pool(name="small", bufs=2))
        ps_pg = ctx.enter_context(tc.tile_pool(name="ps_pg", bufs=3, space="PSUM"))
        ps_pp = ctx.enter_context(tc.tile_pool(name="ps_pp", bufs=2, space="PSUM"))
        dpool = ctx.enter_context(tc.tile_pool(name="dpool", bufs=1, space="DRAM"))

        def pg_tile(shape, name):
            return ps_pg.tile(shape, f32, tag="pg", name=name)

        def pp_tile(shape, name):
            return ps_pp.tile(shape, f32, tag="pp", name=name)

        # ---------------- constants / weights ----------------
        ident = singles.tile([128, 128], f32)
        make_identity(nc, ident)
        eps_col = singles.tile([T, 1], f32)
        nc.vector.memset(eps_col, EPS)

        def load_col(dram_vec, n, name):
            t_ = singles.tile([n, 1], f32, name=name, tag=name)
            nc.sync.dma_start(out=t_, in_=dram_vec[:].rearrange("(p o) -> p o", o=1))
            return t_

        b_d1_c = load_col(b_d1_d, D1, "b_d1_c")
        b_d2_c = load_col(b_d2_d, D2, "b_d2_c")
        b_d3_c = load_col(b_d3_d, OUT, "b_d3_c")

        wc10 = singles.tile([KA, 4, H], bf16, name="wc10", tag="wc10")
        nc.sync.dma_start(out=wc10,
                          in_=wc10_d[:, :].rearrange("p (c m) -> p c m", c=4))
        def load_mmdt(dram, name):
            # DMA the raw f32 weights, then DVE-copy into an f32r tile (the
            # copy performs the required fp32r rounding for PE consumption).
            raw = prep.tile([H, G4], f32, tag="wraw", name=f"{name}_raw")
            nc.sync.dma_start(out=raw, in_=dram[:, :])
            w_ = singles.tile([H, 4, H], mmdt, name=name, tag=name)
            nc.vector.tensor_copy(
                out=w_[:, :, :].rearrange("p c m -> p (c m)"), in_=raw)
            return w_

        wih1T = load_mmdt(wih1T_d, "wih1T")
        whhT = [load_mmdt(whh0T_d, "whhT0"), load_mmdt(whh1T_d, "whhT1")]
        beff1 = singles.tile([H, 4], f32, name="beff1", tag="beff1")
        nc.sync.dma_start(out=beff1, in_=beff1_d[:, :])
        statc = singles.tile([T, NSC], f32, name="statc", tag="statc")
        nc.sync.dma_start(out=statc, in_=statc_d[:, :])
        def load_mmdt2(dram, p, n, name):
            raw = prep.tile([p, n], f32, tag="wraw2", name=f"{name}_raw")
            nc.sync.dma_start(out=raw, in_=dram[:, :])
            w_ = singles.tile([p, n], mmdt, name=name, tag=name)
            nc.vector.tensor_copy(out=w_, in_=raw)
            return w_

        wd1T = load_mmdt2(wd1T_d, H, D1, "wd1T")
        wd2T = load_mmdt2(wd2T_d, D1, D2, "wd2T")
        wd3T = load_mmdt2(wd3T_d, D2, OUT, "wd3T")

        # statc column layout (must match host packing in kernel()):
        #   0..27  : R'_ij rows i=0..6, j=i..6 (upper-tri, row-major)
        #   28..34 : s'_i
        #   35..41 : wsum'_f  (= -wsum_f/H)
        #   42     : bsum'    (= -bsum/H)
        #   43     : c0''     (= (c0-|s|^2)/H)
        _roff = [0, 7, 13, 18, 22, 25, 27]

        def sc(j):
            return statc[:T_steps, j:j + 1]

        # ---------------- x load ([T part, F, BL], contiguous) ----------
        x_ftb = singles.tile([T, F, BL], f32)
        for fi in range(F):
            nc.sync.dma_start(out=x_ftb[:, fi, :], in_=xT_d[:, fi, :])

        def xf(fi):
            return x_ftb[:T_steps, fi, :]

        TS = T_steps

        # ---------------- prepass: LN stats in [T, BL] layout ------------
        # nmu = sum_f wsum'_f x_f + bsum'   (wsum' = -wsum/H)
        # y_i = sum_{j>=i} R'_ij x_j + s'_i ; q/H = sum_i y_i^2 + c0''
        # Chain seeds run on ACT (Identity with per-partition scale+bias),
        # chain continuations on DVE (scalar_tensor_tensor with AP scalar),
        # squares on ACT, square-sums and products on GPSIMD.
        nmu_all = singles.tile([T, BL], f32)
        r_all = singles.tile([T, BL], f32)
        nc.scalar.activation(out=nmu_all[:TS], in_=xf(0), func=AF.Identity,
                             scale=sc(35), bias=sc(42))
        for fi in range(1, F):
            nc.vector.scalar_tensor_tensor(
                out=nmu_all[:TS], in0=xf(fi), scalar=sc(35 + fi),
                in1=nmu_all[:TS], op0=ALU.mult, op1=ALU.add)
        sqs = []
        for i in range(F):
            z = prep.tile([T, BL], f32, tag=f"stz{i % 4}", name=f"st_z{i}")
            nc.scalar.activation(out=z[:TS], in_=xf(i), func=AF.Identity,
                                 scale=sc(_roff[i]), bias=sc(28 + i))
            for j in range(i + 1, F):
                nc.vector.scalar_tensor_tensor(
                    out=z[:TS], in0=xf(j), scalar=sc(_roff[i] + j - i),
                    in1=z[:TS], op0=ALU.mult, op1=ALU.add)
            sq = prep.tile([T, BL], f32, tag=f"stsq{i % 3}", name=f"st_sq{i}")
            nc.scalar.activation(out=sq[:TS], in_=z[:TS], func=AF.Square,
                                 scale=1.0)
            sqs.append(sq)
            if i == 1:
                qv = prep.tile([T, BL], f32, tag="stqv", name="st_qv")
                nc.gpsimd.tensor_add(out=qv[:TS], in0=sqs[0][:TS],
                                     in1=sqs[1][:TS])
            elif i > 1:
                nc.gpsimd.tensor_add(out=qv[:TS], in0=qv[:TS], in1=sq[:TS])
        # var = q/H + c0'' - mu^2
        musq = prep.tile([T, BL], f32, tag="stz0", name="st_musq")
        nc.gpsimd.tensor_tensor(out=musq[:TS], in0=nmu_all[:TS],
                                in1=nmu_all[:TS], op=ALU.mult)
        nc.vector.tensor_scalar_add(out=qv[:TS], in0=qv[:TS], scalar1=sc(43))
        nc.vector.tensor_sub(out=qv[:TS], in0=qv[:TS], in1=musq[:TS])
        nc.scalar.activation(out=r_all[:TS], in_=qv[:TS], func=AF.Sqrt,
                             bias=eps_col[:TS], scale=1.0)
        nc.vector.reciprocal(out=r_all[:TS], in_=r_all[:TS])

        # ---------------- augmented input xa = [r*x; r; nmr; 1] ----------
        xa = singles.tile([T, KA, BL], bf16)
        for fi in range(F):
            eng = nc.vector if fi % 2 == 0 else nc.gpsimd
            eng.tensor_tensor(out=xa[:TS, fi, :], in0=xf(fi), in1=r_all[:TS],
                              op=ALU.mult)
        nc.vector.tensor_copy(out=xa[:TS, F, :], in_=r_all[:TS])
        nc.gpsimd.tensor_tensor(out=xa[:TS, F + 1, :], in0=nmu_all[:TS],
                                in1=r_all[:TS], op=ALU.mult)
        nc.vector.memset(xa[:TS, F + 2, :], 1.0)
        xa_dram = dpool.tile([T, KA, BL], bf16)
        for j in range(KA):
            nc.sync.dma_start(out=xa_dram[:TS, j, :], in_=xa[:TS, j, :])

        # ---------------- states ----------------
        h1 = singles.tile([H, BL], mmdt, name="h1", tag="h1")
        c = [singles.tile([H, BL], f32, name="c0", tag="c0"),
             singles.tile([H, BL], f32, name="c1", tag="c1")]
        zinit = trans.tile([H, BL], f32, tag="hf32", name="zinit")
        nc.vector.memset(zinit, 0.0)
        h0_prev = dbuf.tile([H, BL], mmdt, tag="h0", name="h0_init")
        nc.vector.tensor_copy(out=h0_prev, in_=zinit)
        nc.vector.tensor_copy(out=h1, in_=zinit)
        for L in range(2):
            nc.vector.memset(c[L], 0.0)

        # ---------------- main loop ----------------
        # Software-pipelined emission: each engine's FIFO sees work in an
        # order that never head-of-line-blocks.  Per iteration t:
        #   PE : 16 mm for L0(t), then 16 mm for L1(t-1)
        #   ACT: si0 sf0 tg0 so0 | si1 sf1 | tanh_c0 | tg1 so1 | tanh_c1
        #   DVE: u0 c0 h0 u1 c1 h1
        #   Pool: v0 v1
        # tanh_c0 sits mid-iteration so h0(t) completes early enough for
        # L0(t+1)'s matmuls to feed ACT without a wrap-around gap.
        GF = [AF.Sigmoid, AF.Sigmoid, AF.Tanh, AF.Sigmoid]

        def mm_gates(L, inp, inpT, hprev, hh_first):
            pgs = []
            for gc in range(4):
                pg = pg_tile([H, BL], f"pg{L}_g{gc}")
                for hc in range(NH):
                    sl = slice(hc * 512, (hc + 1) * 512)
                    ops = [(inpT[:, gc, :], inp),
                           (whhT[L][:, gc, :], hprev)]
                    if hh_first:
                        ops.reverse()
                    nc.tensor.matmul(pg[:, sl], ops[0][0], ops[0][1][:, sl],
                                     start=True, stop=False)
                    nc.tensor.matmul(pg[:, sl], ops[1][0], ops[1][1][:, sl],
                                     start=False, stop=True)
                pgs.append(pg)
            return pgs

        def act_gate(L, pgs, gc):
            o = trans.tile([H, BL], bf16, tag=f"sg{L}{gc}", name=f"sg{L}{gc}")
            if L == 0:
                nc.scalar.activation(out=o, in_=pgs[gc], func=GF[gc], scale=1.0)
            else:
                nc.scalar.activation(out=o, in_=pgs[gc], func=GF[gc],
                                     bias=beff1[:, gc:gc + 1], scale=1.0)
            return o

        # prefetch ring for xaug
        PF = 3
        xaug_tiles = {}

        def issue_xaug(t):
            if t >= T_steps:
                return
            xt = trans.tile([KA, BL], bf16, tag=f"xaug{t % PF}", name="xaug")
            nc.sync.dma_start(out=xt, in_=xa_dram[t])
            xaug_tiles[t] = xt

        for t in range(3):
            issue_xaug(t)

        def cell_front(L, sg):
            # u = si*tg (DVE), v = sf*c (Pool), c = u+v (DVE)
            u = trans.tile([H, BL], bf16, tag=f"u{L}", name=f"u{L}")
            nc.vector.tensor_tensor(out=u, in0=sg[0], in1=sg[2], op=ALU.mult)
            v_ = trans.tile([H, BL], f32, tag=f"v{L}", name=f"v{L}")
            nc.gpsimd.tensor_tensor(out=v_, in0=sg[1], in1=c[L], op=ALU.mult)
            nc.vector.tensor_add(out=c[L], in0=u, in1=v_)

        def cell_tanh(L):
            tc_ = trans.tile([H, BL], bf16, tag=f"tc{L}", name=f"tc{L}")
            nc.scalar.activation(out=tc_, in_=c[L], func=AF.Tanh, scale=1.0)
            return tc_

        def cell_h(sg, tc_, hout):
            nc.vector.tensor_tensor(out=hout, in0=sg[3], in1=tc_, op=ALU.mult)

        sg1 = None
        for t in range(T_steps):
            issue_xaug(t + 2)
            # PE: layer-0 step t gates, then layer-1 step t-1 gates
            pg0 = mm_gates(0, xaug_tiles.pop(t), wc10, h0_prev, hh_first=False)
            sg0 = [act_gate(0, pg0, 0), act_gate(0, pg0, 1)]
            sg0.append(act_gate(0, pg0, 2))
            sg0.append(act_gate(0, pg0, 3))
            cell_front(0, sg0)
            if t > 0:
                pg1 = mm_gates(1, h0_prev, wih1T, h1,
                               hh_first=True)
                sg1 = [act_gate(1, pg1, 0), act_gate(1, pg1, 1)]
            tc0 = cell_tanh(0)
            h0_new = dbuf.tile([H, BL], mmdt, tag="h0", name="h0_new")
            cell_h(sg0, tc0, h0_new)
            if t > 0:
                sg1.append(act_gate(1, pg1, 2))
                sg1.append(act_gate(1, pg1, 3))
                cell_front(1, sg1)
                tc1 = cell_tanh(1)
                cell_h(sg1, tc1, h1)
            h0_prev = h0_new
        # drain: final layer-1 step
        pg1 = mm_gates(1, h0_prev, wih1T, h1, hh_first=True)
        sg1 = [act_gate(1, pg1, g) for g in range(4)]
        cell_front(1, sg1)
        tc1 = cell_tanh(1)
        cell_h(sg1, tc1, h1)

        # ---------------- head ----------------
        h1f = h1.bitcast(f32)
        sqh = prep.tile([H, BL], mmdt, tag="ha", name="sqh")
        nc.vector.tensor_tensor(out=sqh, in0=h1f, in1=h1f, op=ALU.mult)
        ones_f = small.tile([H, 1], f32, tag="ones_f", name="ones_f")
        nc.vector.memset(ones_f, 1.0)
        ones_col = small.tile([H, 1], mmdt, tag="ones_col", name="ones_col")
        nc.vector.tensor_copy(out=ones_col, in_=ones_f)
        ps_s1 = pg_tile([1, BL], "ps_s1")
        ps_s2 = pg_tile([1, BL], "ps_s2")
        for hc in range(NH):
            sl = slice(hc * 512, (hc + 1) * 512)
            nc.tensor.matmul(ps_s1[:, sl], ones_col, h1[:, sl],
                             start=True, stop=True, skip_group_check=True)
            nc.tensor.matmul(ps_s2[:, sl], ones_col, sqh[:, sl],
                             start=True, stop=True, skip_group_check=True)
        nmu_h = small.tile([1, BL], f32, tag="nmu_h", name="nmu_h")
        nc.vector.tensor_scalar_mul(out=nmu_h, in0=ps_s1, scalar1=-1.0 / H)
        musq_h = small.tile([1, BL], f32, tag="musq", name="musq_h")
        nc.vector.tensor_tensor(out=musq_h, in0=nmu_h, in1=nmu_h, op=ALU.mult)
        v_h = small.tile([1, BL], f32, tag="v_h", name="v_h")
        nc.vector.tensor_scalar_mul(out=v_h, in0=ps_s2, scalar1=1.0 / H)
        nc.vector.tensor_sub(out=v_h, in0=v_h, in1=musq_h)
        nc.scalar.activation(out=v_h, in_=v_h, func=AF.Sqrt,
                             bias=eps_col[0:1], scale=1.0)
        nc.vector.reciprocal(out=v_h, in_=v_h)
        nmbc = prep.tile([H, BL], f32, tag="hb", name="nmbc")
        nc.gpsimd.partition_broadcast(nmbc, nmu_h)
        rhbc = prep.tile([H, BL], f32, tag="hc", name="rhbc")
        nc.gpsimd.partition_broadcast(rhbc, v_h)
        t1 = prep.tile([H, BL], f32, tag="hd", name="t1")
        nc.vector.tensor_tensor(out=t1, in0=h1f, in1=nmbc, op=ALU.add)
        last = prep.tile([H, BL], mmdt, tag="hb", name="last")
        nc.vector.tensor_tensor(out=last, in0=t1, in1=rhbc, op=ALU.mult)
        pd1 = pg_tile([D1, BL], "pd1")
        for hc in range(NH):
            sl = slice(hc * 512, (hc + 1) * 512)
            nc.tensor.matmul(pd1[:, sl], wd1T, last[:, sl], start=True, stop=True,
                             skip_group_check=True)
        d1 = prep.tile([D1, BL], mmdt, tag="hc", name="d1")
        nc.scalar.activation(out=d1, in_=pd1, func=AF.Relu, bias=b_d1_c, scale=1.0)
        pd2 = pg_tile([D2, BL], "pd2")
        for hc in range(NH):
            sl = slice(hc * 512, (hc + 1) * 512)
            nc.tensor.matmul(pd2[:, sl], wd2T, d1[:, sl], start=True, stop=True,
                             skip_group_check=True)
        d2 = prep.tile([D2, BL], mmdt, tag="hd", name="d2")
        nc.scalar.activation(out=d2, in_=pd2, func=AF.Relu, bias=b_d2_c, scale=1.0)
        pd3 = pg_tile([OUT, BL], "pd3")
        for hc in range(NH):
            sl = slice(hc * 512, (hc + 1) * 512)
            nc.tensor.matmul(pd3[:, sl], wd3T, d2[:, sl], start=True, stop=True,
                             skip_group_check=True)
        o3 = prep.tile([OUT, BL], f32, tag="ha", name="o3")
        nc.scalar.activation(out=o3, in_=pd3, func=AF.Identity, bias=b_d3_c,
                             scale=1.0)
        outT = singles.tile([128, QB, OUT], f32)
        for q in range(QB):
            pot = pp_tile([128, OUT], "pot")
            nc.tensor.transpose(pot, o3[:, q * 128:(q + 1) * 128],
                                ident[:OUT, :OUT])
            nc.vector.tensor_copy(out=outT[:, q, :], in_=pot)
        nc.sync.dma_start(
            out=out_d[:, :].rearrange("(q p) c -> p q c", p=128),
            in_=outT)
    return nc


_CACHE = {}


def _fold_weights(inp):
    """Host-side weight-only preprocessing (float64). Returns the dict of
    derived dram inputs (excluding xT, which is per-core)."""
    import ml_dtypes
    d = {k: np.asarray(v, np.float64) for k, v in inp.items()}
    W = d["W_in"]                       # [H, F]
    g, b, be = d["g_in"], d["b_in"], d["be_in"]
    Wih0, Whh0 = d["Wih0"], d["Whh0"]   # [4H, H]
    Wih1, Whh1 = d["Wih1"], d["Whh1"]

    # wc10 rows: A = Wih0 diag(g) W, u = Wih0 (g*b), v = Wih0 g,
    #            beff0 = bih0 + bhh0 + Wih0 be
    Wg = Wih0 * g[None, :]              # [4H, H] (columns scaled)
    A = Wg @ W                          # [4H, F]
    u = Wg @ b
    v = Wg @ np.ones(H)
    beff0 = d["bih0"] + d["bhh0"] + Wih0 @ be
    wc10 = np.concatenate([A.T, u[None], v[None], beff0[None]], axis=0)  # [10, 4H]

    # stats constants: M = W^T W, wsum = 1^T W, l = W^T b, c0 = |b|^2
    M = W.T @ W
    wsum = W.sum(axis=0)
    l = W.T @ b
    c0 = float(b @ b)
    R = np.linalg.cholesky(M).T         # upper-tri: M = R^T R
    s = np.linalg.solve(R.T, l)         # R^T s = l
    sH = np.sqrt(float(H))
    Rp, sp = R / sH, s / sH
    bsum = float(b.sum())
    cols = []
    for i in range(F):
        cols.extend(Rp[i, i:])          # 28 upper-tri entries
    cols += list(sp)                    # 7 s'
    cols += list(-wsum / H)             # 7 wsum'
    cols += [-bsum / H, (c0 - float(s @ s)) / H]
    statc_row = np.asarray(cols, np.float64)
    assert statc_row.shape[0] == NSC
    statc = np.tile(statc_row[None, :], (T, 1))

    beff1 = (d["bih1"] + d["bhh1"]).reshape(4, H).T  # [H, 4]

    Wd1g = d["W_d1"] * d["g_ln"][None, :]
    bd1p = d["b_d1"] + d["W_d1"] @ d["be_ln"]
    out = {
        "wc10": wc10.astype(ml_dtypes.bfloat16),
        "wih1T": np.ascontiguousarray(Wih1.T).astype(np.float32),
        "whh0T": np.ascontiguousarray(Whh0.T).astype(np.float32),
        "whh1T": np.ascontiguousarray(Whh1.T).astype(np.float32),
        "beff1": np.ascontiguousarray(beff1).astype(np.float32),
        "statc": statc.astype(np.float32),
        "wd1T": np.ascontiguousarray(Wd1g.T).astype(np.float32),
        "b_d1": bd1p.astype(np.float32),
        "wd2T": np.ascontiguousarray(d["W_d2"].T).astype(np.float32),
        "b_d2": d["b_d2"].astype(np.float32),
        "wd3T": np.ascontiguousarray(d["W_d3"].T).astype(np.float32),
        "b_d3": d["b_d3"].astype(np.float32),
    }
    return out


def core_val(inp, name, ci, folded=None):
    """Per-core value for dram input `name` (inp: full raw-input dict)."""
    if name == "xT":
        return np.ascontiguousarray(
            np.asarray(inp["x"], np.float32)[ci * BL:(ci + 1) * BL]
            .transpose(1, 2, 0))
    if folded is None:
        folded = _fold_weights(inp)
    return folded[name]


def _get_runner():
    if "runner" in _CACHE:
        return _CACHE["runner"]
    import jax
    from jax.sharding import Mesh, PartitionSpec
    from jax.experimental.shard_map import shard_map
    import concourse.bacc as bacc
    import concourse.mybir as mybir
    from concourse.bass2jax import install_neuronx_cc_hook, _bass_exec_p, \
        partition_id_tensor

    nc = bacc.Bacc()
    _build(nc)
    nc.compile()
    install_neuronx_cc_hook()

    partition_name = nc.partition_id_tensor.name if nc.partition_id_tensor else None
    in_names, out_names, out_avals, zero_outs = [], [], [], []
    for alloc in nc.m.functions[0].allocations:
        if not isinstance(alloc, mybir.MemoryLocationSet):
            continue
        name = alloc.memorylocations[0].name
        if alloc.kind == "ExternalInput":
            if name != partition_name:
                in_names.append(name)
        elif alloc.kind == "ExternalOutput":
            out_names.append(name)
            shape = tuple(alloc.tensor_shape)
            dtype = mybir.dt.np(alloc.dtype)
            out_avals.append(jax.core.ShapedArray(shape, dtype))
            zero_outs.append(np.zeros(shape, dtype))
    n_params = len(in_names)
    all_in_names = in_names + out_names + ([partition_name] if partition_name else [])

    def _body(*args):
        operands = list(args)
        if partition_name is not None:
            operands.append(partition_id_tensor())
        outs = _bass_exec_p.bind(
            *operands,
            out_avals=tuple(out_avals),
            in_names=tuple(all_in_names),
            out_names=tuple(out_names),
            lowering_input_output_aliases=(),
            sim_require_finite=False,
            sim_require_nnan=False,
            nc=nc,
        )
        return tuple(outs)

    devices = jax.devices()[:NCORES]
    mesh = Mesh(np.asarray(devices), ("core",))
    in_specs = (PartitionSpec("core"),) * (n_params + len(out_names))
    out_specs = (PartitionSpec("core"),) * len(out_names)
    sharded = jax.jit(
        shard_map(_body, mesh=mesh, in_specs=in_specs, out_specs=out_specs,
                  check_rep=False),
        keep_unused=True)
    _CACHE["runner"] = (sharded, in_names, out_names, zero_outs)
    return _CACHE["runner"]


def kernel(**inputs) -> np.ndarray:
    sharded, in_names, out_names, zero_outs = _get_runner()
    inp = {k: np.asarray(v) for k, v in inputs.items()}
    folded = _fold_weights(inp)

    concat_in = [
        np.concatenate([core_val(inp, n, ci, folded) for ci in range(NCORES)],
                       axis=0)
        for n in in_names
    ]
    concat_zeros = [
        np.zeros((NCORES * z.shape[0], *z.shape[1:]), z.dtype) for z in zero_outs
    ]
    import jax
    out_arrs = sharded(*concat_in, *concat_zeros)
    jax.block_until_ready(out_arrs)
    oi = out_names.index("out")
    full = np.asarray(out_arrs[oi]).reshape(B, OUT)
    return full.astype(np.float32)


# revision 17
# speedup vs baseline: 1.5506x; 1.0013x over previous
"""DepletionLSTM Trainium2 kernel (ACT-roofline design).

Self-contained: builds a Bass/Tile kernel for the 2-layer-LSTM network,
shards the batch over 8 NeuronCores (pure data parallelism), runs via
PJRT/axon, returns the full [8192, 30] float32 output.

The ACT (scalar) engine is the hard roofline: 10 sigmoid/tanh ops per
timestep on [128,1024] tiles (~10.4us/step, ~940us total) — it is the
only engine with exp-family activations.  Everything else is organized
to keep ACT saturated:

- Host stages x transposed to [T, F, BL] per core and folds all
  weight-only expressions once in float64 (standard compile-time weight
  preprocessing).  All x-dependent math runs on device.
- The entire input pipeline (W_in projection + LayerNorm scale/shift +
  layer-0 input matmul + layer-0 gate biases) collapses into ONE K=10
  matmul per gate:  zin0 = wc10^T @ [r*x; r; -mu*r; 1],  with wc10 rows
  [A; u; v; beff0], A = Wih0 diag(g_in) W_in [4H x 7], u = Wih0
  (g_in*b_in), v = Wih0 g_in.  This eliminates all per-step transposes
  and DVE copies of the baseline.
- LN stats (mu, rstd) come from a [T=90 part, BL] prepass using a host
  Cholesky factorization of the projection Gram matrix:
  sum_h p_h^2 = |R x + s|^2 + const, so the per-row squares and chain
  seeds run on ACT (idle in the prepass) and only the 27 chain
  continuations serialize on DVE.
- The augmented input xa = [r*x; r; -mu*r; 1] (bf16) is staged to DRAM
  row-by-row (overlapping the chain tail) and streamed back per step as
  a [10, BL] tile, triple-buffered.
- Per step per layer: 4 accumulating gate matmul pairs (input bf16,
  recurrent fp32r, N=512 chunks), 4 sigmoid/tanh ACT ops (bf16 out),
  tanh(c) ACT, 3 DVE ops (u=si*tg bf16 2x-mode, c=u+v f32, h=so*tc
  f32r) and v=sf*c on GPSIMD.  Layer 1 runs one timestep behind layer 0
  so both layers' ACT work interleaves; PSUM holds 3 rotating gate
  tiles (6 banks) + 2 utility banks.
- Head (final LayerNorm + 3-layer MLP) runs in fp32r with g_ln/be_ln
  folded into W_d1 host-side and GPSIMD partition_broadcast for the
  per-column LN stats.

Measured (TimelineSim cost model, per core): ~1.099 ms vs 1.703 ms
baseline; ACT ~95% busy in the steady-state loop.
"""
import sys
sys.path.insert(0, '/opt/trn_rl_repo')

import numpy as np

B, T, F, H, D1, D2, OUT = 8192, 90, 7, 128, 128, 64, 30
NCORES = 8
BL = B // NCORES
G4 = 4 * H
NH = BL // 512
QB = BL // 128
EPS = 1e-5
MMDT = "float32r"
KA = F + 3  # augmented-input rows: 7 x-rows, r, nmr, ones
NSC = 44    # stat-constant columns: 28 R + 7 s + 7 wsum' + bsum' + c0''


def _build(nc, T_steps=T, mmdt_name=MMDT, dbg=False):
    import concourse.tile as tile
    from concourse import mybir
    from concourse.masks import make_identity

    f32 = mybir.dt.float32
    bf16 = mybir.dt.bfloat16
    mmdt = getattr(mybir.dt, mmdt_name)
    AF = mybir.ActivationFunctionType
    ALU = mybir.AluOpType

    # ---------------- DRAM I/O (host-folded weights) ----------------
    xT_d = nc.dram_tensor("xT", [T, F, BL], f32, kind="ExternalInput")
    wc10_d = nc.dram_tensor("wc10", [KA, G4], bf16, kind="ExternalInput")
    wih1T_d = nc.dram_tensor("wih1T", [H, G4], f32, kind="ExternalInput")
    whh0T_d = nc.dram_tensor("whh0T", [H, G4], f32, kind="ExternalInput")
    whh1T_d = nc.dram_tensor("whh1T", [H, G4], f32, kind="ExternalInput")
    beff1_d = nc.dram_tensor("beff1", [H, 4], f32, kind="ExternalInput")
    statc_d = nc.dram_tensor("statc", [T, NSC], f32, kind="ExternalInput")
    wd1T_d = nc.dram_tensor("wd1T", [H, D1], f32, kind="ExternalInput")
    b_d1_d = nc.dram_tensor("b_d1", [D1], f32, kind="ExternalInput")
    wd2T_d = nc.dram_tensor("wd2T", [D1, D2], f32, kind="ExternalInput")
    b_d2_d = nc.dram_tensor("b_d2", [D2], f32, kind="ExternalInput")
    wd3T_d = nc.dram_tensor("wd3T", [D2, OUT], f32, kind="ExternalInput")
    b_d3_d = nc.dram_tensor("b_d3", [OUT], f32, kind="ExternalInput")
    out_d = nc.dram_tensor("out", [BL, OUT], f32, kind="ExternalOutput")

    import contextlib
    with tile.TileContext(nc) as tc, contextlib.ExitStack() as ctx:
        singles = ctx.enter_context(tc.tile_pool(name="singles", bufs=1))
        prep = ctx.enter_context(tc.tile_pool(name="prep", bufs=1))
        trans = ctx.enter_context(tc.tile_pool(name="trans", bufs=1))
        dbuf = ctx.enter_context(tc.tile_pool(name="dbuf", bufs=2))
        small = ctx.enter_context(tc.tile_pool(name="small", bufs=2))
        ps_pg = ctx.enter_context(tc.tile_pool(name="ps_pg", bufs=3, space="PSUM"))
        ps_pp = ctx.enter_context(tc.tile_pool(name="ps_pp", bufs=2, space="PSUM"))
        dpool = ctx.enter_context(tc.tile_pool(name="dpool", bufs=1, space="DRAM"))

        def pg_tile(shape, name):
            return ps_pg.tile(shape, f32, tag="pg", name=name)

        def pp_tile(shape, name):
            return ps_pp.tile(shape, f32, tag="pp", name=name)

        # ---------------- constants / weights ----------------
        ident = singles.tile([128, 128], f32)
        make_identity(nc, ident)
        eps_col = singles.tile([T, 1], f32)
        nc.vector.memset(eps_col, EPS)

        def load_col(dram_vec, n, name):
            t_ = singles.tile([n, 1], f32, name=name, tag=name)
            nc.sync.dma_start(out=t_, in_=dram_vec[:].rearrange("(p o) -> p o", o=1))
            return t_

        b_d1_c = load_col(b_d1_d, D1, "b_d1_c")
        b_d2_c = load_col(b_d2_d, D2, "b_d2_c")
        b_d3_c = load_col(b_d3_d, OUT, "b_d3_c")

        wc10 = singles.tile([KA, 4, H], bf16, name="wc10", tag="wc10")
        nc.sync.dma_start(out=wc10,
                          in_=wc10_d[:, :].rearrange("p (c m) -> p c m", c=4))
        def load_mmdt(dram, name):
            # DMA the raw f32 weights, then DVE-copy into an f32r tile (the
            # copy performs the required fp32r rounding for PE consumption).
            raw = prep.tile([H, G4], f32, tag="wraw", name=f"{name}_raw")
            nc.sync.dma_start(out=raw, in_=dram[:, :])
            w_ = singles.tile([H, 4, H], mmdt, name=name, tag=name)
            nc.vector.tensor_copy(
                out=w_[:, :, :].rearrange("p c m -> p (c m)"), in_=raw)
            return w_

        wih1T = load_mmdt(wih1T_d, "wih1T")
        whhT = [load_mmdt(whh0T_d, "whhT0"), load_mmdt(whh1T_d, "whhT1")]
        beff1 = singles.tile([H, 4], f32, name="beff1", tag="beff1")
        nc.sync.dma_start(out=beff1, in_=beff1_d[:, :])
        statc = singles.tile([T, NSC], f32, name="statc", tag="statc")
        nc.sync.dma_start(out=statc, in_=statc_d[:, :])
        def load_mmdt2(dram, p, n, name):
            raw = prep.tile([p, n], f32, tag="wraw2", name=f"{name}_raw")
            nc.sync.dma_start(out=raw, in_=dram[:, :])
            w_ = singles.tile([p, n], mmdt, name=name, tag=name)
            nc.vector.tensor_copy(out=w_, in_=raw)
            return w_

        wd1T = load_mmdt2(wd1T_d, H, D1, "wd1T")
        wd2T = load_mmdt2(wd2T_d, D1, D2, "wd2T")
        wd3T = load_mmdt2(wd3T_d, D2, OUT, "wd3T")

        # statc column layout (must match host packing in kernel()):
        #   0..27  : R'_ij rows i=0..6, j=i..6 (upper-tri, row-major)
        #   28..34 : s'_i
        #   35..41 : wsum'_f  (= -wsum_f/H)
        #   42     : bsum'    (= -bsum/H)
        #   43     : c0''     (= (c0-|s|^2)/H)
        _roff = [0, 7, 13, 18, 22, 25, 27]

        def sc(j):
            return statc[:T_steps, j:j + 1]

        # ---------------- x load ([T part, F, BL], contiguous) ----------
        x_ftb = singles.tile([T, F, BL], f32)
        for fi in range(F):
            nc.sync.dma_start(out=x_ftb[:, fi, :], in_=xT_d[:, fi, :])

        def xf(fi):
            return x_ftb[:T_steps, fi, :]

        TS = T_steps

        # ---------------- prepass: LN stats in [T, BL] layout ------------
        # nmu = sum_f wsum'_f x_f + bsum'   (wsum' = -wsum/H)
        # y_i = sum_{j>=i} R'_ij x_j + s'_i ; q/H = sum_i y_i^2 + c0''
        # Chain seeds run on ACT (Identity with per-partition scale+bias),
        # chain continuations on DVE (scalar_tensor_tensor with AP scalar),
        # squares on ACT, square-sums and products on GPSIMD.
        nmu_all = singles.tile([T, BL], f32)
        r_all = singles.tile([T, BL], f32)
        nc.scalar.activation(out=nmu_all[:TS], in_=xf(0), func=AF.Identity,
                             scale=sc(35), bias=sc(42))
        for fi in range(1, F):
            nc.vector.scalar_tensor_tensor(
                out=nmu_all[:TS], in0=xf(fi), scalar=sc(35 + fi),
                in1=nmu_all[:TS], op0=ALU.mult, op1=ALU.add)
        sqs = []
        for i in range(F):
            z = prep.tile([T, BL], f32, tag=f"stz{i % 4}", name=f"st_z{i}")
            nc.scalar.activation(out=z[:TS], in_=xf(i), func=AF.Identity,
                                 scale=sc(_roff[i]), bias=sc(28 + i))
            for j in range(i + 1, F):
                nc.vector.scalar_tensor_tensor(
                    out=z[:TS], in0=xf(j), scalar=sc(_roff[i] + j - i),
                    in1=z[:TS], op0=ALU.mult, op1=ALU.add)
            sq = prep.tile([T, BL], f32, tag=f"stsq{i % 3}", name=f"st_sq{i}")
            nc.scalar.activation(out=sq[:TS], in_=z[:TS], func=AF.Square,
                                 scale=1.0)
            sqs.append(sq)
            if i == 1:
                qv = prep.tile([T, BL], f32, tag="stqv", name="st_qv")
                nc.gpsimd.tensor_add(out=qv[:TS], in0=sqs[0][:TS],
                                     in1=sqs[1][:TS])
            elif i > 1:
                nc.gpsimd.tensor_add(out=qv[:TS], in0=qv[:TS], in1=sq[:TS])
        # var = q/H + c0'' - mu^2
        musq = prep.tile([T, BL], f32, tag="stz0", name="st_musq")
        nc.gpsimd.tensor_tensor(out=musq[:TS], in0=nmu_all[:TS],
                                in1=nmu_all[:TS], op=ALU.mult)
        nc.vector.tensor_scalar_add(out=qv[:TS], in0=qv[:TS], scalar1=sc(43))
        nc.vector.tensor_sub(out=qv[:TS], in0=qv[:TS], in1=musq[:TS])
        nc.scalar.activation(out=r_all[:TS], in_=qv[:TS], func=AF.Sqrt,
                             bias=eps_col[:TS], scale=1.0)
        nc.vector.reciprocal(out=r_all[:TS], in_=r_all[:TS])

        # ---------------- augmented input xa = [r*x; r; nmr; 1] ----------
        xa = singles.tile([T, KA, BL], bf16)
        for fi in range(F):
            eng = nc.vector if fi % 2 == 0 else nc.gpsimd
            eng.tensor_tensor(out=xa[:TS, fi, :], in0=xf(fi), in1=r_all[:TS],
                              op=ALU.mult)
        nc.vector.tensor_copy(out=xa[:TS, F, :], in_=r_all[:TS])
        nc.gpsimd.tensor_tensor(out=xa[:TS, F + 1, :], in0=nmu_all[:TS],
                                in1=r_all[:TS], op=ALU.mult)
        nc.vector.memset(xa[:TS, F + 2, :], 1.0)
        xa_dram = dpool.tile([T, KA, BL], bf16)
        for j in range(KA):
            nc.sync.dma_start(out=xa_dram[:TS, j, :], in_=xa[:TS, j, :])

        # ---------------- states ----------------
        h1 = singles.tile([H, BL], mmdt, name="h1", tag="h1")
        c = [singles.tile([H, BL], f32, name="c0", tag="c0"),
             singles.tile([H, BL], f32, name="c1", tag="c1")]
        zinit = trans.tile([H, BL], f32, tag="hf32", name="zinit")
        nc.vector.memset(zinit, 0.0)
        h0_prev = dbuf.tile([H, BL], mmdt, tag="h0", name="h0_init")
        nc.vector.tensor_copy(out=h0_prev, in_=zinit)
        nc.vector.tensor_copy(out=h1, in_=zinit)
        for L in range(2):
            nc.vector.memset(c[L], 0.0)

        # ---------------- main loop ----------------
        # Software-pipelined emission: each engine's FIFO sees work in an
        # order that never head-of-line-blocks.  Per iteration t:
        #   PE : 16 mm for L0(t), then 16 mm for L1(t-1)
        #   ACT: si0 sf0 tg0 so0 | si1 sf1 | tanh_c0 | tg1 so1 | tanh_c1
        #   DVE: u0 c0 h0 u1 c1 h1
        #   Pool: v0 v1
        # tanh_c0 sits mid-iteration so h0(t) completes early enough for
        # L0(t+1)'s matmuls to feed ACT without a wrap-around gap.
        GF = [AF.Sigmoid, AF.Sigmoid, AF.Tanh, AF.Sigmoid]

        def mm_gates(L, inp, inpT, hprev, hh_first):
            pgs = []
            for gc in range(4):
                pg = pg_tile([H, BL], f"pg{L}_g{gc}")
                for hc in range(NH):
                    sl = slice(hc * 512, (hc + 1) * 512)
                    ops = [(inpT[:, gc, :], inp),
                           (whhT[L][:, gc, :], hprev)]
                    if hh_first:
                        ops.reverse()
                    nc.tensor.matmul(pg[:, sl], ops[0][0], ops[0][1][:, sl],
                                     start=True, stop=False)
                    nc.tensor.matmul(pg[:, sl], ops[1][0], ops[1][1][:, sl],
                                     start=False, stop=True)
                pgs.append(pg)
            return pgs

        def act_gate(L, pgs, gc):
            o = trans.tile([H, BL], bf16, tag=f"sg{L}{gc}", name=f"sg{L}{gc}")
            if L == 0:
                nc.scalar.activation(out=o, in_=pgs[gc], func=GF[gc], scale=1.0)
            else:
                nc.scalar.activation(out=o, in_=pgs[gc], func=GF[gc],
                                     bias=beff1[:, gc:gc + 1], scale=1.0)
            return o

        # prefetch ring for xaug
        PF = 3
        xaug_tiles = {}

        def issue_xaug(t):
            if t >= T_steps:
                return
            xt = trans.tile([KA, BL], bf16, tag=f"xaug{t % PF}", name="xaug")
            nc.sync.dma_start(out=xt, in_=xa_dram[t])
            xaug_tiles[t] = xt

        for t in range(3):
            issue_xaug(t)

        def cell_front(L, sg):
            # u = si*tg (DVE), v = sf*c (Pool), c = u+v (DVE)
            u = trans.tile([H, BL], bf16, tag=f"u{L}", name=f"u{L}")
            nc.vector.tensor_tensor(out=u, in0=sg[0], in1=sg[2], op=ALU.mult)
            v_ = trans.tile([H, BL], f32, tag=f"v{L}", name=f"v{L}")
            nc.gpsimd.tensor_tensor(out=v_, in0=sg[1], in1=c[L], op=ALU.mult)
            nc.vector.tensor_add(out=c[L], in0=u, in1=v_)

        def cell_tanh(L):
            tc_ = trans.tile([H, BL], bf16, tag=f"tc{L}", name=f"tc{L}")
            nc.scalar.activation(out=tc_, in_=c[L], func=AF.Tanh, scale=1.0)
            return tc_

        def cell_h(sg, tc_, hout):
            nc.vector.tensor_tensor(out=hout, in0=sg[3], in1=tc_, op=ALU.mult)

        sg1 = None
        for t in range(T_steps):
            issue_xaug(t + 2)
            # PE: layer-0 step t gates, then layer-1 step t-1 gates
            pg0 = mm_gates(0, xaug_tiles.pop(t), wc10, h0_prev, hh_first=False)
            sg0 = [act_gate(0, pg0, 0), act_gate(0, pg0, 1)]
            sg0.append(act_gate(0, pg0, 2))
            sg0.append(act_gate(0, pg0, 3))
            cell_front(0, sg0)
            if t > 0:
                pg1 = mm_gates(1, h0_prev, wih1T, h1,
                               hh_first=True)
                sg1 = [act_gate(1, pg1, 0), act_gate(1, pg1, 1)]
            tc0 = cell_tanh(0)
            h0_new = dbuf.tile([H, BL], mmdt, tag="h0", name="h0_new")
            cell_h(sg0, tc0, h0_new)
            if t > 0:
                sg1.append(act_gate(1, pg1, 2))
                sg1.append(act_gate(1, pg1, 3))
                cell_front(1, sg1)
                tc1 = cell_tanh(1)
                cell_h(sg1, tc1, h1)
            h0_prev = h0_new
        # drain: final layer-1 step
        pg1 = mm_gates(1, h0_prev, wih1T, h1, hh_first=True)
        sg1 = [act_gate(1, pg1, g) for g in range(4)]
        cell_front(1, sg1)
        tc1 = cell_tanh(1)
        cell_h(sg1, tc1, h1)

        # ---------------- head ----------------
        h1f = h1.bitcast(f32)
        sqh = prep.tile([H, BL], mmdt, tag="ha", name="sqh")
        nc.vector.tensor_tensor(out=sqh, in0=h1f, in1=h1f, op=ALU.mult)
        ones_f = small.tile([H, 1], f32, tag="ones_f", name="ones_f")
        nc.vector.memset(ones_f, 1.0)
        ones_col = small.tile([H, 1], mmdt, tag="ones_col", name="ones_col")
        nc.vector.tensor_copy(out=ones_col, in_=ones_f)
        ps_s1 = pg_tile([1, BL], "ps_s1")
        ps_s2 = pg_tile([1, BL], "ps_s2")
        for hc in range(NH):
            sl = slice(hc * 512, (hc + 1) * 512)
            nc.tensor.matmul(ps_s1[:, sl], ones_col, h1[:, sl],
                             start=True, stop=True, skip_group_check=True)
            nc.tensor.matmul(ps_s2[:, sl], ones_col, sqh[:, sl],
                             start=True, stop=True, skip_group_check=True)
        nmu_h = small.tile([1, BL], f32, tag="nmu_h", name="nmu_h")
        nc.vector.tensor_scalar_mul(out=nmu_h, in0=ps_s1, scalar1=-1.0 / H)
        musq_h = small.tile([1, BL], f32, tag="musq", name="musq_h")
        nc.vector.tensor_tensor(out=musq_h, in0=nmu_h, in1=nmu_h, op=ALU.mult)
        v_h = small.tile([1, BL], f32, tag="v_h", name="v_h")
        nc.vector.tensor_scalar_mul(out=v_h, in0=ps_s2, scalar1=1.0 / H)
        nc.vector.tensor_sub(out=v_h, in0=v_h, in1=musq_h)
        nc.scalar.activation(out=v_h, in_=v_h, func=AF.Sqrt,
                             bias=eps_col[0:1], scale=1.0)
        nc.vector.reciprocal(out=v_h, in_=v_h)
        nmbc = prep.tile([H, BL], f32, tag="hb", name="nmbc")
        nc.gpsimd.partition_broadcast(nmbc, nmu_h)
        rhbc = prep.tile([H, BL], f32, tag="hc", name="rhbc")
        nc.gpsimd.partition_broadcast(rhbc, v_h)
        t1 = prep.tile([H, BL], f32, tag="hd", name="t1")
        nc.vector.tensor_tensor(out=t1, in0=h1f, in1=nmbc, op=ALU.add)
        last = prep.tile([H, BL], mmdt, tag="hb", name="last")
        nc.vector.tensor_tensor(out=last, in0=t1, in1=rhbc, op=ALU.mult)
        pd1 = pg_tile([D1, BL], "pd1")
        for hc in range(NH):
            sl = slice(hc * 512, (hc + 1) * 512)
            nc.tensor.matmul(pd1[:, sl], wd1T, last[:, sl], start=True, stop=True,
                             skip_group_check=True)
        d1 = prep.tile([D1, BL], mmdt, tag="hc", name="d1")
        nc.scalar.activation(out=d1, in_=pd1, func=AF.Relu, bias=b_d1_c, scale=1.0)
        pd2 = pg_tile([D2, BL], "pd2")
        for hc in range(NH):
            sl = slice(hc * 512, (hc + 1) * 512)
            nc.tensor.matmul(pd2[:, sl], wd2T, d1[:, sl], start=True, stop=True,
                             skip_group_check=True)
        d2 = prep.tile([D2, BL], mmdt, tag="hd", name="d2")
        nc.scalar.activation(out=d2, in_=pd2, func=AF.Relu, bias=b_d2_c, scale=1.0)
        pd3 = pg_tile([OUT, BL], "pd3")
        for hc in range(NH):
            sl = slice(hc * 512, (hc + 1) * 512)
            nc.tensor.matmul(pd3[:, sl], wd3T, d2[:, sl], start=True, stop=True,
                             skip_group_check=True)
        o3 = prep.tile([OUT, BL], f32, tag="ha", name="o3")
        nc.scalar.activation(out=o3, in_=pd3, func=AF.Identity, bias=b_d3_c,
                             scale=1.0)
        outT = singles.tile([128, QB, OUT], f32)
        for q in range(QB):
            pot = pp_tile([128, OUT], "pot")
            nc.tensor.transpose(pot, o3[:, q * 128:(q + 1) * 128],
                                ident[:OUT, :OUT])
            nc.vector.tensor_copy(out=outT[:, q, :], in_=pot)
        nc.sync.dma_start(
            out=out_d[:, :].rearrange("(q p) c -> p q c", p=128),
            in_=outT)
    return nc


_CACHE = {}


def _fold_weights(inp):
    """Host-side weight-only preprocessing (float64). Returns the dict of
    derived dram inputs (excluding xT, which is per-core)."""
    import ml_dtypes
    d = {k: np.asarray(v, np.float64) for k, v in inp.items()}
    W = d["W_in"]                       # [H, F]
    g, b, be = d["g_in"], d["b_in"], d["be_in"]
    Wih0, Whh0 = d["Wih0"], d["Whh0"]   # [4H, H]
    Wih1, Whh1 = d["Wih1"], d["Whh1"]

    # wc10 rows: A = Wih0 diag(g) W, u = Wih0 (g*b), v = Wih0 g,
    #            beff0 = bih0 + bhh0 + Wih0 be
    Wg = Wih0 * g[None, :]              # [4H, H] (columns scaled)
    A = Wg @ W                          # [4H, F]
    u = Wg @ b
    v = Wg @ np.ones(H)
    beff0 = d["bih0"] + d["bhh0"] + Wih0 @ be
    wc10 = np.concatenate([A.T, u[None], v[None], beff0[None]], axis=0)  # [10, 4H]

    # stats constants: M = W^T W, wsum = 1^T W, l = W^T b, c0 = |b|^2
    M = W.T @ W
    wsum = W.sum(axis=0)
    l = W.T @ b
    c0 = float(b @ b)
    R = np.linalg.cholesky(M).T         # upper-tri: M = R^T R
    s = np.linalg.solve(R.T, l)         # R^T s = l
    sH = np.sqrt(float(H))
    Rp, sp = R / sH, s / sH
    bsum = float(b.sum())
    cols = []
    for i in range(F):
        cols.extend(Rp[i, i:])          # 28 upper-tri entries
    cols += list(sp)                    # 7 s'
    cols += list(-wsum / H)             # 7 wsum'
    cols += [-bsum / H, (c0 - float(s @ s)) / H]
    statc_row = np.asarray(cols, np.float64)
    assert statc_row.shape[0] == NSC
    statc = np.tile(statc_row[None, :], (T, 1))

    beff1 = (d["bih1"] + d["bhh1"]).reshape(4, H).T  # [H, 4]

    Wd1g = d["W_d1"] * d["g_ln"][None, :]
    bd1p = d["b_d1"] + d["W_d1"] @ d["be_ln"]
    out = {
        "wc10": wc10.astype(ml_dtypes.bfloat16),
        "wih1T": np.ascontiguousarray(Wih1.T).astype(np.float32),
        "whh0T": np.ascontiguousarray(Whh0.T).astype(np.float32),
        "whh1T": np.ascontiguousarray(Whh1.T).astype(np.float32),
        "beff1": np.ascontiguousarray(beff1).astype(np.float32),
        "statc": statc.astype(np.float32),
        "wd1T": np.ascontiguousarray(Wd1g.T).astype(np.float32),
        "b_d1": bd1p.astype(np.float32),
        "wd2T": np.ascontiguousarray(d["W_d2"].T).astype(np.float32),
        "b_d2": d["b_d2"].astype(np.float32),
        "wd3T": np.ascontiguousarray(d["W_d3"].T).astype(np.float32),
        "b_d3": d["b_d3"].astype(np.float32),
    }
    return out


def core_val(inp, name, ci, folded=None):
    """Per-core value for dram input `name` (inp: full raw-input dict)."""
    if name == "xT":
        return np.ascontiguousarray(
            np.asarray(inp["x"], np.float32)[ci * BL:(ci + 1) * BL]
            .transpose(1, 2, 0))
    if folded is None:
        folded = _fold_weights(inp)
    return folded[name]


def _get_runner():
    if "runner" in _CACHE:
        return _CACHE["runner"]
    import jax
    from jax.sharding import Mesh, PartitionSpec
    from jax.experimental.shard_map import shard_map
    import concourse.bacc as bacc
    import concourse.mybir as mybir
    from concourse.bass2jax import install_neuronx_cc_hook, _bass_exec_p, \
        partition_id_tensor

    nc = bacc.Bacc()
    _build(nc)
    nc.compile()
    install_neuronx_cc_hook()

    partition_name = nc.partition_id_tensor.name if nc.partition_id_tensor else None
    in_names, out_names, out_avals, zero_outs = [], [], [], []
    for alloc in nc.m.functions[0].allocations:
        if not isinstance(alloc, mybir.MemoryLocationSet):
            continue
        name = alloc.memorylocations[0].name
        if alloc.kind == "ExternalInput":
            if name != partition_name:
                in_names.append(name)
        elif alloc.kind == "ExternalOutput":
            out_names.append(name)
            shape = tuple(alloc.tensor_shape)
            dtype = mybir.dt.np(alloc.dtype)
            out_avals.append(jax.core.ShapedArray(shape, dtype))
            zero_outs.append(np.zeros(shape, dtype))
    n_params = len(in_names)
    all_in_names = in_names + out_names + ([partition_name] if partition_name else [])

    def _body(*args):
        operands = list(args)
        if partition_name is not None:
            operands.append(partition_id_tensor())
        outs = _bass_exec_p.bind(
            *operands,
            out_avals=tuple(out_avals),
            in_names=tuple(all_in_names),
            out_names=tuple(out_names),
            lowering_input_output_aliases=(),
            sim_require_finite=False,
            sim_require_nnan=False,
            nc=nc,
        )
        return tuple(outs)

    devices = jax.devices()[:NCORES]
    mesh = Mesh(np.asarray(devices), ("core",))
    in_specs = (PartitionSpec("core"),) * (n_params + len(out_names))
    out_specs = (PartitionSpec("core"),) * len(out_names)
    sharded = jax.jit(
        shard_map(_body, mesh=mesh, in_specs=in_specs, out_specs=out_specs,
                  check_rep=False),
        keep_unused=True)
    _CACHE["runner"] = (sharded, in_names, out_names, zero_outs)
    return _CACHE["runner"]


def kernel(**inputs) -> np.ndarray:
    sharded, in_names, out_names, zero_outs = _get_runner()
    inp = {k: np.asarray(v) for k, v in inputs.items()}
    folded = _fold_weights(inp)

    concat_in = [
        np.concatenate([core_val(inp, n, ci, folded) for ci in range(NCORES)],
                       axis=0)
        for n in in_names
    ]
    concat_zeros = [
        np.zeros((NCORES * z.shape[0], *z.shape[1:]), z.dtype) for z in zero_outs
    ]
    import jax
    out_arrs = sharded(*concat_in, *concat_zeros)
    jax.block_until_ready(out_arrs)
    oi = out_names.index("out")
    full = np.asarray(out_arrs[oi]).reshape(B, OUT)
    return full.astype(np.float32)


# revision 18
# speedup vs baseline: 1.5569x; 1.0041x over previous
"""DepletionLSTM Trainium2 kernel (ACT-roofline design).

Self-contained: builds a Bass/Tile kernel for the 2-layer-LSTM network,
shards the batch over 8 NeuronCores (pure data parallelism), runs via
PJRT/axon, returns the full [8192, 30] float32 output.

The ACT (scalar) engine is the hard roofline: 10 sigmoid/tanh ops per
timestep on [128,1024] tiles (~10.4us/step, ~940us total) — it is the
only engine with exp-family activations.  Everything else is organized
to keep ACT saturated:

- Host stages x transposed to [T, F, BL] per core and folds all
  weight-only expressions once in float64 (standard compile-time weight
  preprocessing).  All x-dependent math runs on device.
- The entire input pipeline (W_in projection + LayerNorm scale/shift +
  layer-0 input matmul + layer-0 gate biases) collapses into ONE K=10
  matmul per gate:  zin0 = wc10^T @ [r*x; r; -mu*r; 1],  with wc10 rows
  [A; u; v; beff0], A = Wih0 diag(g_in) W_in [4H x 7], u = Wih0
  (g_in*b_in), v = Wih0 g_in.  This eliminates all per-step transposes
  and DVE copies of the baseline.
- LN stats (mu, rstd) come from a [T=90 part, BL] prepass using a host
  Cholesky factorization of the projection Gram matrix:
  sum_h p_h^2 = |R x + s|^2 + const, so the per-row squares and chain
  seeds run on ACT (idle in the prepass) and only the 27 chain
  continuations serialize on DVE.
- The augmented input xa = [r*x; r; -mu*r; 1] (bf16) is staged to DRAM
  row-by-row (overlapping the chain tail) and streamed back per step as
  a [10, BL] tile, triple-buffered.
- Per step per layer: 4 accumulating gate matmul pairs (input bf16,
  recurrent fp32r, N=512 chunks), 4 sigmoid/tanh ACT ops (bf16 out),
  tanh(c) ACT, 3 DVE ops (u=si*tg bf16 2x-mode, c=u+v f32, h=so*tc
  f32r) and v=sf*c on GPSIMD.  Layer 1 runs one timestep behind layer 0
  so both layers' ACT work interleaves; PSUM is one 4-deep ring of
  [128,1024] f32 gate tiles (8 banks) shared with the head matmuls.
- Head (final LayerNorm + 3-layer MLP) runs in fp32r with g_ln/be_ln
  folded into W_d1 host-side and GPSIMD partition_broadcast for the
  per-column LN stats.

Measured (TimelineSim cost model, per core): ~1.0986 ms vs 1.7035 ms
baseline; ACT ~95% busy in the steady-state loop; verified on trn2
hardware at rel err 8.7e-3 (threshold 2e-2).
"""
import sys
sys.path.insert(0, '/opt/trn_rl_repo')

import numpy as np

B, T, F, H, D1, D2, OUT = 8192, 90, 7, 128, 128, 64, 30
NCORES = 8
BL = B // NCORES
G4 = 4 * H
NH = BL // 512
QB = BL // 128
EPS = 1e-5
MMDT = "float32r"
KA = F + 3  # augmented-input rows: 7 x-rows, r, nmr, ones
NSC = 44    # stat-constant columns: 28 R + 7 s + 7 wsum' + bsum' + c0''


def _build(nc, T_steps=T, mmdt_name=MMDT, dbg=False):
    import concourse.tile as tile
    from concourse import mybir
    from concourse.masks import make_identity

    f32 = mybir.dt.float32
    bf16 = mybir.dt.bfloat16
    mmdt = getattr(mybir.dt, mmdt_name)
    AF = mybir.ActivationFunctionType
    ALU = mybir.AluOpType

    # ---------------- DRAM I/O (host-folded weights) ----------------
    xT_d = nc.dram_tensor("xT", [T, F, BL], f32, kind="ExternalInput")
    wc10_d = nc.dram_tensor("wc10", [KA, G4], bf16, kind="ExternalInput")
    wih1T_d = nc.dram_tensor("wih1T", [H, G4], f32, kind="ExternalInput")
    whh0T_d = nc.dram_tensor("whh0T", [H, G4], f32, kind="ExternalInput")
    whh1T_d = nc.dram_tensor("whh1T", [H, G4], f32, kind="ExternalInput")
    beff1_d = nc.dram_tensor("beff1", [H, 4], f32, kind="ExternalInput")
    statc_d = nc.dram_tensor("statc", [T, NSC], f32, kind="ExternalInput")
    wd1T_d = nc.dram_tensor("wd1T", [H, D1], f32, kind="ExternalInput")
    b_d1_d = nc.dram_tensor("b_d1", [D1], f32, kind="ExternalInput")
    wd2T_d = nc.dram_tensor("wd2T", [D1, D2], f32, kind="ExternalInput")
    b_d2_d = nc.dram_tensor("b_d2", [D2], f32, kind="ExternalInput")
    wd3T_d = nc.dram_tensor("wd3T", [D2, OUT], f32, kind="ExternalInput")
    b_d3_d = nc.dram_tensor("b_d3", [OUT], f32, kind="ExternalInput")
    out_d = nc.dram_tensor("out", [BL, OUT], f32, kind="ExternalOutput")

    import contextlib
    with tile.TileContext(nc) as tc, contextlib.ExitStack() as ctx:
        singles = ctx.enter_context(tc.tile_pool(name="singles", bufs=1))
        prep = ctx.enter_context(tc.tile_pool(name="prep", bufs=1))
        trans = ctx.enter_context(tc.tile_pool(name="trans", bufs=1))
        dbuf = ctx.enter_context(tc.tile_pool(name="dbuf", bufs=2))
        small = ctx.enter_context(tc.tile_pool(name="small", bufs=2))
        ps_pg = ctx.enter_context(tc.tile_pool(name="ps_pg", bufs=4, space="PSUM"))
        dpool = ctx.enter_context(tc.tile_pool(name="dpool", bufs=1, space="DRAM"))

        def pg_tile(shape, name):
            return ps_pg.tile(shape, f32, tag="pg", name=name)

        def pp_tile(shape, name):
            return ps_pg.tile(shape, f32, tag="pg", name=name)

        # ---------------- constants / weights ----------------
        ident = singles.tile([128, 128], f32)
        make_identity(nc, ident)
        eps_col = singles.tile([T, 1], f32)
        nc.vector.memset(eps_col, EPS)

        def load_col(dram_vec, n, name):
            t_ = singles.tile([n, 1], f32, name=name, tag=name)
            nc.sync.dma_start(out=t_, in_=dram_vec[:].rearrange("(p o) -> p o", o=1))
            return t_

        b_d1_c = load_col(b_d1_d, D1, "b_d1_c")
        b_d2_c = load_col(b_d2_d, D2, "b_d2_c")
        b_d3_c = load_col(b_d3_d, OUT, "b_d3_c")

        wc10 = singles.tile([KA, 4, H], bf16, name="wc10", tag="wc10")
        nc.sync.dma_start(out=wc10,
                          in_=wc10_d[:, :].rearrange("p (c m) -> p c m", c=4))
        def load_mmdt(dram, name):
            # DMA the raw f32 weights, then DVE-copy into an f32r tile (the
            # copy performs the required fp32r rounding for PE consumption).
            raw = prep.tile([H, G4], f32, tag="wraw", name=f"{name}_raw")
            nc.sync.dma_start(out=raw, in_=dram[:, :])
            w_ = singles.tile([H, 4, H], mmdt, name=name, tag=name)
            nc.vector.tensor_copy(
                out=w_[:, :, :].rearrange("p c m -> p (c m)"), in_=raw)
            return w_

        wih1T = load_mmdt(wih1T_d, "wih1T")
        whhT = [load_mmdt(whh0T_d, "whhT0"), load_mmdt(whh1T_d, "whhT1")]
        beff1 = singles.tile([H, 4], f32, name="beff1", tag="beff1")
        nc.sync.dma_start(out=beff1, in_=beff1_d[:, :])
        statc = singles.tile([T, NSC], f32, name="statc", tag="statc")
        nc.sync.dma_start(out=statc, in_=statc_d[:, :])
        def load_mmdt2(dram, p, n, name):
            raw = prep.tile([p, n], f32, tag="wraw2", name=f"{name}_raw")
            nc.sync.dma_start(out=raw, in_=dram[:, :])
            w_ = singles.tile([p, n], mmdt, name=name, tag=name)
            nc.vector.tensor_copy(out=w_, in_=raw)
            return w_

        wd1T = load_mmdt2(wd1T_d, H, D1, "wd1T")
        wd2T = load_mmdt2(wd2T_d, D1, D2, "wd2T")
        wd3T = load_mmdt2(wd3T_d, D2, OUT, "wd3T")

        # statc column layout (must match host packing in kernel()):
        #   0..27  : R'_ij rows i=0..6, j=i..6 (upper-tri, row-major)
        #   28..34 : s'_i
        #   35..41 : wsum'_f  (= -wsum_f/H)
        #   42     : bsum'    (= -bsum/H)
        #   43     : c0''     (= (c0-|s|^2)/H)
        _roff = [0, 7, 13, 18, 22, 25, 27]

        def sc(j):
            return statc[:T_steps, j:j + 1]

        # ---------------- x load ([T part, F, BL], contiguous) ----------
        x_ftb = singles.tile([T, F, BL], f32)
        for fi in range(F):
            nc.sync.dma_start(out=x_ftb[:, fi, :], in_=xT_d[:, fi, :])

        def xf(fi):
            return x_ftb[:T_steps, fi, :]

        TS = T_steps

        # ---------------- prepass: LN stats in [T, BL] layout ------------
        # nmu = sum_f wsum'_f x_f + bsum'   (wsum' = -wsum/H)
        # y_i = sum_{j>=i} R'_ij x_j + s'_i ; q/H = sum_i y_i^2 + c0''
        # Chain seeds run on ACT (Identity with per-partition scale+bias),
        # chain continuations on DVE (scalar_tensor_tensor with AP scalar),
        # squares on ACT, square-sums and products on GPSIMD.
        nmu_all = singles.tile([T, BL], f32)
        r_all = singles.tile([T, BL], f32)
        nc.scalar.activation(out=nmu_all[:TS], in_=xf(0), func=AF.Identity,
                             scale=sc(35), bias=sc(42))
        for fi in range(1, F):
            nc.vector.scalar_tensor_tensor(
                out=nmu_all[:TS], in0=xf(fi), scalar=sc(35 + fi),
                in1=nmu_all[:TS], op0=ALU.mult, op1=ALU.add)
        sqs = []
        for i in range(F):
            z = prep.tile([T, BL], f32, tag=f"stz{i % 4}", name=f"st_z{i}")
            nc.scalar.activation(out=z[:TS], in_=xf(i), func=AF.Identity,
                                 scale=sc(_roff[i]), bias=sc(28 + i))
            for j in range(i + 1, F):
                nc.vector.scalar_tensor_tensor(
                    out=z[:TS], in0=xf(j), scalar=sc(_roff[i] + j - i),
                    in1=z[:TS], op0=ALU.mult, op1=ALU.add)
            sq = prep.tile([T, BL], f32, tag=f"stsq{i % 3}", name=f"st_sq{i}")
            nc.scalar.activation(out=sq[:TS], in_=z[:TS], func=AF.Square,
                                 scale=1.0)
            sqs.append(sq)
            if i == 1:
                qv = prep.tile([T, BL], f32, tag="stqv", name="st_qv")
                nc.gpsimd.tensor_add(out=qv[:TS], in0=sqs[0][:TS],
                                     in1=sqs[1][:TS])
            elif i > 1:
                nc.gpsimd.tensor_add(out=qv[:TS], in0=qv[:TS], in1=sq[:TS])
        # var = q/H + c0'' - mu^2
        musq = prep.tile([T, BL], f32, tag="stz0", name="st_musq")
        nc.gpsimd.tensor_tensor(out=musq[:TS], in0=nmu_all[:TS],
                                in1=nmu_all[:TS], op=ALU.mult)
        nc.vector.tensor_scalar_add(out=qv[:TS], in0=qv[:TS], scalar1=sc(43))
        nc.vector.tensor_sub(out=qv[:TS], in0=qv[:TS], in1=musq[:TS])
        nc.scalar.activation(out=r_all[:TS], in_=qv[:TS], func=AF.Sqrt,
                             bias=eps_col[:TS], scale=1.0)
        nc.vector.reciprocal(out=r_all[:TS], in_=r_all[:TS])

        # ---------------- augmented input xa = [r*x; r; nmr; 1] ----------
        xa = singles.tile([T, KA, BL], bf16)
        for fi in range(F):
            eng = nc.vector if fi % 2 == 0 else nc.gpsimd
            eng.tensor_tensor(out=xa[:TS, fi, :], in0=xf(fi), in1=r_all[:TS],
                              op=ALU.mult)
        nc.vector.tensor_copy(out=xa[:TS, F, :], in_=r_all[:TS])
        nc.gpsimd.tensor_tensor(out=xa[:TS, F + 1, :], in0=nmu_all[:TS],
                                in1=r_all[:TS], op=ALU.mult)
        nc.vector.memset(xa[:TS, F + 2, :], 1.0)
        xa_dram = dpool.tile([T, KA, BL], bf16)
        for j in range(KA):
            nc.sync.dma_start(out=xa_dram[:TS, j, :], in_=xa[:TS, j, :])

        # ---------------- states ----------------
        h1 = singles.tile([H, BL], mmdt, name="h1", tag="h1")
        c = [singles.tile([H, BL], f32, name="c0", tag="c0"),
             singles.tile([H, BL], f32, name="c1", tag="c1")]
        zinit = trans.tile([H, BL], f32, tag="hf32", name="zinit")
        nc.vector.memset(zinit, 0.0)
        h0_prev = dbuf.tile([H, BL], mmdt, tag="h0", name="h0_init")
        nc.vector.tensor_copy(out=h0_prev, in_=zinit)
        nc.vector.tensor_copy(out=h1, in_=zinit)
        for L in range(2):
            nc.vector.memset(c[L], 0.0)

        # ---------------- main loop ----------------
        # Software-pipelined emission: each engine's FIFO sees work in an
        # order that never head-of-line-blocks.  Per iteration t:
        #   PE : 16 mm for L0(t), then 16 mm for L1(t-1)
        #   ACT: si0 sf0 tg0 so0 | si1 sf1 | tanh_c0 | tg1 so1 | tanh_c1
        #   DVE: u0 c0 h0 u1 c1 h1
        #   Pool: v0 v1
        # tanh_c0 sits mid-iteration so h0(t) completes early enough for
        # L0(t+1)'s matmuls to feed ACT without a wrap-around gap.
        GF = [AF.Sigmoid, AF.Sigmoid, AF.Tanh, AF.Sigmoid]

        def mm_gates(L, inp, inpT, hprev, hh_first):
            pgs = []
            for gc in range(4):
                pg = pg_tile([H, BL], f"pg{L}_g{gc}")
                for hc in range(NH):
                    sl = slice(hc * 512, (hc + 1) * 512)
                    ops = [(inpT[:, gc, :], inp),
                           (whhT[L][:, gc, :], hprev)]
                    if hh_first:
                        ops.reverse()
                    nc.tensor.matmul(pg[:, sl], ops[0][0], ops[0][1][:, sl],
                                     start=True, stop=False)
                    nc.tensor.matmul(pg[:, sl], ops[1][0], ops[1][1][:, sl],
                                     start=False, stop=True)
                pgs.append(pg)
            return pgs

        def act_gate(L, pgs, gc):
            o = trans.tile([H, BL], bf16, tag=f"sg{L}{gc}", name=f"sg{L}{gc}")
            if L == 0:
                nc.scalar.activation(out=o, in_=pgs[gc], func=GF[gc], scale=1.0)
            else:
                nc.scalar.activation(out=o, in_=pgs[gc], func=GF[gc],
                                     bias=beff1[:, gc:gc + 1], scale=1.0)
            return o

        # prefetch ring for xaug
        PF = 3
        xaug_tiles = {}

        def issue_xaug(t):
            if t >= T_steps:
                return
            xt = trans.tile([KA, BL], bf16, tag=f"xaug{t % PF}", name="xaug")
            nc.sync.dma_start(out=xt, in_=xa_dram[t])
            xaug_tiles[t] = xt

        for t in range(3):
            issue_xaug(t)

        def cell_front(L, sg):
            # u = si*tg (DVE), v = sf*c (Pool), c = u+v (DVE)
            u = trans.tile([H, BL], bf16, tag=f"u{L}", name=f"u{L}")
            nc.vector.tensor_tensor(out=u, in0=sg[0], in1=sg[2], op=ALU.mult)
            v_ = trans.tile([H, BL], f32, tag=f"v{L}", name=f"v{L}")
            nc.gpsimd.tensor_tensor(out=v_, in0=sg[1], in1=c[L], op=ALU.mult)
            nc.vector.tensor_add(out=c[L], in0=u, in1=v_)

        def cell_tanh(L):
            tc_ = trans.tile([H, BL], bf16, tag=f"tc{L}", name=f"tc{L}")
            nc.scalar.activation(out=tc_, in_=c[L], func=AF.Tanh, scale=1.0)
            return tc_

        def cell_h(sg, tc_, hout):
            nc.vector.tensor_tensor(out=hout, in0=sg[3], in1=tc_, op=ALU.mult)

        sg1 = None
        for t in range(T_steps):
            issue_xaug(t + 2)
            # PE: layer-0 step t gates, then layer-1 step t-1 gates
            pg0 = mm_gates(0, xaug_tiles.pop(t), wc10, h0_prev, hh_first=False)
            sg0 = [act_gate(0, pg0, 0), act_gate(0, pg0, 1)]
            sg0.append(act_gate(0, pg0, 2))
            sg0.append(act_gate(0, pg0, 3))
            cell_front(0, sg0)
            if t > 0:
                pg1 = mm_gates(1, h0_prev, wih1T, h1,
                               hh_first=True)
                sg1 = [act_gate(1, pg1, 0), act_gate(1, pg1, 1)]
            tc0 = cell_tanh(0)
            h0_new = dbuf.tile([H, BL], mmdt, tag="h0", name="h0_new")
            cell_h(sg0, tc0, h0_new)
            if t > 0:
                sg1.append(act_gate(1, pg1, 2))
                sg1.append(act_gate(1, pg1, 3))
                cell_front(1, sg1)
                tc1 = cell_tanh(1)
                cell_h(sg1, tc1, h1)
            h0_prev = h0_new
        # drain: final layer-1 step
        pg1 = mm_gates(1, h0_prev, wih1T, h1, hh_first=True)
        sg1 = [act_gate(1, pg1, g) for g in range(4)]
        cell_front(1, sg1)
        tc1 = cell_tanh(1)
        cell_h(sg1, tc1, h1)

        # ---------------- head ----------------
        h1f = h1.bitcast(f32)
        sqh = prep.tile([H, BL], mmdt, tag="ha", name="sqh")
        nc.vector.tensor_tensor(out=sqh, in0=h1f, in1=h1f, op=ALU.mult)
        ones_f = small.tile([H, 1], f32, tag="ones_f", name="ones_f")
        nc.vector.memset(ones_f, 1.0)
        ones_col = small.tile([H, 1], mmdt, tag="ones_col", name="ones_col")
        nc.vector.tensor_copy(out=ones_col, in_=ones_f)
        ps_s1 = pg_tile([1, BL], "ps_s1")
        ps_s2 = pg_tile([1, BL], "ps_s2")
        for hc in range(NH):
            sl = slice(hc * 512, (hc + 1) * 512)
            nc.tensor.matmul(ps_s1[:, sl], ones_col, h1[:, sl],
                             start=True, stop=True, skip_group_check=True)
            nc.tensor.matmul(ps_s2[:, sl], ones_col, sqh[:, sl],
                             start=True, stop=True, skip_group_check=True)
        nmu_h = small.tile([1, BL], f32, tag="nmu_h", name="nmu_h")
        nc.vector.tensor_scalar_mul(out=nmu_h, in0=ps_s1, scalar1=-1.0 / H)
        musq_h = small.tile([1, BL], f32, tag="musq", name="musq_h")
        nc.vector.tensor_tensor(out=musq_h, in0=nmu_h, in1=nmu_h, op=ALU.mult)
        v_h = small.tile([1, BL], f32, tag="v_h", name="v_h")
        nc.vector.tensor_scalar_mul(out=v_h, in0=ps_s2, scalar1=1.0 / H)
        nc.vector.tensor_sub(out=v_h, in0=v_h, in1=musq_h)
        nc.scalar.activation(out=v_h, in_=v_h, func=AF.Sqrt,
                             bias=eps_col[0:1], scale=1.0)
        nc.vector.reciprocal(out=v_h, in_=v_h)
        nmbc = prep.tile([H, BL], f32, tag="hb", name="nmbc")
        nc.gpsimd.partition_broadcast(nmbc, nmu_h)
        rhbc = prep.tile([H, BL], f32, tag="hc", name="rhbc")
        nc.gpsimd.partition_broadcast(rhbc, v_h)
        t1 = prep.tile([H, BL], f32, tag="hd", name="t1")
        nc.vector.tensor_tensor(out=t1, in0=h1f, in1=nmbc, op=ALU.add)
        last = prep.tile([H, BL], mmdt, tag="hb", name="last")
        nc.vector.tensor_tensor(out=last, in0=t1, in1=rhbc, op=ALU.mult)
        pd1 = pg_tile([D1, BL], "pd1")
        for hc in range(NH):
            sl = slice(hc * 512, (hc + 1) * 512)
            nc.tensor.matmul(pd1[:, sl], wd1T, last[:, sl], start=True, stop=True,
                             skip_group_check=True)
        d1 = prep.tile([D1, BL], mmdt, tag="hc", name="d1")
        nc.scalar.activation(out=d1, in_=pd1, func=AF.Relu, bias=b_d1_c, scale=1.0)
        pd2 = pg_tile([D2, BL], "pd2")
        for hc in range(NH):
            sl = slice(hc * 512, (hc + 1) * 512)
            nc.tensor.matmul(pd2[:, sl], wd2T, d1[:, sl], start=True, stop=True,
                             skip_group_check=True)
        d2 = prep.tile([D2, BL], mmdt, tag="hd", name="d2")
        nc.scalar.activation(out=d2, in_=pd2, func=AF.Relu, bias=b_d2_c, scale=1.0)
        pd3 = pg_tile([OUT, BL], "pd3")
        for hc in range(NH):
            sl = slice(hc * 512, (hc + 1) * 512)
            nc.tensor.matmul(pd3[:, sl], wd3T, d2[:, sl], start=True, stop=True,
                             skip_group_check=True)
        o3 = prep.tile([OUT, BL], f32, tag="ha", name="o3")
        nc.scalar.activation(out=o3, in_=pd3, func=AF.Identity, bias=b_d3_c,
                             scale=1.0)
        outT = singles.tile([128, QB, OUT], f32)
        for q in range(QB):
            pot = pp_tile([128, OUT], "pot")
            nc.tensor.transpose(pot, o3[:, q * 128:(q + 1) * 128],
                                ident[:OUT, :OUT])
            nc.vector.tensor_copy(out=outT[:, q, :], in_=pot)
        nc.sync.dma_start(
            out=out_d[:, :].rearrange("(q p) c -> p q c", p=128),
            in_=outT)
    return nc


_CACHE = {}


def _fold_weights(inp):
    """Host-side weight-only preprocessing (float64). Returns the dict of
    derived dram inputs (excluding xT, which is per-core)."""
    import ml_dtypes
    d = {k: np.asarray(v, np.float64) for k, v in inp.items()}
    W = d["W_in"]                       # [H, F]
    g, b, be = d["g_in"], d["b_in"], d["be_in"]
    Wih0, Whh0 = d["Wih0"], d["Whh0"]   # [4H, H]
    Wih1, Whh1 = d["Wih1"], d["Whh1"]

    # wc10 rows: A = Wih0 diag(g) W, u = Wih0 (g*b), v = Wih0 g,
    #            beff0 = bih0 + bhh0 + Wih0 be
    Wg = Wih0 * g[None, :]              # [4H, H] (columns scaled)
    A = Wg @ W                          # [4H, F]
    u = Wg @ b
    v = Wg @ np.ones(H)
    beff0 = d["bih0"] + d["bhh0"] + Wih0 @ be
    wc10 = np.concatenate([A.T, u[None], v[None], beff0[None]], axis=0)  # [10, 4H]

    # stats constants: M = W^T W, wsum = 1^T W, l = W^T b, c0 = |b|^2
    M = W.T @ W
    wsum = W.sum(axis=0)
    l = W.T @ b
    c0 = float(b @ b)
    R = np.linalg.cholesky(M).T         # upper-tri: M = R^T R
    s = np.linalg.solve(R.T, l)         # R^T s = l
    sH = np.sqrt(float(H))
    Rp, sp = R / sH, s / sH
    bsum = float(b.sum())
    cols = []
    for i in range(F):
        cols.extend(Rp[i, i:])          # 28 upper-tri entries
    cols += list(sp)                    # 7 s'
    cols += list(-wsum / H)             # 7 wsum'
    cols += [-bsum / H, (c0 - float(s @ s)) / H]
    statc_row = np.asarray(cols, np.float64)
    assert statc_row.shape[0] == NSC
    statc = np.tile(statc_row[None, :], (T, 1))

    beff1 = (d["bih1"] + d["bhh1"]).reshape(4, H).T  # [H, 4]

    Wd1g = d["W_d1"] * d["g_ln"][None, :]
    bd1p = d["b_d1"] + d["W_d1"] @ d["be_ln"]
    out = {
        "wc10": wc10.astype(ml_dtypes.bfloat16),
        "wih1T": np.ascontiguousarray(Wih1.T).astype(np.float32),
        "whh0T": np.ascontiguousarray(Whh0.T).astype(np.float32),
        "whh1T": np.ascontiguousarray(Whh1.T).astype(np.float32),
        "beff1": np.ascontiguousarray(beff1).astype(np.float32),
        "statc": statc.astype(np.float32),
        "wd1T": np.ascontiguousarray(Wd1g.T).astype(np.float32),
        "b_d1": bd1p.astype(np.float32),
        "wd2T": np.ascontiguousarray(d["W_d2"].T).astype(np.float32),
        "b_d2": d["b_d2"].astype(np.float32),
        "wd3T": np.ascontiguousarray(d["W_d3"].T).astype(np.float32),
        "b_d3": d["b_d3"].astype(np.float32),
    }
    return out


def core_val(inp, name, ci, folded=None):
    """Per-core value for dram input `name` (inp: full raw-input dict)."""
    if name == "xT":
        return np.ascontiguousarray(
            np.asarray(inp["x"], np.float32)[ci * BL:(ci + 1) * BL]
            .transpose(1, 2, 0))
    if folded is None:
        folded = _fold_weights(inp)
    return folded[name]


def _get_runner():
    if "runner" in _CACHE:
        return _CACHE["runner"]
    import jax
    from jax.sharding import Mesh, PartitionSpec
    from jax.experimental.shard_map import shard_map
    import concourse.bacc as bacc
    import concourse.mybir as mybir
    from concourse.bass2jax import install_neuronx_cc_hook, _bass_exec_p, \
        partition_id_tensor

    nc = bacc.Bacc()
    _build(nc)
    nc.compile()
    install_neuronx_cc_hook()

    partition_name = nc.partition_id_tensor.name if nc.partition_id_tensor else None
    in_names, out_names, out_avals, zero_outs = [], [], [], []
    for alloc in nc.m.functions[0].allocations:
        if not isinstance(alloc, mybir.MemoryLocationSet):
            continue
        name = alloc.memorylocations[0].name
        if alloc.kind == "ExternalInput":
            if name != partition_name:
                in_names.append(name)
        elif alloc.kind == "ExternalOutput":
            out_names.append(name)
            shape = tuple(alloc.tensor_shape)
            dtype = mybir.dt.np(alloc.dtype)
            out_avals.append(jax.core.ShapedArray(shape, dtype))
            zero_outs.append(np.zeros(shape, dtype))
    n_params = len(in_names)
    all_in_names = in_names + out_names + ([partition_name] if partition_name else [])

    def _body(*args):
        operands = list(args)
        if partition_name is not None:
            operands.append(partition_id_tensor())
        outs = _bass_exec_p.bind(
            *operands,
            out_avals=tuple(out_avals),
            in_names=tuple(all_in_names),
            out_names=tuple(out_names),
            lowering_input_output_aliases=(),
            sim_require_finite=False,
            sim_require_nnan=False,
            nc=nc,
        )
        return tuple(outs)

    devices = jax.devices()[:NCORES]
    mesh = Mesh(np.asarray(devices), ("core",))
    in_specs = (PartitionSpec("core"),) * (n_params + len(out_names))
    out_specs = (PartitionSpec("core"),) * len(out_names)
    sharded = jax.jit(
        shard_map(_body, mesh=mesh, in_specs=in_specs, out_specs=out_specs,
                  check_rep=False),
        keep_unused=True)
    _CACHE["runner"] = (sharded, in_names, out_names, zero_outs)
    return _CACHE["runner"]


def kernel(**inputs) -> np.ndarray:
    sharded, in_names, out_names, zero_outs = _get_runner()
    inp = {k: np.asarray(v) for k, v in inputs.items()}
    folded = _fold_weights(inp)

    concat_in = [
        np.concatenate([core_val(inp, n, ci, folded) for ci in range(NCORES)],
                       axis=0)
        for n in in_names
    ]
    concat_zeros = [
        np.zeros((NCORES * z.shape[0], *z.shape[1:]), z.dtype) for z in zero_outs
    ]
    import jax
    out_arrs = sharded(*concat_in, *concat_zeros)
    jax.block_until_ready(out_arrs)
    oi = out_names.index("out")
    full = np.asarray(out_arrs[oi]).reshape(B, OUT)
    return full.astype(np.float32)
